# revision 23
# baseline (speedup 1.0000x reference)
"""Linformer self-attention on 8 Trainium2 NeuronCores.

Problem (hardcoded shapes): x [4,4096,1024] f32; per batch:
  q = scale*(x@Wq); kv = x@Wkv; keys/values compressed 4096->256 via
  proj_k/proj_v; 16-head attention (dh=64, k=256); out @ Wproj + bproj.

Sharding: 8 cores = 4 batches x 2 head-groups (8 heads / 512 cols each).
Each core computes a partial [4096,1024] output (Wproj row-split); host
sums the pair and adds bias.

Per-core dataflow (all matmuls use out = lhsT.T @ rhs, K<=128 partitions):
  A : xcxvT[1024,512] = x.T @ [proj_k|proj_v]      (contract n, x natural)
  A2: kprojT[512,256] = Wk_g.T @ xcT ; vproj[256,512] = xvT.T @ Wv_g
  B : qT[512,4096] = Wq_g.T @ xT    (xT provided by host, plain DMA)
  C : per (head,fc k-chunk): scoresT[128,512] -> exp (Act) -> pexp bf16
  S : per (n-chunk, head): sums[n,1] = pexp.T @ ones  (N=1 matmuls)
  D : po[n, 8*64] = pexp.T @ vproj_h per head; normalize via DVE
      tensor_tensor with per-head recip broadcast -> o bf16
  T : oT via one batched DMA transpose per [128,512] tile
  E : out[n,1024] = oT.T-chunks @ Wproj_g, bf16 store via gpsimd SWDGE

B(nb+1), E(nb-1), D(nb) are hand-interleaved in the PE stream per
n-block so Act exp latency hides under PE matmuls.
"""

import os
import numpy as np

import concourse.bass as bass
import concourse.mybir as mybir
import concourse.tile as tile
from concourse import bacc
from concourse.bass_utils import run_bass_kernel_spmd

P = 128
N, D, K, DG, DH = 4096, 1024, 256, 512, 64
NB = 8                    # n-blocks of 512
HL = 8                    # heads per core
F32 = mybir.dt.float32

MMDT_NAME = os.environ.get("LINF_MMDT", "bfloat16")
MMDT = getattr(mybir.dt, MMDT_NAME)
Exp = mybir.ActivationFunctionType.Exp

_cache = {}


def build_nc():
    nc = bacc.Bacc(None, target_bir_lowering=False, debug=False)

    x_d = nc.dram_tensor("x", [N, D], MMDT, kind="ExternalInput")
    xt_d = nc.dram_tensor("xt", [D, N], MMDT, kind="ExternalInput")
    pkv_d = nc.dram_tensor("projkv", [N, 2 * K], MMDT, kind="ExternalInput")
    wq_d = nc.dram_tensor("wq", [D, DG], MMDT, kind="ExternalInput")
    wk_d = nc.dram_tensor("wk", [D, DG], MMDT, kind="ExternalInput")
    wv_d = nc.dram_tensor("wv", [D, DG], MMDT, kind="ExternalInput")
    wp_d = nc.dram_tensor("wproj", [DG, D], MMDT, kind="ExternalInput")
    out_d = nc.dram_tensor("out", [N, D], MMDT, kind="ExternalOutput")

    with tile.TileContext(nc) as tc:
        from contextlib import ExitStack
        with ExitStack() as ctx:
            res = ctx.enter_context(tc.tile_pool(name="res", bufs=1))
            ones_sb = res.tile([P, 1], MMDT, tag="ones")
            nc.vector.memset(ones_sb[:], 1.0)

            wq_sb = res.tile([P, 8 * DG], MMDT, tag="wq")
            wk_sb = res.tile([P, 8 * DG], MMDT, tag="wk")
            wv_sb = res.tile([P, 8 * DG], MMDT, tag="wv")
            wproj_sb = res.tile([P, 4 * D], MMDT, tag="wproj")
            kprojT_sb = res.tile([P, 4 * K], MMDT, tag="kprojT")
            vproj_sb = res.tile([P, 2 * DG], MMDT, tag="vproj")
            xcxv_sb = res.tile([P, 8 * 2 * K], MMDT, tag="xcxv")

            # rolling pools for the merged loop
            xtp = ctx.enter_context(tc.tile_pool(name="xtp", bufs=4))
            qtp = ctx.enter_context(tc.tile_pool(name="qtp", bufs=2))
            pexp_p = ctx.enter_context(tc.tile_pool(name="pexp", bufs=2))
            op_ = ctx.enter_context(tc.tile_pool(name="op", bufs=8))
            otp = ctx.enter_context(tc.tile_pool(name="otp", bufs=8))
            outp = ctx.enter_context(tc.tile_pool(name="outp", bufs=3))
            rcp = ctx.enter_context(tc.tile_pool(name="rcp", bufs=2))

            def load_w(dst, src, nchunk, w):
                # dst[p, c*w + j] = src[c*128 + p, j]
                nc.sync.dma_start(
                    out=dst[:].rearrange("p (c j) -> p c j", c=nchunk),
                    in_=src[:, :].rearrange("(c p) j -> p c j", p=P))

            def load_xt(nb):
                xt = xtp.tile([P, 8 * DG], MMDT, tag="xt", name=f"xt{nb}")
                nc.sync.dma_start(
                    out=xt[:].rearrange("p (d j) -> p d j", d=8),
                    in_=xt_d[:, nb * DG:(nb + 1) * DG]
                        .rearrange("(d p) j -> p d j", p=P))
                return xt

            # ---------------- Phase A ----------------
            with ExitStack() as actx:
                xin = actx.enter_context(tc.tile_pool(name="xin", bufs=3))
                pa_ctx = ExitStack()
                pa = pa_ctx.enter_context(tc.tile_pool(name="pa", bufs=1, space="PSUM"))
                accs = [pa.tile([P, 2 * K], F32, tag=f"pa{dd}", name=f"pa{dd}")
                        for dd in range(8)]
                # First chunk loads alone (small, fast) so PE starts ASAP;
                # all weight/xt loads go after the 8 batches — the DMA
                # transfer path is a serial resource and phase A is tight.
                for b4 in range(8):
                    if b4 == 0:
                        x4 = xin.tile([P, 4 * D], MMDT, tag="x4")
                        kv4 = xin.tile([P, 4 * 2 * K], MMDT, tag="kv4")
                        nc.sync.dma_start(out=x4[:, :D], in_=x_d[0:P, :])
                        nc.sync.dma_start(out=kv4[:, :2 * K], in_=pkv_d[0:P, :])
                        nc.sync.dma_start(
                            out=x4[:, D:].rearrange("p (c j) -> p c j", c=3),
                            in_=x_d[P:512, :].rearrange("(c p) j -> p c j", p=P))
                        nc.sync.dma_start(
                            out=kv4[:, 2 * K:].rearrange("p (c j) -> p c j", c=3),
                            in_=pkv_d[P:512, :].rearrange("(c p) j -> p c j", p=P))
                    elif b4 == 1:
                        # 2+2 split: first half arrives before the PE (still
                        # in p-state ramp) finishes batch 0
                        x4 = xin.tile([P, 4 * D], MMDT, tag="x4")
                        kv4 = xin.tile([P, 4 * 2 * K], MMDT, tag="kv4")
                        for hf in range(2):
                            r0 = 512 + hf * 256
                            nc.sync.dma_start(
                                out=x4[:, hf * 2 * D:(hf + 1) * 2 * D]
                                    .rearrange("p (c j) -> p c j", c=2),
                                in_=x_d[r0:r0 + 256, :]
                                    .rearrange("(c p) j -> p c j", p=P))
                            nc.sync.dma_start(
                                out=kv4[:, hf * 4 * K:(hf + 1) * 4 * K]
                                    .rearrange("p (c j) -> p c j", c=2),
                                in_=pkv_d[r0:r0 + 256, :]
                                    .rearrange("(c p) j -> p c j", p=P))
                    else:
                        x4 = xin.tile([P, 4 * D], MMDT, tag="x4")
                        kv4 = xin.tile([P, 4 * 2 * K], MMDT, tag="kv4")
                        nc.sync.dma_start(
                            out=x4[:].rearrange("p (c j) -> p c j", c=4),
                            in_=x_d[b4 * 512:(b4 + 1) * 512, :]
                                .rearrange("(c p) j -> p c j", p=P))
                        nc.sync.dma_start(
                            out=kv4[:].rearrange("p (c j) -> p c j", c=4),
                            in_=pkv_d[b4 * 512:(b4 + 1) * 512, :]
                                .rearrange("(c p) j -> p c j", p=P))
                    for c in range(4):
                        nn = b4 * 4 + c
                        for dd in range(8):
                            nc.tensor.matmul(
                                accs[dd][:],
                                lhsT=x4[:, c * D + dd * P: c * D + (dd + 1) * P],
                                rhs=kv4[:, c * 2 * K:(c + 1) * 2 * K],
                                start=(nn == 0), stop=(nn == 31))
                # ordered by first use: wk/wv (A2), wq+xt0 (B prologue),
                # wproj (E(0)), xt1 (B(1))
                load_w(wk_sb, wk_d, 8, DG)
                load_w(wv_sb, wv_d, 8, DG)
                load_w(wq_sb, wq_d, 8, DG)
                xt_tiles = {0: load_xt(0)}
                load_w(wproj_sb, wp_d, 4, D)
                xt_tiles[1] = load_xt(1)
                for dd in range(8):
                    eng = nc.vector if dd % 2 else nc.scalar
                    if dd % 2:
                        nc.vector.tensor_copy(
                            xcxv_sb[:, dd * 2 * K:(dd + 1) * 2 * K], accs[dd][:])
                    else:
                        nc.scalar.copy(
                            out=xcxv_sb[:, dd * 2 * K:(dd + 1) * 2 * K],
                            in_=accs[dd][:])

                # Phase A2 — release the A accumulators' banks first
                pa_ctx.close()
                pa2 = actx.enter_context(tc.tile_pool(name="pa2", bufs=4, space="PSUM"))
                for jc in range(4):
                    acc = pa2.tile([P, K], F32, tag="kpj")
                    for dd in range(8):
                        nc.tensor.matmul(
                            acc[:],
                            lhsT=wk_sb[:, dd * DG + jc * P: dd * DG + (jc + 1) * P],
                            rhs=xcxv_sb[:, dd * 2 * K: dd * 2 * K + K],
                            start=(dd == 0), stop=(dd == 7))
                    if jc % 2:
                        nc.scalar.copy(out=kprojT_sb[:, jc * K:(jc + 1) * K],
                                       in_=acc[:])
                    else:
                        nc.vector.tensor_copy(kprojT_sb[:, jc * K:(jc + 1) * K],
                                              acc[:])
                for fc in range(2):
                    acc2 = pa2.tile([P, DG], F32, tag="vpj")
                    for dd in range(8):
                        nc.tensor.matmul(
                            acc2[:],
                            lhsT=xcxv_sb[:, dd * 2 * K + K + fc * P:
                                         dd * 2 * K + K + (fc + 1) * P],
                            rhs=wv_sb[:, dd * DG:(dd + 1) * DG],
                            start=(dd == 0), stop=(dd == 7))
                    if fc:
                        nc.scalar.copy(out=vproj_sb[:, fc * DG:(fc + 1) * DG],
                                       in_=acc2[:])
                    else:
                        nc.vector.tensor_copy(vproj_sb[:, fc * DG:(fc + 1) * DG],
                                              acc2[:])

            # ---------------- merged loop pools (PSUM) ----------------
            scp = ctx.enter_context(tc.tile_pool(name="scp", bufs=2, space="PSUM"))
            accp = ctx.enter_context(tc.tile_pool(name="accp", bufs=3, space="PSUM"))
            pop = ctx.enter_context(tc.tile_pool(name="pop", bufs=2, space="PSUM"))
            smp = ctx.enter_context(tc.tile_pool(name="smp", bufs=1, space="PSUM"))

            def b_block(xt, qt, jc):
                accq = accp.tile([P, DG], F32, tag="acc")
                for dd in range(8):
                    nc.tensor.matmul(
                        accq[:],
                        lhsT=wq_sb[:, dd * DG + jc * P: dd * DG + (jc + 1) * P],
                        rhs=xt[:, dd * DG:(dd + 1) * DG],
                        start=(dd == 0), stop=(dd == 7))
                nc.vector.tensor_copy(qt[:, jc * DG:(jc + 1) * DG], accq[:])

            def sc_block(qt, h, pexps):
                jc, p0 = h // 2, (h % 2) * DH
                for fc in range(2):
                    st = scp.tile([P, DG], F32, tag="sc")
                    nc.tensor.matmul(
                        st[:],
                        lhsT=kprojT_sb[p0:p0 + DH,
                                       jc * K + fc * P: jc * K + (fc + 1) * P],
                        rhs=qt[p0:p0 + DH, jc * DG:(jc + 1) * DG],
                        start=True, stop=True)
                    pexp = pexp_p.tile([P, DG], MMDT, tag=f"px{h}_{fc}")
                    nc.scalar.activation(pexp[:], st[:], Exp)
                    pexps[(h, fc)] = pexp

            def d_group(nb, nn2, pexps, sp, recips):
                po = pop.tile([P, DG], F32, tag="po")
                for h in range(HL):
                    for fc in range(2):
                        px = pexps[(h, fc)]
                        nc.tensor.matmul(
                            po[:, h * DH:(h + 1) * DH],
                            lhsT=px[:, nn2 * P:(nn2 + 1) * P],
                            rhs=vproj_sb[:, fc * DG + h * DH:
                                         fc * DG + (h + 1) * DH],
                            start=(fc == 0), stop=(fc == 1))
                        nc.tensor.matmul(
                            sp[:, nn2 * HL + h: nn2 * HL + h + 1],
                            lhsT=px[:, nn2 * P:(nn2 + 1) * P],
                            rhs=ones_sb[:],
                            start=(fc == 0), stop=(fc == 1))
                nc.vector.reciprocal(
                    recips[:, nn2 * HL:(nn2 + 1) * HL],
                    sp[:, nn2 * HL:(nn2 + 1) * HL])
                o_t = op_.tile([P, DG], MMDT, tag="o", name=f"o{nb}_{nn2}")
                nc.vector.tensor_tensor(
                    out=o_t[:].rearrange("p (h j) -> p h j", h=HL),
                    in0=po[:].rearrange("p (h j) -> p h j", h=HL),
                    in1=recips[:, nn2 * HL:(nn2 + 1) * HL]
                        .broadcast_to([P, HL, DH]),
                    op=mybir.AluOpType.mult)
                ot = otp.tile([P, DG], MMDT, tag="ot", name=f"ot{nb}_{nn2}")
                nc.scalar.dma_start_transpose(
                    out=ot[:].rearrange("p (c j) -> p c j", c=4),
                    in_=o_t[:])
                return ot

            def e_group(nb, nn2, ot):
                ci = nb * 4 + nn2
                outsb = outp.tile([P, D], MMDT, tag="outsb")
                for half in range(2):
                    pe_acc = accp.tile([P, DG], F32, tag="acc")
                    for jc2 in range(4):
                        nc.tensor.matmul(
                            pe_acc[:],
                            lhsT=ot[:, jc2 * P:(jc2 + 1) * P],
                            rhs=wproj_sb[:, jc2 * D + half * DG:
                                         jc2 * D + (half + 1) * DG],
                            start=(jc2 == 0), stop=(jc2 == 3))
                    nc.vector.tensor_copy(
                        outsb[:, half * DG:(half + 1) * DG], pe_acc[:])
                nc.gpsimd.dma_start(out=out_d[ci * P:(ci + 1) * P, :],
                                    in_=outsb[:])

            # ---------------- prologue: B(0) ----------------
            qts = {0: qtp.tile([P, 4 * DG], MMDT, tag="qt", name="qt0")}
            for jc in range(4):
                b_block(xt_tiles[0], qts[0], jc)

            # ---------------- merged loop ----------------
            xt_tiles[2] = load_xt(2)
            prev_ots = None
            for nb in range(NB):
                if nb + 3 < NB:
                    xt_tiles[nb + 3] = load_xt(nb + 3)
                pexps = {}
                cur_ots = []
                sp = smp.tile([P, 4 * HL], F32, tag="sums")
                recips = rcp.tile([P, 4 * HL], F32, tag="recips")
                have_b = nb + 1 < NB
                if have_b:
                    qts[nb + 1] = qtp.tile([P, 4 * DG], MMDT, tag="qt",
                                           name=f"qt{nb + 1}")
                # interleave scores(nb) with B(nb+1) on the PE stream
                for h in range(HL):
                    sc_block(qts[nb], h, pexps)
                    if have_b and h % 2 == 1 and h // 2 < 4:
                        b_block(xt_tiles[nb + 1], qts[nb + 1], h // 2)
                # interleave E(nb-1) with D(nb) so the po recycle chain
                # (recip -> tensor_tensor -> transpose) hides under E matmuls.
                # Last iter (no B work) keeps E fully before D: the exps
                # aren't done early enough to start D sooner anyway.
                if prev_ots is not None:
                    if have_b:
                        for nn2 in range(4):
                            e_group(nb - 1, nn2, prev_ots[nn2])
                            cur_ots.append(d_group(nb, nn2, pexps, sp, recips))
                    else:
                        for nn2 in range(4):
                            e_group(nb - 1, nn2, prev_ots[nn2])
                        for nn2 in range(4):
                            cur_ots.append(d_group(nb, nn2, pexps, sp, recips))
                else:
                    for nn2 in range(4):
                        cur_ots.append(d_group(nb, nn2, pexps, sp, recips))
                prev_ots = cur_ots
            for nn2 in range(4):
                e_group(NB - 1, nn2, prev_ots[nn2])
    nc.compile()
    return nc


def _np_mm(a):
    return np.ascontiguousarray(np.asarray(a), dtype=mybir.dt.np(MMDT))


def kernel(x, Wq, Wkv, Wproj, bproj, proj_k, proj_v):
    x = np.asarray(x)
    Wq, Wkv, Wproj = np.asarray(Wq), np.asarray(Wkv), np.asarray(Wproj)
    bproj, proj_k, proj_v = np.asarray(bproj), np.asarray(proj_k), np.asarray(proj_v)

    if "nc" not in _cache:
        _cache["nc"] = build_nc()
    nc = _cache["nc"]

    scale = np.float32(DH ** -0.5)
    projkv = _np_mm(np.concatenate([proj_k, proj_v], axis=1))
    in_maps = []
    for c in range(8):
        b, g = c // 2, c % 2
        cols = slice(g * DG, (g + 1) * DG)
        xb = _np_mm(x[b])
        in_maps.append({
            "x": xb,
            "xt": np.ascontiguousarray(xb.T),
            "projkv": projkv,
            "wq": _np_mm(scale * Wq[:, cols]),
            "wk": _np_mm(Wkv[:, :D][:, cols]),
            "wv": _np_mm(Wkv[:, D:][:, cols]),
            "wproj": _np_mm(Wproj[cols, :]),
        })
    res = run_bass_kernel_spmd(nc, in_maps, list(range(8)),
                               trace=bool(os.environ.get("LINF_TRACE")))
    _cache["last_result"] = res
    outs = [np.asarray(r["out"], dtype=np.float32) for r in res.results]
    full = np.stack([outs[2 * b] + outs[2 * b + 1] for b in range(4)])
    full = full + np.asarray(bproj, np.float32)
    return full.astype(np.float32)


# revision 24
# speedup vs baseline: 1.2238x; 1.2238x over previous
"""Linformer self-attention on 8 Trainium2 NeuronCores.

Problem (hardcoded shapes): x [4,4096,1024] f32; per batch:
  q = scale*(x@Wq); kv = x@Wkv; keys/values compressed 4096->256 via
  proj_k/proj_v; 16-head attention (dh=64, k=256); out @ Wproj + bproj.

Sharding: 8 cores = 4 batches x 2 head-groups (8 heads / 512 cols each).
Each core computes a partial [4096,1024] output (Wproj row-split); host
sums the pair and adds bias.

Per-core dataflow (all matmuls use out = lhsT.T @ rhs, K<=128 partitions):
  A : xcxvT[1024,512] = x.T @ [proj_k|proj_v]      (contract n, x natural)
  A2: kprojT[512,256] = Wk_g.T @ xcT ; vproj[256,512] = xvT.T @ Wv_g
  B : qT[512,4096] = Wq_g.T @ xT    (xT provided by host, plain DMA)
  C : per (head,fc k-chunk): scoresT[128,512] -> exp (Act) -> pexp bf16
  S : per (n-chunk, head): sums[n,1] = pexp.T @ ones  (N=1 matmuls)
  D : po[n, 8*64] = pexp.T @ vproj_h per head; normalize via DVE
      tensor_tensor with per-head recip broadcast -> o bf16
  T : oT via one batched DMA transpose per [128,512] tile
  E : out[n,1024] = oT.T-chunks @ Wproj_g, bf16 store via gpsimd SWDGE

B(nb+1), E(nb-1), D(nb) are hand-interleaved in the PE stream per
n-block so Act exp latency hides under PE matmuls.
"""

import os
import numpy as np

import concourse.bass as bass
import concourse.mybir as mybir
import concourse.tile as tile
from concourse import bacc
from concourse.bass_utils import run_bass_kernel_spmd

P = 128
N, D, K, DG, DH = 4096, 1024, 256, 512, 64
NB = 8                    # n-blocks of 512
HL = 8                    # heads per core
F32 = mybir.dt.float32

MMDT_NAME = os.environ.get("LINF_MMDT", "bfloat16")
MMDT = getattr(mybir.dt, MMDT_NAME)
Exp = mybir.ActivationFunctionType.Exp

_cache = {}


def build_nc():
    nc = bacc.Bacc(None, target_bir_lowering=False, debug=False)

    x_d = nc.dram_tensor("x", [N, D], MMDT, kind="ExternalInput")
    xt_d = nc.dram_tensor("xt", [D, N], MMDT, kind="ExternalInput")
    pkv_d = nc.dram_tensor("projkv", [N, 2 * K], MMDT, kind="ExternalInput")
    wq_d = nc.dram_tensor("wq", [D, DG], MMDT, kind="ExternalInput")
    wk_d = nc.dram_tensor("wk", [D, DG], MMDT, kind="ExternalInput")
    wv_d = nc.dram_tensor("wv", [D, DG], MMDT, kind="ExternalInput")
    wp_d = nc.dram_tensor("wproj", [DG, D], MMDT, kind="ExternalInput")
    out_d = nc.dram_tensor("out", [N, D], MMDT, kind="ExternalOutput")

    with tile.TileContext(nc) as tc:
        from contextlib import ExitStack
        with ExitStack() as ctx:
            res = ctx.enter_context(tc.tile_pool(name="res", bufs=1))
            ones_sb = res.tile([P, 1], MMDT, tag="ones")
            nc.vector.memset(ones_sb[:], 1.0)

            wq_sb = res.tile([P, 8 * DG], MMDT, tag="wq")
            wk_sb = res.tile([P, 8 * DG], MMDT, tag="wk")
            wv_sb = res.tile([P, 8 * DG], MMDT, tag="wv")
            wproj_sb = res.tile([P, 4 * D], MMDT, tag="wproj")
            kprojT_sb = res.tile([P, 4 * K], MMDT, tag="kprojT")
            vproj_sb = res.tile([P, 2 * DG], MMDT, tag="vproj")
            xcxv_sb = res.tile([P, 8 * 2 * K], MMDT, tag="xcxv")

            # rolling pools for the merged loop
            xtp = ctx.enter_context(tc.tile_pool(name="xtp", bufs=4))
            qtp = ctx.enter_context(tc.tile_pool(name="qtp", bufs=2))
            pexp_p = ctx.enter_context(tc.tile_pool(name="pexp", bufs=2))
            op_ = ctx.enter_context(tc.tile_pool(name="op", bufs=8))
            otp = ctx.enter_context(tc.tile_pool(name="otp", bufs=8))
            outp = ctx.enter_context(tc.tile_pool(name="outp", bufs=3))
            rcp = ctx.enter_context(tc.tile_pool(name="rcp", bufs=2))

            def load_w(dst, src, nchunk, w):
                # dst[p, c*w + j] = src[c*128 + p, j]
                nc.sync.dma_start(
                    out=dst[:].rearrange("p (c j) -> p c j", c=nchunk),
                    in_=src[:, :].rearrange("(c p) j -> p c j", p=P))

            def load_xt(nb):
                xt = xtp.tile([P, 8 * DG], MMDT, tag="xt", name=f"xt{nb}")
                nc.sync.dma_start(
                    out=xt[:].rearrange("p (d j) -> p d j", d=8),
                    in_=xt_d[:, nb * DG:(nb + 1) * DG]
                        .rearrange("(d p) j -> p d j", p=P))
                return xt

            # ---------------- Phase A ----------------
            with ExitStack() as actx:
                xin = actx.enter_context(tc.tile_pool(name="xin", bufs=3))
                pa_ctx = ExitStack()
                pa = pa_ctx.enter_context(tc.tile_pool(name="pa", bufs=1, space="PSUM"))
                accs = [pa.tile([P, 2 * K], F32, tag=f"pa{dd}", name=f"pa{dd}")
                        for dd in range(8)]
                # First chunk loads alone (small, fast) so PE starts ASAP;
                # all weight/xt loads go after the 8 batches — the DMA
                # transfer path is a serial resource and phase A is tight.
                for b4 in range(8):
                    if b4 == 0:
                        x4 = xin.tile([P, 4 * D], MMDT, tag="x4")
                        kv4 = xin.tile([P, 4 * 2 * K], MMDT, tag="kv4")
                        nc.sync.dma_start(out=x4[:, :D], in_=x_d[0:P, :])
                        nc.sync.dma_start(out=kv4[:, :2 * K], in_=pkv_d[0:P, :])
                        nc.sync.dma_start(
                            out=x4[:, D:].rearrange("p (c j) -> p c j", c=3),
                            in_=x_d[P:512, :].rearrange("(c p) j -> p c j", p=P))
                        nc.sync.dma_start(
                            out=kv4[:, 2 * K:].rearrange("p (c j) -> p c j", c=3),
                            in_=pkv_d[P:512, :].rearrange("(c p) j -> p c j", p=P))
                    elif b4 == 1:
                        # 2+2 split: first half arrives before the PE (still
                        # in p-state ramp) finishes batch 0
                        x4 = xin.tile([P, 4 * D], MMDT, tag="x4")
                        kv4 = xin.tile([P, 4 * 2 * K], MMDT, tag="kv4")
                        for hf in range(2):
                            r0 = 512 + hf * 256
                            nc.sync.dma_start(
                                out=x4[:, hf * 2 * D:(hf + 1) * 2 * D]
                                    .rearrange("p (c j) -> p c j", c=2),
                                in_=x_d[r0:r0 + 256, :]
                                    .rearrange("(c p) j -> p c j", p=P))
                            nc.sync.dma_start(
                                out=kv4[:, hf * 4 * K:(hf + 1) * 4 * K]
                                    .rearrange("p (c j) -> p c j", c=2),
                                in_=pkv_d[r0:r0 + 256, :]
                                    .rearrange("(c p) j -> p c j", p=P))
                    else:
                        x4 = xin.tile([P, 4 * D], MMDT, tag="x4")
                        kv4 = xin.tile([P, 4 * 2 * K], MMDT, tag="kv4")
                        nc.sync.dma_start(
                            out=x4[:].rearrange("p (c j) -> p c j", c=4),
                            in_=x_d[b4 * 512:(b4 + 1) * 512, :]
                                .rearrange("(c p) j -> p c j", p=P))
                        nc.sync.dma_start(
                            out=kv4[:].rearrange("p (c j) -> p c j", c=4),
                            in_=pkv_d[b4 * 512:(b4 + 1) * 512, :]
                                .rearrange("(c p) j -> p c j", p=P))
                    for c in range(4):
                        nn = b4 * 4 + c
                        for dd in range(8):
                            nc.tensor.matmul(
                                accs[dd][:],
                                lhsT=x4[:, c * D + dd * P: c * D + (dd + 1) * P],
                                rhs=kv4[:, c * 2 * K:(c + 1) * 2 * K],
                                start=(nn == 0), stop=(nn == 31))
                # ordered by first use: wk/wv (A2), wq+xt0 (B prologue),
                # wproj (E(0)), xt1 (B(1))
                load_w(wk_sb, wk_d, 8, DG)
                load_w(wv_sb, wv_d, 8, DG)
                load_w(wq_sb, wq_d, 8, DG)
                xt_tiles = {0: load_xt(0)}
                load_w(wproj_sb, wp_d, 4, D)
                xt_tiles[1] = load_xt(1)
                for dd in range(8):
                    eng = nc.vector if dd % 2 else nc.scalar
                    if dd % 2:
                        nc.vector.tensor_copy(
                            xcxv_sb[:, dd * 2 * K:(dd + 1) * 2 * K], accs[dd][:])
                    else:
                        nc.scalar.copy(
                            out=xcxv_sb[:, dd * 2 * K:(dd + 1) * 2 * K],
                            in_=accs[dd][:])

                # Phase A2 — release the A accumulators' banks first
                pa_ctx.close()
                pa2 = actx.enter_context(tc.tile_pool(name="pa2", bufs=4, space="PSUM"))
                for jc in range(4):
                    acc = pa2.tile([P, K], F32, tag="kpj")
                    for dd in range(8):
                        nc.tensor.matmul(
                            acc[:],
                            lhsT=wk_sb[:, dd * DG + jc * P: dd * DG + (jc + 1) * P],
                            rhs=xcxv_sb[:, dd * 2 * K: dd * 2 * K + K],
                            start=(dd == 0), stop=(dd == 7))
                    if jc % 2:
                        nc.scalar.copy(out=kprojT_sb[:, jc * K:(jc + 1) * K],
                                       in_=acc[:])
                    else:
                        nc.vector.tensor_copy(kprojT_sb[:, jc * K:(jc + 1) * K],
                                              acc[:])
                for fc in range(2):
                    acc2 = pa2.tile([P, DG], F32, tag="vpj")
                    for dd in range(8):
                        nc.tensor.matmul(
                            acc2[:],
                            lhsT=xcxv_sb[:, dd * 2 * K + K + fc * P:
                                         dd * 2 * K + K + (fc + 1) * P],
                            rhs=wv_sb[:, dd * DG:(dd + 1) * DG],
                            start=(dd == 0), stop=(dd == 7))
                    if fc:
                        nc.scalar.copy(out=vproj_sb[:, fc * DG:(fc + 1) * DG],
                                       in_=acc2[:])
                    else:
                        nc.vector.tensor_copy(vproj_sb[:, fc * DG:(fc + 1) * DG],
                                              acc2[:])

            # ---------------- merged loop pools (PSUM) ----------------
            scp = ctx.enter_context(tc.tile_pool(name="scp", bufs=2, space="PSUM"))
            accp = ctx.enter_context(tc.tile_pool(name="accp", bufs=3, space="PSUM"))
            pop = ctx.enter_context(tc.tile_pool(name="pop", bufs=2, space="PSUM"))
            smp = ctx.enter_context(tc.tile_pool(name="smp", bufs=1, space="PSUM"))

            def b_block(xt, qt, jc):
                accq = accp.tile([P, DG], F32, tag="acc")
                for dd in range(8):
                    nc.tensor.matmul(
                        accq[:],
                        lhsT=wq_sb[:, dd * DG + jc * P: dd * DG + (jc + 1) * P],
                        rhs=xt[:, dd * DG:(dd + 1) * DG],
                        start=(dd == 0), stop=(dd == 7))
                nc.vector.tensor_copy(qt[:, jc * DG:(jc + 1) * DG], accq[:])

            def sc_block(qt, h, pexps):
                jc, p0 = h // 2, (h % 2) * DH
                for fc in range(2):
                    st = scp.tile([P, DG], F32, tag="sc")
                    nc.tensor.matmul(
                        st[:],
                        lhsT=kprojT_sb[p0:p0 + DH,
                                       jc * K + fc * P: jc * K + (fc + 1) * P],
                        rhs=qt[p0:p0 + DH, jc * DG:(jc + 1) * DG],
                        start=True, stop=True)
                    pexp = pexp_p.tile([P, DG], MMDT, tag=f"px{h}_{fc}")
                    nc.scalar.activation(pexp[:], st[:], Exp)
                    pexps[(h, fc)] = pexp

            def d_group(nb, nn2, pexps, sp, recips):
                po = pop.tile([P, DG], F32, tag="po")
                for h in range(HL):
                    for fc in range(2):
                        px = pexps[(h, fc)]
                        nc.tensor.matmul(
                            po[:, h * DH:(h + 1) * DH],
                            lhsT=px[:, nn2 * P:(nn2 + 1) * P],
                            rhs=vproj_sb[:, fc * DG + h * DH:
                                         fc * DG + (h + 1) * DH],
                            start=(fc == 0), stop=(fc == 1))
                        nc.tensor.matmul(
                            sp[:, nn2 * HL + h: nn2 * HL + h + 1],
                            lhsT=px[:, nn2 * P:(nn2 + 1) * P],
                            rhs=ones_sb[:],
                            start=(fc == 0), stop=(fc == 1))
                nc.vector.reciprocal(
                    recips[:, nn2 * HL:(nn2 + 1) * HL],
                    sp[:, nn2 * HL:(nn2 + 1) * HL])
                o_t = op_.tile([P, DG], MMDT, tag="o", name=f"o{nb}_{nn2}")
                nc.vector.tensor_tensor(
                    out=o_t[:].rearrange("p (h j) -> p h j", h=HL),
                    in0=po[:].rearrange("p (h j) -> p h j", h=HL),
                    in1=recips[:, nn2 * HL:(nn2 + 1) * HL]
                        .broadcast_to([P, HL, DH]),
                    op=mybir.AluOpType.mult)
                ot = otp.tile([P, DG], MMDT, tag="ot", name=f"ot{nb}_{nn2}")
                nc.sync.dma_start_transpose(
                    out=ot[:].rearrange("p (c j) -> p c j", c=4),
                    in_=o_t[:])
                return ot

            def e_group(nb, nn2, ot):
                ci = nb * 4 + nn2
                outsb = outp.tile([P, D], MMDT, tag="outsb")
                for half in range(2):
                    pe_acc = accp.tile([P, DG], F32, tag="acc")
                    for jc2 in range(4):
                        nc.tensor.matmul(
                            pe_acc[:],
                            lhsT=ot[:, jc2 * P:(jc2 + 1) * P],
                            rhs=wproj_sb[:, jc2 * D + half * DG:
                                         jc2 * D + (half + 1) * DG],
                            start=(jc2 == 0), stop=(jc2 == 3))
                    nc.vector.tensor_copy(
                        outsb[:, half * DG:(half + 1) * DG], pe_acc[:])
                nc.gpsimd.dma_start(out=out_d[ci * P:(ci + 1) * P, :],
                                    in_=outsb[:])

            # ---------------- prologue: B(0) ----------------
            qts = {0: qtp.tile([P, 4 * DG], MMDT, tag="qt", name="qt0")}
            for jc in range(4):
                b_block(xt_tiles[0], qts[0], jc)

            # ---------------- merged loop ----------------
            xt_tiles[2] = load_xt(2)
            prev_ots = None
            for nb in range(NB):
                if nb + 3 < NB:
                    xt_tiles[nb + 3] = load_xt(nb + 3)
                pexps = {}
                cur_ots = []
                sp = smp.tile([P, 4 * HL], F32, tag="sums")
                recips = rcp.tile([P, 4 * HL], F32, tag="recips")
                have_b = nb + 1 < NB
                if have_b:
                    qts[nb + 1] = qtp.tile([P, 4 * DG], MMDT, tag="qt",
                                           name=f"qt{nb + 1}")
                # interleave scores(nb) with B(nb+1) on the PE stream
                for h in range(HL):
                    sc_block(qts[nb], h, pexps)
                    if have_b and h % 2 == 1 and h // 2 < 4:
                        b_block(xt_tiles[nb + 1], qts[nb + 1], h // 2)
                # interleave E(nb-1) with D(nb) so the po recycle chain
                # (recip -> tensor_tensor -> transpose) hides under E matmuls.
                # Last iter (no B work) keeps E fully before D: the exps
                # aren't done early enough to start D sooner anyway.
                if prev_ots is not None:
                    if have_b:
                        for nn2 in range(4):
                            e_group(nb - 1, nn2, prev_ots[nn2])
                            cur_ots.append(d_group(nb, nn2, pexps, sp, recips))
                    else:
                        for nn2 in range(4):
                            e_group(nb - 1, nn2, prev_ots[nn2])
                        for nn2 in range(4):
                            cur_ots.append(d_group(nb, nn2, pexps, sp, recips))
                else:
                    for nn2 in range(4):
                        cur_ots.append(d_group(nb, nn2, pexps, sp, recips))
                prev_ots = cur_ots
            for nn2 in range(4):
                e_group(NB - 1, nn2, prev_ots[nn2])
    nc.compile()
    return nc


def _np_mm(a):
    return np.ascontiguousarray(np.asarray(a), dtype=mybir.dt.np(MMDT))


def kernel(x, Wq, Wkv, Wproj, bproj, proj_k, proj_v):
    x = np.asarray(x)
    Wq, Wkv, Wproj = np.asarray(Wq), np.asarray(Wkv), np.asarray(Wproj)
    bproj, proj_k, proj_v = np.asarray(bproj), np.asarray(proj_k), np.asarray(proj_v)

    if "nc" not in _cache:
        _cache["nc"] = build_nc()
    nc = _cache["nc"]

    scale = np.float32(DH ** -0.5)
    projkv = _np_mm(np.concatenate([proj_k, proj_v], axis=1))
    in_maps = []
    for c in range(8):
        b, g = c // 2, c % 2
        cols = slice(g * DG, (g + 1) * DG)
        xb = _np_mm(x[b])
        in_maps.append({
            "x": xb,
            "xt": np.ascontiguousarray(xb.T),
            "projkv": projkv,
            "wq": _np_mm(scale * Wq[:, cols]),
            "wk": _np_mm(Wkv[:, :D][:, cols]),
            "wv": _np_mm(Wkv[:, D:][:, cols]),
            "wproj": _np_mm(Wproj[cols, :]),
        })
    res = run_bass_kernel_spmd(nc, in_maps, list(range(8)),
                               trace=bool(os.environ.get("LINF_TRACE")))
    _cache["last_result"] = res
    outs = [np.asarray(r["out"], dtype=np.float32) for r in res.results]
    full = np.stack([outs[2 * b] + outs[2 * b + 1] for b in range(4)])
    full = full + np.asarray(bproj, np.float32)
    return full.astype(np.float32)


# revision 25
# speedup vs baseline: 1.2524x; 1.0234x over previous
"""Linformer self-attention on 8 Trainium2 NeuronCores.

Problem (hardcoded shapes): x [4,4096,1024] f32; per batch:
  q = scale*(x@Wq); kv = x@Wkv; keys/values compressed 4096->256 via
  proj_k/proj_v; 16-head attention (dh=64, k=256); out @ Wproj + bproj.

Sharding: 8 cores = 4 batches x 2 head-groups (8 heads / 512 cols each).
Each core computes a partial [4096,1024] output (Wproj row-split); host
sums the pair and adds bias.

Per-core dataflow (all matmuls use out = lhsT.T @ rhs, K<=128 partitions):
  A : xcxvT[1024,512] = x.T @ [proj_k|proj_v]      (contract n, x natural)
  A2: kprojT[512,256] = Wk_g.T @ xcT ; vproj[256,512] = xvT.T @ Wv_g
  B : qT[512,4096] = Wq_g.T @ xT    (xT provided by host, plain DMA)
  C : per (head,fc k-chunk): scoresT[128,512] -> exp (Act) -> pexp bf16
  S : per (n-chunk, head): sums[n,1] = pexp.T @ ones  (N=1 matmuls)
  D : po[n, 8*64] = pexp.T @ vproj_h per head; normalize via DVE
      tensor_tensor with per-head recip broadcast -> o bf16
  T : oT via one batched DMA transpose per [128,512] tile
  E : out[n,1024] = oT.T-chunks @ Wproj_g, bf16 store via gpsimd SWDGE

B(nb+1), E(nb-1), D(nb) are hand-interleaved in the PE stream per
n-block so Act exp latency hides under PE matmuls.
"""

import os
import numpy as np

import concourse.bass as bass
import concourse.mybir as mybir
import concourse.tile as tile
from concourse import bacc
from concourse.bass_utils import run_bass_kernel_spmd

P = 128
N, D, K, DG, DH = 4096, 1024, 256, 512, 64
NB = 8                    # n-blocks of 512
HL = 8                    # heads per core
F32 = mybir.dt.float32

MMDT_NAME = os.environ.get("LINF_MMDT", "bfloat16")
MMDT = getattr(mybir.dt, MMDT_NAME)
Exp = mybir.ActivationFunctionType.Exp

_cache = {}


def build_nc():
    nc = bacc.Bacc(None, target_bir_lowering=False, debug=False)

    x_d = nc.dram_tensor("x", [N, D], MMDT, kind="ExternalInput")
    xt_d = nc.dram_tensor("xt", [D, N], MMDT, kind="ExternalInput")
    pkv_d = nc.dram_tensor("projkv", [N, 2 * K], MMDT, kind="ExternalInput")
    wq_d = nc.dram_tensor("wq", [D, DG], MMDT, kind="ExternalInput")
    wk_d = nc.dram_tensor("wk", [D, DG], MMDT, kind="ExternalInput")
    wv_d = nc.dram_tensor("wv", [D, DG], MMDT, kind="ExternalInput")
    wp_d = nc.dram_tensor("wproj", [DG, D], MMDT, kind="ExternalInput")
    out_d = nc.dram_tensor("out", [N, D], MMDT, kind="ExternalOutput")

    with tile.TileContext(nc) as tc:
        from contextlib import ExitStack
        with ExitStack() as ctx:
            res = ctx.enter_context(tc.tile_pool(name="res", bufs=1))
            ones_sb = res.tile([P, 1], MMDT, tag="ones")
            nc.vector.memset(ones_sb[:], 1.0)

            wq_sb = res.tile([P, 8 * DG], MMDT, tag="wq")
            wk_sb = res.tile([P, 8 * DG], MMDT, tag="wk")
            wv_sb = res.tile([P, 8 * DG], MMDT, tag="wv")
            wproj_sb = res.tile([P, 4 * D], MMDT, tag="wproj")
            kprojT_sb = res.tile([P, 4 * K], MMDT, tag="kprojT")
            vproj_sb = res.tile([P, 2 * DG], MMDT, tag="vproj")
            xcxv_sb = res.tile([P, 8 * 2 * K], MMDT, tag="xcxv")

            # rolling pools for the merged loop
            xtp = ctx.enter_context(tc.tile_pool(name="xtp", bufs=4))
            qtp = ctx.enter_context(tc.tile_pool(name="qtp", bufs=2))
            pexp_p = ctx.enter_context(tc.tile_pool(name="pexp", bufs=2))
            op_ = ctx.enter_context(tc.tile_pool(name="op", bufs=8))
            otp = ctx.enter_context(tc.tile_pool(name="otp", bufs=8))
            outp = ctx.enter_context(tc.tile_pool(name="outp", bufs=3))
            rcp = ctx.enter_context(tc.tile_pool(name="rcp", bufs=2))

            def load_w(dst, src, nchunk, w):
                # dst[p, c*w + j] = src[c*128 + p, j]
                nc.sync.dma_start(
                    out=dst[:].rearrange("p (c j) -> p c j", c=nchunk),
                    in_=src[:, :].rearrange("(c p) j -> p c j", p=P))

            def load_xt(nb):
                xt = xtp.tile([P, 8 * DG], MMDT, tag="xt", name=f"xt{nb}")
                nc.sync.dma_start(
                    out=xt[:].rearrange("p (d j) -> p d j", d=8),
                    in_=xt_d[:, nb * DG:(nb + 1) * DG]
                        .rearrange("(d p) j -> p d j", p=P))
                return xt

            # ---------------- Phase A ----------------
            with ExitStack() as actx:
                xin = actx.enter_context(tc.tile_pool(name="xin", bufs=3))
                pa_ctx = ExitStack()
                pa = pa_ctx.enter_context(tc.tile_pool(name="pa", bufs=1, space="PSUM"))
                accs = [pa.tile([P, 2 * K], F32, tag=f"pa{dd}", name=f"pa{dd}")
                        for dd in range(8)]
                # First chunk loads alone (small, fast) so PE starts ASAP;
                # all weight/xt loads go after the 8 batches — the DMA
                # transfer path is a serial resource and phase A is tight.
                for b4 in range(8):
                    if b4 == 0:
                        x4 = xin.tile([P, 4 * D], MMDT, tag="x4")
                        kv4 = xin.tile([P, 4 * 2 * K], MMDT, tag="kv4")
                        nc.sync.dma_start(out=x4[:, :D], in_=x_d[0:P, :])
                        nc.sync.dma_start(out=kv4[:, :2 * K], in_=pkv_d[0:P, :])
                        nc.sync.dma_start(
                            out=x4[:, D:].rearrange("p (c j) -> p c j", c=3),
                            in_=x_d[P:512, :].rearrange("(c p) j -> p c j", p=P))
                        nc.sync.dma_start(
                            out=kv4[:, 2 * K:].rearrange("p (c j) -> p c j", c=3),
                            in_=pkv_d[P:512, :].rearrange("(c p) j -> p c j", p=P))
                    elif b4 == 1:
                        # 2+2 split: first half arrives before the PE (still
                        # in p-state ramp) finishes batch 0
                        x4 = xin.tile([P, 4 * D], MMDT, tag="x4")
                        kv4 = xin.tile([P, 4 * 2 * K], MMDT, tag="kv4")
                        for hf in range(2):
                            r0 = 512 + hf * 256
                            nc.sync.dma_start(
                                out=x4[:, hf * 2 * D:(hf + 1) * 2 * D]
                                    .rearrange("p (c j) -> p c j", c=2),
                                in_=x_d[r0:r0 + 256, :]
                                    .rearrange("(c p) j -> p c j", p=P))
                            nc.sync.dma_start(
                                out=kv4[:, hf * 4 * K:(hf + 1) * 4 * K]
                                    .rearrange("p (c j) -> p c j", c=2),
                                in_=pkv_d[r0:r0 + 256, :]
                                    .rearrange("(c p) j -> p c j", p=P))
                    else:
                        x4 = xin.tile([P, 4 * D], MMDT, tag="x4")
                        kv4 = xin.tile([P, 4 * 2 * K], MMDT, tag="kv4")
                        nc.sync.dma_start(
                            out=x4[:].rearrange("p (c j) -> p c j", c=4),
                            in_=x_d[b4 * 512:(b4 + 1) * 512, :]
                                .rearrange("(c p) j -> p c j", p=P))
                        nc.sync.dma_start(
                            out=kv4[:].rearrange("p (c j) -> p c j", c=4),
                            in_=pkv_d[b4 * 512:(b4 + 1) * 512, :]
                                .rearrange("(c p) j -> p c j", p=P))
                    for c in range(4):
                        nn = b4 * 4 + c
                        for dd in range(8):
                            nc.tensor.matmul(
                                accs[dd][:],
                                lhsT=x4[:, c * D + dd * P: c * D + (dd + 1) * P],
                                rhs=kv4[:, c * 2 * K:(c + 1) * 2 * K],
                                start=(nn == 0), stop=(nn == 31))
                # ordered by first use: wk/wv (A2), wq+xt0 (B prologue),
                # wproj (E(0)), xt1 (B(1))
                load_w(wk_sb, wk_d, 8, DG)
                load_w(wv_sb, wv_d, 8, DG)
                load_w(wq_sb, wq_d, 8, DG)
                xt_tiles = {0: load_xt(0)}
                load_w(wproj_sb, wp_d, 4, D)
                xt_tiles[1] = load_xt(1)
                for dd in range(8):
                    eng = nc.vector if dd % 2 else nc.scalar
                    if dd % 2:
                        nc.vector.tensor_copy(
                            xcxv_sb[:, dd * 2 * K:(dd + 1) * 2 * K], accs[dd][:])
                    else:
                        nc.scalar.copy(
                            out=xcxv_sb[:, dd * 2 * K:(dd + 1) * 2 * K],
                            in_=accs[dd][:])

                # Phase A2 — release the A accumulators' banks first
                pa_ctx.close()
                pa2 = actx.enter_context(tc.tile_pool(name="pa2", bufs=4, space="PSUM"))
                for jc in range(4):
                    acc = pa2.tile([P, K], F32, tag="kpj")
                    for dd in range(8):
                        nc.tensor.matmul(
                            acc[:],
                            lhsT=wk_sb[:, dd * DG + jc * P: dd * DG + (jc + 1) * P],
                            rhs=xcxv_sb[:, dd * 2 * K: dd * 2 * K + K],
                            start=(dd == 0), stop=(dd == 7))
                    if jc % 2:
                        nc.scalar.copy(out=kprojT_sb[:, jc * K:(jc + 1) * K],
                                       in_=acc[:])
                    else:
                        nc.vector.tensor_copy(kprojT_sb[:, jc * K:(jc + 1) * K],
                                              acc[:])
                for fc in range(2):
                    acc2 = pa2.tile([P, DG], F32, tag="vpj")
                    for dd in range(8):
                        nc.tensor.matmul(
                            acc2[:],
                            lhsT=xcxv_sb[:, dd * 2 * K + K + fc * P:
                                         dd * 2 * K + K + (fc + 1) * P],
                            rhs=wv_sb[:, dd * DG:(dd + 1) * DG],
                            start=(dd == 0), stop=(dd == 7))
                    if fc:
                        nc.scalar.copy(out=vproj_sb[:, fc * DG:(fc + 1) * DG],
                                       in_=acc2[:])
                    else:
                        nc.vector.tensor_copy(vproj_sb[:, fc * DG:(fc + 1) * DG],
                                              acc2[:])

            # ---------------- merged loop pools (PSUM) ----------------
            scp = ctx.enter_context(tc.tile_pool(name="scp", bufs=2, space="PSUM"))
            accp = ctx.enter_context(tc.tile_pool(name="accp", bufs=3, space="PSUM"))
            pop = ctx.enter_context(tc.tile_pool(name="pop", bufs=2, space="PSUM"))
            smp = ctx.enter_context(tc.tile_pool(name="smp", bufs=1, space="PSUM"))

            def b_block(xt, qt, jc):
                accq = accp.tile([P, DG], F32, tag="acc")
                for dd in range(8):
                    nc.tensor.matmul(
                        accq[:],
                        lhsT=wq_sb[:, dd * DG + jc * P: dd * DG + (jc + 1) * P],
                        rhs=xt[:, dd * DG:(dd + 1) * DG],
                        start=(dd == 0), stop=(dd == 7))
                nc.vector.tensor_copy(qt[:, jc * DG:(jc + 1) * DG], accq[:])

            def sc_block(qt, h, pexps):
                jc, p0 = h // 2, (h % 2) * DH
                for fc in range(2):
                    st = scp.tile([P, DG], F32, tag="sc")
                    nc.tensor.matmul(
                        st[:],
                        lhsT=kprojT_sb[p0:p0 + DH,
                                       jc * K + fc * P: jc * K + (fc + 1) * P],
                        rhs=qt[p0:p0 + DH, jc * DG:(jc + 1) * DG],
                        start=True, stop=True)
                    pexp = pexp_p.tile([P, DG], MMDT, tag=f"px{h}_{fc}")
                    nc.scalar.activation(pexp[:], st[:], Exp)
                    pexps[(h, fc)] = pexp

            def d_group(nb, nn2, pexps, sp, recips):
                po = pop.tile([P, DG], F32, tag="po")
                for h in range(HL):
                    for fc in range(2):
                        px = pexps[(h, fc)]
                        nc.tensor.matmul(
                            po[:, h * DH:(h + 1) * DH],
                            lhsT=px[:, nn2 * P:(nn2 + 1) * P],
                            rhs=vproj_sb[:, fc * DG + h * DH:
                                         fc * DG + (h + 1) * DH],
                            start=(fc == 0), stop=(fc == 1))
                        nc.tensor.matmul(
                            sp[:, nn2 * HL + h: nn2 * HL + h + 1],
                            lhsT=px[:, nn2 * P:(nn2 + 1) * P],
                            rhs=ones_sb[:],
                            start=(fc == 0), stop=(fc == 1))
                nc.vector.reciprocal(
                    recips[:, nn2 * HL:(nn2 + 1) * HL],
                    sp[:, nn2 * HL:(nn2 + 1) * HL])
                o_t = op_.tile([P, DG], MMDT, tag="o", name=f"o{nb}_{nn2}")
                nc.vector.tensor_tensor(
                    out=o_t[:].rearrange("p (h j) -> p h j", h=HL),
                    in0=po[:].rearrange("p (h j) -> p h j", h=HL),
                    in1=recips[:, nn2 * HL:(nn2 + 1) * HL]
                        .broadcast_to([P, HL, DH]),
                    op=mybir.AluOpType.mult)
                ot = otp.tile([P, DG], MMDT, tag="ot", name=f"ot{nb}_{nn2}")
                nc.sync.dma_start_transpose(
                    out=ot[:].rearrange("p (c j) -> p c j", c=4),
                    in_=o_t[:])
                return ot

            def e_group(nb, nn2, ot):
                ci = nb * 4 + nn2
                outsb = outp.tile([P, D], MMDT, tag="outsb")
                for half in range(2):
                    pe_acc = accp.tile([P, DG], F32, tag="acc")
                    for jc2 in range(4):
                        nc.tensor.matmul(
                            pe_acc[:],
                            lhsT=ot[:, jc2 * P:(jc2 + 1) * P],
                            rhs=wproj_sb[:, jc2 * D + half * DG:
                                         jc2 * D + (half + 1) * DG],
                            start=(jc2 == 0), stop=(jc2 == 3))
                    nc.vector.tensor_copy(
                        outsb[:, half * DG:(half + 1) * DG], pe_acc[:])
                nc.gpsimd.dma_start(out=out_d[ci * P:(ci + 1) * P, :],
                                    in_=outsb[:])

            # ---------------- prologue: B(0) ----------------
            qts = {0: qtp.tile([P, 4 * DG], MMDT, tag="qt", name="qt0")}
            for jc in range(4):
                b_block(xt_tiles[0], qts[0], jc)

            # ---------------- merged loop ----------------
            xt_tiles[2] = load_xt(2)
            prev_ots = None
            for nb in range(NB):
                if nb + 3 < NB:
                    xt_tiles[nb + 3] = load_xt(nb + 3)
                pexps = {}
                cur_ots = []
                sp = smp.tile([P, 4 * HL], F32, tag="sums")
                recips = rcp.tile([P, 4 * HL], F32, tag="recips")
                have_b = nb + 1 < NB
                if have_b:
                    qts[nb + 1] = qtp.tile([P, 4 * DG], MMDT, tag="qt",
                                           name=f"qt{nb + 1}")
                # interleave scores(nb) with B(nb+1) on the PE stream
                for h in range(HL):
                    sc_block(qts[nb], h, pexps)
                    if have_b and h % 2 == 1 and h // 2 < 4:
                        b_block(xt_tiles[nb + 1], qts[nb + 1], h // 2)
                if prev_ots is not None:
                    for nn2 in range(4):
                        e_group(nb - 1, nn2, prev_ots[nn2])
                for nn2 in range(4):
                    cur_ots.append(d_group(nb, nn2, pexps, sp, recips))
                prev_ots = cur_ots
            for nn2 in range(4):
                e_group(NB - 1, nn2, prev_ots[nn2])
    nc.compile()
    return nc


def _np_mm(a):
    return np.ascontiguousarray(np.asarray(a), dtype=mybir.dt.np(MMDT))


def kernel(x, Wq, Wkv, Wproj, bproj, proj_k, proj_v):
    x = np.asarray(x)
    Wq, Wkv, Wproj = np.asarray(Wq), np.asarray(Wkv), np.asarray(Wproj)
    bproj, proj_k, proj_v = np.asarray(bproj), np.asarray(proj_k), np.asarray(proj_v)

    if "nc" not in _cache:
        _cache["nc"] = build_nc()
    nc = _cache["nc"]

    scale = np.float32(DH ** -0.5)
    projkv = _np_mm(np.concatenate([proj_k, proj_v], axis=1))
    in_maps = []
    for c in range(8):
        b, g = c // 2, c % 2
        cols = slice(g * DG, (g + 1) * DG)
        xb = _np_mm(x[b])
        in_maps.append({
            "x": xb,
            "xt": np.ascontiguousarray(xb.T),
            "projkv": projkv,
            "wq": _np_mm(scale * Wq[:, cols]),
            "wk": _np_mm(Wkv[:, :D][:, cols]),
            "wv": _np_mm(Wkv[:, D:][:, cols]),
            "wproj": _np_mm(Wproj[cols, :]),
        })
    res = run_bass_kernel_spmd(nc, in_maps, list(range(8)),
                               trace=bool(os.environ.get("LINF_TRACE")))
    _cache["last_result"] = res
    outs = [np.asarray(r["out"], dtype=np.float32) for r in res.results]
    full = np.stack([outs[2 * b] + outs[2 * b + 1] for b in range(4)])
    full = full + np.asarray(bproj, np.float32)
    return full.astype(np.float32)


# revision 47
# speedup vs baseline: 1.2894x; 1.0295x over previous
"""Linformer self-attention on 8 Trainium2 NeuronCores.

Problem (hardcoded shapes): x [4,4096,1024] f32; per batch:
  q = scale*(x@Wq); kv = x@Wkv; keys/values compressed 4096->256 via
  proj_k/proj_v; 16-head attention (dh=64, k=256); out @ Wproj + bproj.

Sharding: 8 cores = 4 batches x 2 head-groups (8 heads / 512 cols each).
Each core computes a partial [4096,1024] output (Wproj row-split); host
sums the pair and adds bias.

Per-core dataflow (all matmuls use out = lhsT.T @ rhs, K<=128 partitions):
  A : xcxvT[1024,512] = x.T @ [proj_k|proj_v]      (contract n, x natural)
  A2: kprojT[512,256] = Wk_g.T @ xcT ; vproj[256,512] = xvT.T @ Wv_g
  B : qT[512,4096] = Wq_g.T @ xT    (xT provided by host, plain DMA)
  C : per (head,fc k-chunk): scoresT[128,512] -> exp (Act) -> pexp bf16
  S : per (n-chunk, head): sums[n,1] = pexp.T @ ones  (N=1 matmuls)
  D : po[n, 8*64] = pexp.T @ vproj_h per head; normalize via DVE
      tensor_tensor with per-head recip broadcast -> o bf16
  T : oT via one batched DMA transpose per [128,512] tile
  E : out[n,1024] = oT.T-chunks @ Wproj_g, bf16 store via gpsimd SWDGE

B(nb+1), E(nb-1), D(nb) are hand-interleaved in the PE stream per
n-block so Act exp latency hides under PE matmuls.
"""

import os
import numpy as np

import concourse.bass as bass
import concourse.mybir as mybir
import concourse.tile as tile
from concourse import bacc
from concourse.bass_utils import run_bass_kernel_spmd

P = 128
N, D, K, DG, DH = 4096, 1024, 256, 512, 64
NB = 8                    # n-blocks of 512
HL = 8                    # heads per core
F32 = mybir.dt.float32

MMDT_NAME = os.environ.get("LINF_MMDT", "bfloat16")
MMDT = getattr(mybir.dt, MMDT_NAME)
Exp = mybir.ActivationFunctionType.Exp

_cache = {}


def build_nc():
    nc = bacc.Bacc(None, target_bir_lowering=False, debug=False)

    x_d = nc.dram_tensor("x", [N, D], MMDT, kind="ExternalInput")
    xt_d = nc.dram_tensor("xt", [D, N], MMDT, kind="ExternalInput")
    pkv_d = nc.dram_tensor("projkv", [N, 2 * K], MMDT, kind="ExternalInput")
    wq_d = nc.dram_tensor("wq", [D, DG], MMDT, kind="ExternalInput")
    wk_d = nc.dram_tensor("wk", [D, DG], MMDT, kind="ExternalInput")
    wv_d = nc.dram_tensor("wv", [D, DG], MMDT, kind="ExternalInput")
    wp_d = nc.dram_tensor("wproj", [DG, D], MMDT, kind="ExternalInput")
    out_d = nc.dram_tensor("out", [N, D], MMDT, kind="ExternalOutput")

    with tile.TileContext(nc) as tc:
        from contextlib import ExitStack
        with ExitStack() as ctx:
            res = ctx.enter_context(tc.tile_pool(name="res", bufs=1))
            ones_sb = res.tile([P, 1], MMDT, tag="ones")
            nc.vector.memset(ones_sb[:], 1.0)
            from concourse.masks import make_identity
            id_mm = res.tile([P, P], MMDT, tag="id_mm")
            make_identity(nc, id_mm[:])

            wq_sb = res.tile([P, 8 * DG], MMDT, tag="wq")
            wk_sb = res.tile([P, 8 * DG], MMDT, tag="wk")
            wv_sb = res.tile([P, 8 * DG], MMDT, tag="wv")
            wproj_sb = res.tile([P, 4 * D], MMDT, tag="wproj")
            kprojT_sb = res.tile([P, 4 * K], MMDT, tag="kprojT")
            vproj_sb = res.tile([P, 2 * DG], MMDT, tag="vproj")
            xcxv_sb = res.tile([P, 8 * 2 * K], MMDT, tag="xcxv")

            # rolling pools for the merged loop
            xtp = ctx.enter_context(tc.tile_pool(name="xtp", bufs=4))
            qtp = ctx.enter_context(tc.tile_pool(name="qtp", bufs=2))
            pexp_p = ctx.enter_context(tc.tile_pool(name="pexp", bufs=2))
            op_ = ctx.enter_context(tc.tile_pool(name="op", bufs=8))
            otp = ctx.enter_context(tc.tile_pool(name="otp", bufs=8))
            outp = ctx.enter_context(tc.tile_pool(name="outp", bufs=3))
            rcp = ctx.enter_context(tc.tile_pool(name="rcp", bufs=2))

            def load_w(dst, src, nchunk, w):
                # dst[p, c*w + j] = src[c*128 + p, j]
                nc.sync.dma_start(
                    out=dst[:].rearrange("p (c j) -> p c j", c=nchunk),
                    in_=src[:, :].rearrange("(c p) j -> p c j", p=P))

            def load_xt(nb):
                xt = xtp.tile([P, 8 * DG], MMDT, tag="xt", name=f"xt{nb}")
                nc.sync.dma_start(
                    out=xt[:].rearrange("p (d j) -> p d j", d=8),
                    in_=xt_d[:, nb * DG:(nb + 1) * DG]
                        .rearrange("(d p) j -> p d j", p=P))
                return xt

            def b_block(xt, qt, jc):
                accq = accp.tile([P, DG], F32, tag="acc")
                for dd in range(8):
                    nc.tensor.matmul(
                        accq[:],
                        lhsT=wq_sb[:, dd * DG + jc * P: dd * DG + (jc + 1) * P],
                        rhs=xt[:, dd * DG:(dd + 1) * DG],
                        start=(dd == 0), stop=(dd == 7))
                nc.vector.tensor_copy(qt[:, jc * DG:(jc + 1) * DG], accq[:])

            # ---------------- Phase A ----------------
            with ExitStack() as actx:
                xin = actx.enter_context(tc.tile_pool(name="xin", bufs=3))
                pa_ctx = ExitStack()
                pa = pa_ctx.enter_context(tc.tile_pool(name="pa", bufs=1, space="PSUM"))
                accs = [pa.tile([P, 2 * K], F32, tag=f"pa{dd}", name=f"pa{dd}")
                        for dd in range(8)]
                # First chunk loads alone (small, fast) so PE starts ASAP;
                # all weight/xt loads go after the 8 batches — the DMA
                # transfer path is a serial resource and phase A is tight.
                for b4 in range(8):
                    if b4 == 0:
                        # per-chunk interleaved loads: chunk c usable as soon
                        # as its own pair of small DMAs lands
                        x4 = xin.tile([P, 4 * D], MMDT, tag="x4")
                        kv4 = xin.tile([P, 4 * 2 * K], MMDT, tag="kv4")
                        # chunk 0 split by dd-slice: the first matmul only
                        # needs x[0:128, 0:128] + kv chunk 0
                        for dd in range(4):
                            nc.sync.dma_start(
                                out=x4[:, dd * 2 * P:(dd + 1) * 2 * P],
                                in_=x_d[0:P, dd * 2 * P:(dd + 1) * 2 * P])
                            if dd == 0:
                                nc.sync.dma_start(out=kv4[:, :2 * K],
                                                  in_=pkv_d[0:P, :])
                        for c in range(1, 4):
                            nc.sync.dma_start(out=x4[:, c * D:(c + 1) * D],
                                              in_=x_d[c * P:(c + 1) * P, :])
                            nc.sync.dma_start(
                                out=kv4[:, c * 2 * K:(c + 1) * 2 * K],
                                in_=pkv_d[c * P:(c + 1) * P, :])
                    elif b4 == 1:
                        # 2+2 split: first half arrives before the PE (still
                        # in p-state ramp) finishes batch 0
                        x4 = xin.tile([P, 4 * D], MMDT, tag="x4")
                        kv4 = xin.tile([P, 4 * 2 * K], MMDT, tag="kv4")
                        for hf in range(2):
                            r0 = 512 + hf * 256
                            nc.sync.dma_start(
                                out=x4[:, hf * 2 * D:(hf + 1) * 2 * D]
                                    .rearrange("p (c j) -> p c j", c=2),
                                in_=x_d[r0:r0 + 256, :]
                                    .rearrange("(c p) j -> p c j", p=P))
                            nc.sync.dma_start(
                                out=kv4[:, hf * 4 * K:(hf + 1) * 4 * K]
                                    .rearrange("p (c j) -> p c j", c=2),
                                in_=pkv_d[r0:r0 + 256, :]
                                    .rearrange("(c p) j -> p c j", p=P))
                    else:
                        x4 = xin.tile([P, 4 * D], MMDT, tag="x4")
                        kv4 = xin.tile([P, 4 * 2 * K], MMDT, tag="kv4")
                        nc.sync.dma_start(
                            out=x4[:].rearrange("p (c j) -> p c j", c=4),
                            in_=x_d[b4 * 512:(b4 + 1) * 512, :]
                                .rearrange("(c p) j -> p c j", p=P))
                        nc.sync.dma_start(
                            out=kv4[:].rearrange("p (c j) -> p c j", c=4),
                            in_=pkv_d[b4 * 512:(b4 + 1) * 512, :]
                                .rearrange("(c p) j -> p c j", p=P))
                    for c in range(4):
                        nn = b4 * 4 + c
                        for dd in range(8):
                            nc.tensor.matmul(
                                accs[dd][:],
                                lhsT=x4[:, c * D + dd * P: c * D + (dd + 1) * P],
                                rhs=kv4[:, c * 2 * K:(c + 1) * 2 * K],
                                start=(nn == 0), stop=(nn == 31))
                # ordered by first use: xt0+wq (B(0), first thing after A),
                # wk/wv (A2), wproj (E(0)), xt1 (B(1))
                xt_tiles = {0: load_xt(0)}
                load_w(wq_sb, wq_d, 8, DG)
                load_w(wk_sb, wk_d, 8, DG)
                load_w(wv_sb, wv_d, 8, DG)
                load_w(wproj_sb, wp_d, 4, D)
                xt_tiles[1] = load_xt(1)
                for dd in range(8):
                    eng = nc.vector if dd % 2 else nc.scalar
                    if dd % 2:
                        nc.vector.tensor_copy(
                            xcxv_sb[:, dd * 2 * K:(dd + 1) * 2 * K], accs[dd][:])
                    else:
                        nc.scalar.copy(
                            out=xcxv_sb[:, dd * 2 * K:(dd + 1) * 2 * K],
                            in_=accs[dd][:])

                # Phase A2 — release the A accumulators' banks first.
                # B(0) blocks are interleaved with the A2 groups: they fill
                # the PE while A2 waits on the xcxv eviction pipeline.
                pa_ctx.close()
                pa2 = actx.enter_context(tc.tile_pool(name="pa2", bufs=2, space="PSUM"))
                accp = ctx.enter_context(tc.tile_pool(name="accp", bufs=3,
                                                      space="PSUM", side="right"))
                qts = {0: qtp.tile([P, 4 * DG], MMDT, tag="qt", name="qt0")}
                for jc in range(4):
                    # B(0) first: it has no dependence on the A evictions, so
                    # it covers the xcxv eviction pipeline latency
                    b_block(xt_tiles[0], qts[0], jc)
                    acc = pa2.tile([P, K], F32, tag="kpj")
                    for dd in range(8):
                        nc.tensor.matmul(
                            acc[:],
                            lhsT=wk_sb[:, dd * DG + jc * P: dd * DG + (jc + 1) * P],
                            rhs=xcxv_sb[:, dd * 2 * K: dd * 2 * K + K],
                            start=(dd == 0), stop=(dd == 7))
                    if jc % 2:
                        nc.scalar.copy(out=kprojT_sb[:, jc * K:(jc + 1) * K],
                                       in_=acc[:])
                    else:
                        nc.vector.tensor_copy(kprojT_sb[:, jc * K:(jc + 1) * K],
                                              acc[:])
                for fc in range(2):
                    acc2 = pa2.tile([P, DG], F32, tag="vpj")
                    for dd in range(8):
                        nc.tensor.matmul(
                            acc2[:],
                            lhsT=xcxv_sb[:, dd * 2 * K + K + fc * P:
                                         dd * 2 * K + K + (fc + 1) * P],
                            rhs=wv_sb[:, dd * DG:(dd + 1) * DG],
                            start=(dd == 0), stop=(dd == 7))
                    if fc:
                        nc.scalar.copy(out=vproj_sb[:, fc * DG:(fc + 1) * DG],
                                       in_=acc2[:])
                    else:
                        nc.vector.tensor_copy(vproj_sb[:, fc * DG:(fc + 1) * DG],
                                              acc2[:])

            # ---------------- merged loop pools (PSUM) ----------------
            # scp last on the left stack: it is released after the final
            # score block to make room for the epilogue transpose pool
            pop = ctx.enter_context(tc.tile_pool(name="pop", bufs=2, space="PSUM"))
            smp = ctx.enter_context(tc.tile_pool(name="smp", bufs=1, space="PSUM"))
            scp_ctx = ExitStack()
            scp = scp_ctx.enter_context(tc.tile_pool(name="scp", bufs=2,
                                                     space="PSUM"))

            def sc_block(qt, h, pexps):
                jc, p0 = h // 2, (h % 2) * DH
                for fc in range(2):
                    st = scp.tile([P, DG], F32, tag="sc")
                    nc.tensor.matmul(
                        st[:],
                        lhsT=kprojT_sb[p0:p0 + DH,
                                       jc * K + fc * P: jc * K + (fc + 1) * P],
                        rhs=qt[p0:p0 + DH, jc * DG:(jc + 1) * DG],
                        start=True, stop=True)
                    pexp = pexp_p.tile([P, DG], MMDT, tag=f"px{h}_{fc}")
                    nc.scalar.activation(pexp[:], st[:], Exp)
                    pexps[(h, fc)] = pexp

            def d_group(nb, nn2, pexps, sp, recips, skip_t=False):
                po = pop.tile([P, DG], F32, tag="po")
                for h in range(HL):
                    for fc in range(2):
                        px = pexps[(h, fc)]
                        nc.tensor.matmul(
                            po[:, h * DH:(h + 1) * DH],
                            lhsT=px[:, nn2 * P:(nn2 + 1) * P],
                            rhs=vproj_sb[:, fc * DG + h * DH:
                                         fc * DG + (h + 1) * DH],
                            start=(fc == 0), stop=(fc == 1))
                        nc.tensor.matmul(
                            sp[:, nn2 * HL + h: nn2 * HL + h + 1],
                            lhsT=px[:, nn2 * P:(nn2 + 1) * P],
                            rhs=ones_sb[:],
                            start=(fc == 0), stop=(fc == 1))
                nc.vector.reciprocal(
                    recips[:, nn2 * HL:(nn2 + 1) * HL],
                    sp[:, nn2 * HL:(nn2 + 1) * HL])
                o_t = op_.tile([P, DG], MMDT, tag="o", name=f"o{nb}_{nn2}")
                nc.vector.tensor_tensor(
                    out=o_t[:].rearrange("p (h j) -> p h j", h=HL),
                    in0=po[:].rearrange("p (h j) -> p h j", h=HL),
                    in1=recips[:, nn2 * HL:(nn2 + 1) * HL]
                        .broadcast_to([P, HL, DH]),
                    op=mybir.AluOpType.mult)
                if skip_t:
                    return o_t
                ot = otp.tile([P, DG], MMDT, tag="ot", name=f"ot{nb}_{nn2}")
                nc.sync.dma_start_transpose(
                    out=ot[:].rearrange("p (c j) -> p c j", c=4),
                    in_=o_t[:])
                return ot

            def e_group(nb, nn2, ot, last=False):
                ci = nb * 4 + nn2
                outsb = outp.tile([P, D], MMDT, tag="outsb")
                for half in range(2):
                    pe_acc = accp.tile([P, DG], F32, tag="acc")
                    for jc2 in range(4):
                        nc.tensor.matmul(
                            pe_acc[:],
                            lhsT=ot[:, jc2 * P:(jc2 + 1) * P],
                            rhs=wproj_sb[:, jc2 * D + half * DG:
                                         jc2 * D + (half + 1) * DG],
                            start=(jc2 == 0), stop=(jc2 == 3))
                    if last and nn2 >= 2:
                        # fast tail: evict on both engines, store each half as
                        # soon as it lands (HWDGE has lower fixed latency)
                        if half == 0:
                            nc.scalar.copy(out=outsb[:, :DG], in_=pe_acc[:])
                        else:
                            nc.vector.tensor_copy(outsb[:, DG:], pe_acc[:])
                        nc.sync.dma_start(
                            out=out_d[ci * P:(ci + 1) * P,
                                      half * DG:(half + 1) * DG],
                            in_=outsb[:, half * DG:(half + 1) * DG])
                    else:
                        nc.vector.tensor_copy(
                            outsb[:, half * DG:(half + 1) * DG], pe_acc[:])
                if not (last and nn2 >= 2):
                    nc.gpsimd.dma_start(out=out_d[ci * P:(ci + 1) * P, :],
                                        in_=outsb[:])

            # ---------------- merged loop (nb = 0..6) ----------------
            xt_tiles[2] = load_xt(2)
            prev_ots = None
            pexps7 = {}
            for nb in range(NB - 1):
                if nb + 3 < NB:
                    xt_tiles[nb + 3] = load_xt(nb + 3)
                pexps = {}
                cur_ots = []
                sp = smp.tile([P, 4 * HL], F32, tag="sums")
                recips = rcp.tile([P, 4 * HL], F32, tag="recips")
                qts[nb + 1] = qtp.tile([P, 4 * DG], MMDT, tag="qt",
                                       name=f"qt{nb + 1}")
                # interleave scores(nb) with B(nb+1) on the PE stream
                for h in range(HL):
                    sc_block(qts[nb], h, pexps)
                    if h % 2 == 1:
                        b_block(xt_tiles[nb + 1], qts[nb + 1], h // 2)
                if prev_ots is not None:
                    for nn2 in range(4):
                        e_group(nb - 1, nn2, prev_ots[nn2])
                for nn2 in range(4):
                    cur_ots.append(d_group(nb, nn2, pexps, sp, recips))
                if nb == NB - 2:
                    # hoist scores(7) into this iter's tail so its exps have
                    # drained before D(7) runs next iter
                    for h in range(HL):
                        sc_block(qts[NB - 1], h, pexps7)
                prev_ots = cur_ots
            # ---- last iter: E(6,3) placed after D(7) to cover latency;
            # o-transposes for block 7 run on the PE (via the freed score
            # banks) instead of the ~3us-latency DMA-transpose path
            scp_ctx.close()
            trp = ctx.enter_context(tc.tile_pool(name="trp", bufs=2,
                                                 space="PSUM"))
            sp = smp.tile([P, 4 * HL], F32, tag="sums")
            recips = rcp.tile([P, 4 * HL], F32, tag="recips")
            for nn2 in range(3):
                e_group(NB - 2, nn2, prev_ots[nn2])
            o7 = [d_group(NB - 1, nn2, pexps7, sp, recips, skip_t=True)
                  for nn2 in range(4)]
            e_group(NB - 2, 3, prev_ots[3])
            ots7 = []
            for nn2 in range(4):
                tr = trp.tile([P, DG], MMDT, tag="tr")
                for c in range(4):
                    nc.tensor.transpose(tr[:, c * P:(c + 1) * P],
                                        o7[nn2][:, c * P:(c + 1) * P],
                                        id_mm[:])
                ot = otp.tile([P, DG], MMDT, tag="ot", name=f"otz{nn2}")
                nc.scalar.copy(out=ot[:], in_=tr[:])
                ots7.append(ot)
            for nn2 in range(4):
                e_group(NB - 1, nn2, ots7[nn2], last=True)
    nc.compile()
    return nc


def _np_mm(a):
    return np.ascontiguousarray(np.asarray(a), dtype=mybir.dt.np(MMDT))


def kernel(x, Wq, Wkv, Wproj, bproj, proj_k, proj_v):
    x = np.asarray(x)
    Wq, Wkv, Wproj = np.asarray(Wq), np.asarray(Wkv), np.asarray(Wproj)
    bproj, proj_k, proj_v = np.asarray(bproj), np.asarray(proj_k), np.asarray(proj_v)

    if "nc" not in _cache:
        _cache["nc"] = build_nc()
    nc = _cache["nc"]

    scale = np.float32(DH ** -0.5)
    projkv = _np_mm(np.concatenate([proj_k, proj_v], axis=1))
    in_maps = []
    for c in range(8):
        b, g = c // 2, c % 2
        cols = slice(g * DG, (g + 1) * DG)
        xb = _np_mm(x[b])
        in_maps.append({
            "x": xb,
            "xt": np.ascontiguousarray(xb.T),
            "projkv": projkv,
            "wq": _np_mm(scale * Wq[:, cols]),
            "wk": _np_mm(Wkv[:, :D][:, cols]),
            "wv": _np_mm(Wkv[:, D:][:, cols]),
            "wproj": _np_mm(Wproj[cols, :]),
        })
    res = run_bass_kernel_spmd(nc, in_maps, list(range(8)),
                               trace=bool(os.environ.get("LINF_TRACE")))
    _cache["last_result"] = res
    outs = [np.asarray(r["out"], dtype=np.float32) for r in res.results]
    full = np.stack([outs[2 * b] + outs[2 * b + 1] for b in range(4)])
    full = full + np.asarray(bproj, np.float32)
    return full.astype(np.float32)


# revision 61
# speedup vs baseline: 1.3000x; 1.0083x over previous
"""Linformer self-attention on 8 Trainium2 NeuronCores.

Problem (hardcoded shapes): x [4,4096,1024] f32; per batch:
  q = scale*(x@Wq); kv = x@Wkv; keys/values compressed 4096->256 via
  proj_k/proj_v; 16-head attention (dh=64, k=256); out @ Wproj + bproj.

Sharding: 8 cores = 4 batches x 2 head-groups (8 heads / 512 cols each).
Each core computes a partial [4096,1024] output (Wproj row-split); host
sums the pair and adds bias.

Per-core dataflow (all matmuls use out = lhsT.T @ rhs, K<=128 partitions):
  A : xcxvT[1024,512] = x.T @ [proj_k|proj_v]      (contract n, x natural)
  A2: kprojT[512,256] = Wk_g.T @ xcT ; vproj[256,512] = xvT.T @ Wv_g
  B : qT[512,4096] = Wq_g.T @ xT    (xT provided by host, plain DMA)
  C : per (head,fc k-chunk): scoresT[128,512] -> exp (Act) -> pexp bf16
  S : per (n-chunk, head): sums[n,1] = pexp.T @ ones  (N=1 matmuls)
  D : po[n, 8*64] = pexp.T @ vproj_h per head; normalize via DVE
      tensor_tensor with per-head recip broadcast -> o bf16
  T : oT via one batched DMA transpose per [128,512] tile
  E : out[n,1024] = oT.T-chunks @ Wproj_g, bf16 store via gpsimd SWDGE

B(nb+1), E(nb-1), D(nb) are hand-interleaved in the PE stream per
n-block so Act exp latency hides under PE matmuls.
"""

import os
import numpy as np

import concourse.bass as bass
import concourse.mybir as mybir
import concourse.tile as tile
from concourse import bacc
from concourse.bass_utils import run_bass_kernel_spmd

P = 128
N, D, K, DG, DH = 4096, 1024, 256, 512, 64
NB = 8                    # n-blocks of 512
HL = 8                    # heads per core
F32 = mybir.dt.float32

MMDT_NAME = os.environ.get("LINF_MMDT", "bfloat16")
MMDT = getattr(mybir.dt, MMDT_NAME)
Exp = mybir.ActivationFunctionType.Exp

_cache = {}


def build_nc():
    nc = bacc.Bacc(None, target_bir_lowering=False, debug=False)

    x_d = nc.dram_tensor("x", [N, D], MMDT, kind="ExternalInput")
    xt_d = nc.dram_tensor("xt", [D, N], MMDT, kind="ExternalInput")
    pkv_d = nc.dram_tensor("projkv", [N, 2 * K], MMDT, kind="ExternalInput")
    wq_d = nc.dram_tensor("wq", [D, DG], MMDT, kind="ExternalInput")
    wk_d = nc.dram_tensor("wk", [D, DG], MMDT, kind="ExternalInput")
    wv_d = nc.dram_tensor("wv", [D, DG], MMDT, kind="ExternalInput")
    wp_d = nc.dram_tensor("wproj", [DG, D], MMDT, kind="ExternalInput")
    out_d = nc.dram_tensor("out", [N, D], MMDT, kind="ExternalOutput")

    with tile.TileContext(nc) as tc:
        from contextlib import ExitStack
        with ExitStack() as ctx:
            res = ctx.enter_context(tc.tile_pool(name="res", bufs=1))
            ones_sb = res.tile([P, 1], MMDT, tag="ones")
            nc.vector.memset(ones_sb[:], 1.0)
            from concourse.masks import make_identity
            id_mm = res.tile([P, P], MMDT, tag="id_mm")
            make_identity(nc, id_mm[:])

            wq_sb = res.tile([P, 8 * DG], MMDT, tag="wq")
            wk_sb = res.tile([P, 8 * DG], MMDT, tag="wk")
            wv_sb = res.tile([P, 8 * DG], MMDT, tag="wv")
            wproj_sb = res.tile([P, 4 * D], MMDT, tag="wproj")
            kprojT_sb = res.tile([P, 4 * K], MMDT, tag="kprojT")
            vproj_sb = res.tile([P, 2 * DG], MMDT, tag="vproj")
            xcxv_sb = res.tile([P, 8 * 2 * K], MMDT, tag="xcxv")

            # rolling pools for the merged loop
            xtp = ctx.enter_context(tc.tile_pool(name="xtp", bufs=4))
            qtp = ctx.enter_context(tc.tile_pool(name="qtp", bufs=2))
            pexp_p = ctx.enter_context(tc.tile_pool(name="pexp", bufs=2))
            op_ = ctx.enter_context(tc.tile_pool(name="op", bufs=8))
            otp = ctx.enter_context(tc.tile_pool(name="otp", bufs=8))
            outp = ctx.enter_context(tc.tile_pool(name="outp", bufs=3))
            rcp = ctx.enter_context(tc.tile_pool(name="rcp", bufs=2))

            def load_w(dst, src, nchunk, w):
                # dst[p, c*w + j] = src[c*128 + p, j]
                nc.sync.dma_start(
                    out=dst[:].rearrange("p (c j) -> p c j", c=nchunk),
                    in_=src[:, :].rearrange("(c p) j -> p c j", p=P))

            def load_xt(nb):
                xt = xtp.tile([P, 8 * DG], MMDT, tag="xt", name=f"xt{nb}")
                nc.sync.dma_start(
                    out=xt[:].rearrange("p (d j) -> p d j", d=8),
                    in_=xt_d[:, nb * DG:(nb + 1) * DG]
                        .rearrange("(d p) j -> p d j", p=P))
                return xt

            def b_block(xt, qt, jc):
                accq = accp.tile([P, DG], F32, tag="acc")
                for dd in range(8):
                    nc.tensor.matmul(
                        accq[:],
                        lhsT=wq_sb[:, dd * DG + jc * P: dd * DG + (jc + 1) * P],
                        rhs=xt[:, dd * DG:(dd + 1) * DG],
                        start=(dd == 0), stop=(dd == 7))
                nc.vector.tensor_copy(qt[:, jc * DG:(jc + 1) * DG], accq[:])

            # ---------------- Phase A ----------------
            with ExitStack() as actx:
                xin = actx.enter_context(tc.tile_pool(name="xin", bufs=3))
                pa_ctx = ExitStack()
                pa = pa_ctx.enter_context(tc.tile_pool(name="pa", bufs=1, space="PSUM"))
                accs = [pa.tile([P, 2 * K], F32, tag=f"pa{dd}", name=f"pa{dd}")
                        for dd in range(8)]
                # PE p-state warm-up: keep the tensor engine continuously
                # busy from t~0.3us so the clock is fully ramped (needs 3us
                # of busy) by the time the first x chunk lands (~2.9us).
                # Output goes to a PSUM region that the first real
                # accumulation group resets (start=True).
                for _ in range(16):
                    nc.tensor.matmul(accs[0][:1, :P], lhsT=ones_sb[:],
                                     rhs=id_mm[:], start=True, stop=True)
                # First chunk loads alone (small, fast) so PE starts ASAP;
                # all weight/xt loads go after the 8 batches — the DMA
                # transfer path is a serial resource and phase A is tight.
                for b4 in range(8):
                    if b4 == 0:
                        # per-chunk interleaved loads: chunk c usable as soon
                        # as its own pair of small DMAs lands
                        x4 = xin.tile([P, 4 * D], MMDT, tag="x4")
                        kv4 = xin.tile([P, 4 * 2 * K], MMDT, tag="kv4")
                        # chunk 0 split by dd-slice: the first matmul only
                        # needs x[0:128, 0:128] + kv chunk 0
                        for dd in range(4):
                            nc.sync.dma_start(
                                out=x4[:, dd * 2 * P:(dd + 1) * 2 * P],
                                in_=x_d[0:P, dd * 2 * P:(dd + 1) * 2 * P])
                            if dd == 0:
                                nc.sync.dma_start(out=kv4[:, :2 * K],
                                                  in_=pkv_d[0:P, :])
                        for c in range(1, 4):
                            nc.sync.dma_start(out=x4[:, c * D:(c + 1) * D],
                                              in_=x_d[c * P:(c + 1) * P, :])
                            nc.sync.dma_start(
                                out=kv4[:, c * 2 * K:(c + 1) * 2 * K],
                                in_=pkv_d[c * P:(c + 1) * P, :])
                    elif b4 == 1:
                        # 2+2 split: first half arrives before the PE (still
                        # in p-state ramp) finishes batch 0
                        x4 = xin.tile([P, 4 * D], MMDT, tag="x4")
                        kv4 = xin.tile([P, 4 * 2 * K], MMDT, tag="kv4")
                        for hf in range(2):
                            r0 = 512 + hf * 256
                            nc.sync.dma_start(
                                out=x4[:, hf * 2 * D:(hf + 1) * 2 * D]
                                    .rearrange("p (c j) -> p c j", c=2),
                                in_=x_d[r0:r0 + 256, :]
                                    .rearrange("(c p) j -> p c j", p=P))
                            nc.sync.dma_start(
                                out=kv4[:, hf * 4 * K:(hf + 1) * 4 * K]
                                    .rearrange("p (c j) -> p c j", c=2),
                                in_=pkv_d[r0:r0 + 256, :]
                                    .rearrange("(c p) j -> p c j", p=P))
                    else:
                        x4 = xin.tile([P, 4 * D], MMDT, tag="x4")
                        kv4 = xin.tile([P, 4 * 2 * K], MMDT, tag="kv4")
                        nc.sync.dma_start(
                            out=x4[:].rearrange("p (c j) -> p c j", c=4),
                            in_=x_d[b4 * 512:(b4 + 1) * 512, :]
                                .rearrange("(c p) j -> p c j", p=P))
                        nc.sync.dma_start(
                            out=kv4[:].rearrange("p (c j) -> p c j", c=4),
                            in_=pkv_d[b4 * 512:(b4 + 1) * 512, :]
                                .rearrange("(c p) j -> p c j", p=P))
                    for c in range(4):
                        nn = b4 * 4 + c
                        for dd in range(8):
                            nc.tensor.matmul(
                                accs[dd][:],
                                lhsT=x4[:, c * D + dd * P: c * D + (dd + 1) * P],
                                rhs=kv4[:, c * 2 * K:(c + 1) * 2 * K],
                                start=(nn == 0), stop=(nn == 31))
                # ordered by first use: xt0+wq (B(0), first thing after A),
                # wk/wv (A2), wproj (E(0)), xt1 (B(1))
                xt_tiles = {0: load_xt(0)}
                load_w(wq_sb, wq_d, 8, DG)
                load_w(wk_sb, wk_d, 8, DG)
                load_w(wv_sb, wv_d, 8, DG)
                load_w(wproj_sb, wp_d, 4, D)
                xt_tiles[1] = load_xt(1)
                for dd in range(8):
                    eng = nc.vector if dd % 2 else nc.scalar
                    if dd % 2:
                        nc.vector.tensor_copy(
                            xcxv_sb[:, dd * 2 * K:(dd + 1) * 2 * K], accs[dd][:])
                    else:
                        nc.scalar.copy(
                            out=xcxv_sb[:, dd * 2 * K:(dd + 1) * 2 * K],
                            in_=accs[dd][:])

                # Phase A2 — release the A accumulators' banks first.
                # B(0) blocks are interleaved with the A2 groups: they fill
                # the PE while A2 waits on the xcxv eviction pipeline.
                pa_ctx.close()
                pa2 = actx.enter_context(tc.tile_pool(name="pa2", bufs=2, space="PSUM"))
                accp = ctx.enter_context(tc.tile_pool(name="accp", bufs=3,
                                                      space="PSUM", side="right"))
                qts = {0: qtp.tile([P, 4 * DG], MMDT, tag="qt", name="qt0")}
                for jc in range(4):
                    # B(0) first: it has no dependence on the A evictions, so
                    # it covers the xcxv eviction pipeline latency
                    b_block(xt_tiles[0], qts[0], jc)
                    acc = pa2.tile([P, K], F32, tag="kpj")
                    for dd in range(8):
                        nc.tensor.matmul(
                            acc[:],
                            lhsT=wk_sb[:, dd * DG + jc * P: dd * DG + (jc + 1) * P],
                            rhs=xcxv_sb[:, dd * 2 * K: dd * 2 * K + K],
                            start=(dd == 0), stop=(dd == 7))
                    if jc % 2:
                        nc.scalar.copy(out=kprojT_sb[:, jc * K:(jc + 1) * K],
                                       in_=acc[:])
                    else:
                        nc.vector.tensor_copy(kprojT_sb[:, jc * K:(jc + 1) * K],
                                              acc[:])
                for fc in range(2):
                    acc2 = pa2.tile([P, DG], F32, tag="vpj")
                    for dd in range(8):
                        nc.tensor.matmul(
                            acc2[:],
                            lhsT=xcxv_sb[:, dd * 2 * K + K + fc * P:
                                         dd * 2 * K + K + (fc + 1) * P],
                            rhs=wv_sb[:, dd * DG:(dd + 1) * DG],
                            start=(dd == 0), stop=(dd == 7))
                    if fc:
                        nc.scalar.copy(out=vproj_sb[:, fc * DG:(fc + 1) * DG],
                                       in_=acc2[:])
                    else:
                        nc.vector.tensor_copy(vproj_sb[:, fc * DG:(fc + 1) * DG],
                                              acc2[:])

            # ---------------- merged loop pools (PSUM) ----------------
            # scp last on the left stack: it is released after the final
            # score block to make room for the epilogue transpose pool
            pop = ctx.enter_context(tc.tile_pool(name="pop", bufs=2, space="PSUM"))
            smp = ctx.enter_context(tc.tile_pool(name="smp", bufs=1, space="PSUM"))
            scp_ctx = ExitStack()
            scp = scp_ctx.enter_context(tc.tile_pool(name="scp", bufs=2,
                                                     space="PSUM"))

            def sc_block(qt, h, pexps):
                jc, p0 = h // 2, (h % 2) * DH
                for fc in range(2):
                    st = scp.tile([P, DG], F32, tag="sc")
                    nc.tensor.matmul(
                        st[:],
                        lhsT=kprojT_sb[p0:p0 + DH,
                                       jc * K + fc * P: jc * K + (fc + 1) * P],
                        rhs=qt[p0:p0 + DH, jc * DG:(jc + 1) * DG],
                        start=True, stop=True)
                    pexp = pexp_p.tile([P, DG], MMDT, tag=f"px{h}_{fc}")
                    nc.scalar.activation(pexp[:], st[:], Exp)
                    pexps[(h, fc)] = pexp

            def d_group(nb, nn2, pexps, sp, recips, skip_t=False):
                po = pop.tile([P, DG], F32, tag="po")
                for h in range(HL):
                    for fc in range(2):
                        px = pexps[(h, fc)]
                        nc.tensor.matmul(
                            po[:, h * DH:(h + 1) * DH],
                            lhsT=px[:, nn2 * P:(nn2 + 1) * P],
                            rhs=vproj_sb[:, fc * DG + h * DH:
                                         fc * DG + (h + 1) * DH],
                            start=(fc == 0), stop=(fc == 1))
                        nc.tensor.matmul(
                            sp[:, nn2 * HL + h: nn2 * HL + h + 1],
                            lhsT=px[:, nn2 * P:(nn2 + 1) * P],
                            rhs=ones_sb[:],
                            start=(fc == 0), stop=(fc == 1))
                nc.vector.reciprocal(
                    recips[:, nn2 * HL:(nn2 + 1) * HL],
                    sp[:, nn2 * HL:(nn2 + 1) * HL])
                o_t = op_.tile([P, DG], MMDT, tag="o", name=f"o{nb}_{nn2}")
                nc.vector.tensor_tensor(
                    out=o_t[:].rearrange("p (h j) -> p h j", h=HL),
                    in0=po[:].rearrange("p (h j) -> p h j", h=HL),
                    in1=recips[:, nn2 * HL:(nn2 + 1) * HL]
                        .broadcast_to([P, HL, DH]),
                    op=mybir.AluOpType.mult)
                if skip_t:
                    return o_t
                ot = otp.tile([P, DG], MMDT, tag="ot", name=f"ot{nb}_{nn2}")
                nc.sync.dma_start_transpose(
                    out=ot[:].rearrange("p (c j) -> p c j", c=4),
                    in_=o_t[:])
                return ot

            def e_group(nb, nn2, ot, last=False, store_eng=None):
                ci = nb * 4 + nn2
                outsb = outp.tile([P, D], MMDT, tag="outsb")
                for half in range(2):
                    pe_acc = accp.tile([P, DG], F32, tag="acc")
                    for jc2 in range(4):
                        nc.tensor.matmul(
                            pe_acc[:],
                            lhsT=ot[:, jc2 * P:(jc2 + 1) * P],
                            rhs=wproj_sb[:, jc2 * D + half * DG:
                                         jc2 * D + (half + 1) * DG],
                            start=(jc2 == 0), stop=(jc2 == 3))
                    if last:
                        # fast tail: evict on both engines, store each half as
                        # soon as it lands (HWDGE has lower fixed latency)
                        if half == 0:
                            nc.scalar.copy(out=outsb[:, :DG], in_=pe_acc[:])
                        else:
                            nc.vector.tensor_copy(outsb[:, DG:], pe_acc[:])
                        nc.sync.dma_start(
                            out=out_d[ci * P:(ci + 1) * P,
                                      half * DG:(half + 1) * DG],
                            in_=outsb[:, half * DG:(half + 1) * DG])
                    else:
                        nc.vector.tensor_copy(
                            outsb[:, half * DG:(half + 1) * DG], pe_acc[:])
                if not last:
                    (store_eng or nc.sync).dma_start(
                        out=out_d[ci * P:(ci + 1) * P, :], in_=outsb[:])

            # ---------------- merged loop (nb = 0..6) ----------------
            xt_tiles[2] = load_xt(2)
            prev_ots = None
            pexps7 = {}
            for nb in range(NB - 1):
                if nb + 3 < NB:
                    xt_tiles[nb + 3] = load_xt(nb + 3)
                pexps = {}
                cur_ots = []
                sp = smp.tile([P, 4 * HL], F32, tag="sums")
                recips = rcp.tile([P, 4 * HL], F32, tag="recips")
                qts[nb + 1] = qtp.tile([P, 4 * DG], MMDT, tag="qt",
                                       name=f"qt{nb + 1}")
                # interleave scores(nb) with B(nb+1) on the PE stream
                for h in range(HL):
                    sc_block(qts[nb], h, pexps)
                    if h % 2 == 1:
                        b_block(xt_tiles[nb + 1], qts[nb + 1], h // 2)
                if prev_ots is not None:
                    for nn2 in range(4):
                        e_group(nb - 1, nn2, prev_ots[nn2])
                for nn2 in range(4):
                    cur_ots.append(d_group(nb, nn2, pexps, sp, recips))
                if nb == NB - 2:
                    # hoist scores(7) into this iter's tail so its exps have
                    # drained before D(7) runs next iter
                    for h in range(HL):
                        sc_block(qts[NB - 1], h, pexps7)
                prev_ots = cur_ots
            # ---- last iter: E(6,3) placed after D(7) to cover latency;
            # o-transposes for block 7 run on the PE (via the freed score
            # banks) instead of the ~3us-latency DMA-transpose path
            scp_ctx.close()
            trp = ctx.enter_context(tc.tile_pool(name="trp", bufs=2,
                                                 space="PSUM"))
            sp = smp.tile([P, 4 * HL], F32, tag="sums")
            recips = rcp.tile([P, 4 * HL], F32, tag="recips")
            for nn2 in range(3):
                e_group(NB - 2, nn2, prev_ots[nn2])
            o7 = [d_group(NB - 1, nn2, pexps7, sp, recips, skip_t=True)
                  for nn2 in range(4)]
            e_group(NB - 2, 3, prev_ots[3], store_eng=nc.scalar)
            ots7 = []
            for nn2 in range(4):
                tr = trp.tile([P, DG], MMDT, tag="tr")
                for c in range(4):
                    nc.tensor.transpose(tr[:, c * P:(c + 1) * P],
                                        o7[nn2][:, c * P:(c + 1) * P],
                                        id_mm[:])
                ot = otp.tile([P, DG], MMDT, tag="ot", name=f"otz{nn2}")
                nc.scalar.copy(out=ot[:], in_=tr[:])
                ots7.append(ot)
            for nn2 in range(4):
                e_group(NB - 1, nn2, ots7[nn2], last=True)
    nc.compile()
    return nc


def _np_mm(a):
    return np.ascontiguousarray(np.asarray(a), dtype=mybir.dt.np(MMDT))


def kernel(x, Wq, Wkv, Wproj, bproj, proj_k, proj_v):
    x = np.asarray(x)
    Wq, Wkv, Wproj = np.asarray(Wq), np.asarray(Wkv), np.asarray(Wproj)
    bproj, proj_k, proj_v = np.asarray(bproj), np.asarray(proj_k), np.asarray(proj_v)

    if "nc" not in _cache:
        _cache["nc"] = build_nc()
    nc = _cache["nc"]

    scale = np.float32(DH ** -0.5)
    projkv = _np_mm(np.concatenate([proj_k, proj_v], axis=1))
    in_maps = []
    for c in range(8):
        b, g = c // 2, c % 2
        cols = slice(g * DG, (g + 1) * DG)
        xb = _np_mm(x[b])
        in_maps.append({
            "x": xb,
            "xt": np.ascontiguousarray(xb.T),
            "projkv": projkv,
            "wq": _np_mm(scale * Wq[:, cols]),
            "wk": _np_mm(Wkv[:, :D][:, cols]),
            "wv": _np_mm(Wkv[:, D:][:, cols]),
            "wproj": _np_mm(Wproj[cols, :]),
        })
    res = run_bass_kernel_spmd(nc, in_maps, list(range(8)),
                               trace=bool(os.environ.get("LINF_TRACE")))
    _cache["last_result"] = res
    outs = [np.asarray(r["out"], dtype=np.float32) for r in res.results]
    full = np.stack([outs[2 * b] + outs[2 * b + 1] for b in range(4)])
    full = full + np.asarray(bproj, np.float32)
    return full.astype(np.float32)


# revision 85
# speedup vs baseline: 1.3005x; 1.0003x over previous
"""Linformer self-attention on 8 Trainium2 NeuronCores.

Problem (hardcoded shapes): x [4,4096,1024] f32; per batch:
  q = scale*(x@Wq); kv = x@Wkv; keys/values compressed 4096->256 via
  proj_k/proj_v; 16-head attention (dh=64, k=256); out @ Wproj + bproj.

Sharding: 8 cores = 4 batches x 2 head-groups (8 heads / 512 cols each).
Each core computes a partial [4096,1024] output (Wproj row-split); host
sums the pair and adds bias.

Per-core dataflow (all matmuls use out = lhsT.T @ rhs, K<=128 partitions):
  A : xcxvT[1024,512] = x.T @ [proj_k|proj_v]      (contract n, x natural)
  A2: kprojT[512,256] = Wk_g.T @ xcT ; vproj[256,512] = xvT.T @ Wv_g
  B : qT[512,4096] = Wq_g.T @ xT    (xT provided by host, plain DMA)
  C : per (head,fc k-chunk): scoresT[128,512] -> exp (Act) -> pexp bf16
  S : per (n-chunk, head): sums[n,1] = pexp.T @ ones  (N=1 matmuls)
  D : po[n, 8*64] = pexp.T @ vproj_h per head; normalize via DVE
      tensor_tensor with per-head recip broadcast -> o bf16
  T : oT via one batched DMA transpose per [128,512] tile
  E : out[n,1024] = oT.T-chunks @ Wproj_g, bf16 store via gpsimd SWDGE

B(nb+1), E(nb-1), D(nb) are hand-interleaved in the PE stream per
n-block so Act exp latency hides under PE matmuls.
"""

import os
import numpy as np

import concourse.bass as bass
import concourse.mybir as mybir
import concourse.tile as tile
from concourse import bacc
from concourse.bass_utils import run_bass_kernel_spmd

P = 128
N, D, K, DG, DH = 4096, 1024, 256, 512, 64
NB = 8                    # n-blocks of 512
HL = 8                    # heads per core
F32 = mybir.dt.float32

MMDT_NAME = os.environ.get("LINF_MMDT", "bfloat16")
MMDT = getattr(mybir.dt, MMDT_NAME)
Exp = mybir.ActivationFunctionType.Exp
# fp8(e4m3) attention-value path with DoubleRow matmuls (0.5 cycles/row)
D_FP8 = os.environ.get("LINF_D_FP8", "0") == "1"
# fp8 output-projection path: wproj scaled x64 on host (values are
# subnormal-small in fp8 otherwise), un-scaled after the host gather
E_FP8 = os.environ.get("LINF_E_FP8", "0") == "1"
F8 = mybir.dt.float8e4

_cache = {}


def build_nc():
    nc = bacc.Bacc(None, target_bir_lowering=False, debug=False)

    x_d = nc.dram_tensor("x", [N, D], MMDT, kind="ExternalInput")
    xt_d = nc.dram_tensor("xt", [D, N], MMDT, kind="ExternalInput")
    pkv_d = nc.dram_tensor("projkv", [N, 2 * K], MMDT, kind="ExternalInput")
    wq_d = nc.dram_tensor("wq", [D, DG], MMDT, kind="ExternalInput")
    wk_d = nc.dram_tensor("wk", [D, DG], MMDT, kind="ExternalInput")
    wv_d = nc.dram_tensor("wv", [D, DG], MMDT, kind="ExternalInput")
    wp_d = nc.dram_tensor("wproj", [DG, D], F8 if E_FP8 else MMDT,
                          kind="ExternalInput")
    out_d = nc.dram_tensor("out", [N, D], MMDT, kind="ExternalOutput")

    with tile.TileContext(nc) as tc:
        from contextlib import ExitStack
        with ExitStack() as ctx:
            res = ctx.enter_context(tc.tile_pool(name="res", bufs=1))
            ones_sb = res.tile([P, 1], MMDT, tag="ones")
            nc.vector.memset(ones_sb[:], 1.0)
            if D_FP8:
                ones2_sb = res.tile([P, 2], F8, tag="ones2")
                nc.vector.memset(ones2_sb[:], 1.0)
                nbias_sb = res.tile([P, 1], F32, tag="nbias")
                nc.vector.memset(nbias_sb[:], -1.5)
            from concourse.masks import make_identity
            id_mm = res.tile([P, P], MMDT, tag="id_mm")
            make_identity(nc, id_mm[:])

            wq_sb = res.tile([P, 8 * DG], MMDT, tag="wq")
            wk_sb = res.tile([P, 8 * DG], MMDT, tag="wk")
            wv_sb = res.tile([P, 8 * DG], MMDT, tag="wv")
            wproj_sb = res.tile([P, 4 * D], F8 if E_FP8 else MMDT,
                                tag="wproj")
            kprojT_sb = res.tile([P, 4 * K], MMDT, tag="kprojT")
            vproj_sb = res.tile([P, 2 * DG], F8 if D_FP8 else MMDT, tag="vproj")
            xcxv_sb = res.tile([P, 8 * 2 * K], MMDT, tag="xcxv")

            # rolling pools for the merged loop
            xtp = ctx.enter_context(tc.tile_pool(name="xtp", bufs=4))
            qtp = ctx.enter_context(tc.tile_pool(name="qtp", bufs=2))
            pexp_p = ctx.enter_context(tc.tile_pool(name="pexp", bufs=2))
            op_ = ctx.enter_context(tc.tile_pool(name="op", bufs=8))
            otp = ctx.enter_context(tc.tile_pool(name="otp", bufs=8))
            outp = ctx.enter_context(tc.tile_pool(name="outp", bufs=3))
            rcp = ctx.enter_context(tc.tile_pool(name="rcp", bufs=2))

            def load_w(dst, src, nchunk, w):
                # dst[p, c*w + j] = src[c*128 + p, j]
                nc.sync.dma_start(
                    out=dst[:].rearrange("p (c j) -> p c j", c=nchunk),
                    in_=src[:, :].rearrange("(c p) j -> p c j", p=P))

            def load_xt(nb, split=False):
                xt = xtp.tile([P, 8 * DG], MMDT, tag="xt", name=f"xt{nb}")
                nhalf = 2 if split else 1
                for hf in range(nhalf):
                    dph = 8 // nhalf
                    nc.sync.dma_start(
                        out=xt[:, hf * dph * DG:(hf + 1) * dph * DG]
                            .rearrange("p (d j) -> p d j", d=dph),
                        in_=xt_d[hf * dph * P:(hf + 1) * dph * P,
                                 nb * DG:(nb + 1) * DG]
                            .rearrange("(d p) j -> p d j", p=P))
                return xt

            def b_block(xt, qt, jc):
                accq = accp.tile([P, DG], F32, tag="acc")
                for dd in range(8):
                    nc.tensor.matmul(
                        accq[:],
                        lhsT=wq_sb[:, dd * DG + jc * P: dd * DG + (jc + 1) * P],
                        rhs=xt[:, dd * DG:(dd + 1) * DG],
                        start=(dd == 0), stop=(dd == 7))
                nc.vector.tensor_copy(qt[:, jc * DG:(jc + 1) * DG], accq[:])

            # ---------------- Phase A ----------------
            with ExitStack() as actx:
                xin = actx.enter_context(tc.tile_pool(name="xin", bufs=4))
                pa_ctx = ExitStack()
                pa = pa_ctx.enter_context(tc.tile_pool(name="pa", bufs=1, space="PSUM"))
                accs = [pa.tile([P, 2 * K], F32, tag=f"pa{dd}", name=f"pa{dd}")
                        for dd in range(8)]
                # PE p-state warm-up: keep the tensor engine continuously
                # busy from t~0.3us so the clock is fully ramped (needs 3us
                # of busy) by the time the first x chunk lands (~2.9us).
                # Output goes to a PSUM region that the first real
                # accumulation group resets (start=True).
                for _ in range(16):
                    nc.tensor.matmul(accs[0][:1, :P], lhsT=ones_sb[:],
                                     rhs=id_mm[:], start=True, stop=True)
                # First chunk loads alone (small, fast) so PE starts ASAP;
                # all weight/xt loads go after the 8 batches — the DMA
                # transfer path is a serial resource and phase A is tight.
                for b4 in range(8):
                    if b4 == 0:
                        # per-chunk interleaved loads: chunk c usable as soon
                        # as its own pair of small DMAs lands
                        x4 = xin.tile([P, 4 * D], MMDT, tag="x4")
                        kv4 = xin.tile([P, 4 * 2 * K], MMDT, tag="kv4")
                        # chunk 0 split by dd-slice: the first matmul only
                        # needs x[0:128, 0:128] + kv chunk 0
                        for dd in range(4):
                            nc.sync.dma_start(
                                out=x4[:, dd * 2 * P:(dd + 1) * 2 * P],
                                in_=x_d[0:P, dd * 2 * P:(dd + 1) * 2 * P])
                            if dd == 0:
                                nc.sync.dma_start(out=kv4[:, :2 * K],
                                                  in_=pkv_d[0:P, :])
                        for c in range(1, 4):
                            nc.sync.dma_start(out=x4[:, c * D:(c + 1) * D],
                                              in_=x_d[c * P:(c + 1) * P, :])
                            nc.sync.dma_start(
                                out=kv4[:, c * 2 * K:(c + 1) * 2 * K],
                                in_=pkv_d[c * P:(c + 1) * P, :])
                    elif b4 == 1:
                        # 2+2 split: first half arrives before the PE (still
                        # in p-state ramp) finishes batch 0
                        x4 = xin.tile([P, 4 * D], MMDT, tag="x4")
                        kv4 = xin.tile([P, 4 * 2 * K], MMDT, tag="kv4")
                        for hf in range(2):
                            r0 = 512 + hf * 256
                            nc.sync.dma_start(
                                out=x4[:, hf * 2 * D:(hf + 1) * 2 * D]
                                    .rearrange("p (c j) -> p c j", c=2),
                                in_=x_d[r0:r0 + 256, :]
                                    .rearrange("(c p) j -> p c j", p=P))
                            nc.sync.dma_start(
                                out=kv4[:, hf * 4 * K:(hf + 1) * 4 * K]
                                    .rearrange("p (c j) -> p c j", c=2),
                                in_=pkv_d[r0:r0 + 256, :]
                                    .rearrange("(c p) j -> p c j", p=P))
                    else:
                        x4 = xin.tile([P, 4 * D], MMDT, tag="x4")
                        kv4 = xin.tile([P, 4 * 2 * K], MMDT, tag="kv4")
                        nc.sync.dma_start(
                            out=x4[:].rearrange("p (c j) -> p c j", c=4),
                            in_=x_d[b4 * 512:(b4 + 1) * 512, :]
                                .rearrange("(c p) j -> p c j", p=P))
                        nc.sync.dma_start(
                            out=kv4[:].rearrange("p (c j) -> p c j", c=4),
                            in_=pkv_d[b4 * 512:(b4 + 1) * 512, :]
                                .rearrange("(c p) j -> p c j", p=P))
                    if b4 == 5:
                        # slot wq+xt0 here: the serialized DMA path has slack
                        # against the PE by now, and B(0) needs them right
                        # at the end of phase A
                        load_w(wq_sb, wq_d, 8, DG)
                        xt_tiles = {0: load_xt(0, split=True)}
                    for c in range(4):
                        nn = b4 * 4 + c
                        for dd in range(8):
                            nc.tensor.matmul(
                                accs[dd][:],
                                lhsT=x4[:, c * D + dd * P: c * D + (dd + 1) * P],
                                rhs=kv4[:, c * 2 * K:(c + 1) * 2 * K],
                                start=(nn == 0), stop=(nn == 31))
                # ordered by first use: wk/wv (A2), wproj (E(0)), xt1 (B(1))
                load_w(wk_sb, wk_d, 8, DG)
                load_w(wv_sb, wv_d, 8, DG)
                load_w(wproj_sb, wp_d, 4, D)
                xt_tiles[1] = load_xt(1)
                for dd in range(8):
                    eng = nc.vector if dd % 2 else nc.scalar
                    if dd % 2:
                        nc.vector.tensor_copy(
                            xcxv_sb[:, dd * 2 * K:(dd + 1) * 2 * K], accs[dd][:])
                    else:
                        nc.scalar.copy(
                            out=xcxv_sb[:, dd * 2 * K:(dd + 1) * 2 * K],
                            in_=accs[dd][:])

                # Phase A2 — release the A accumulators' banks first.
                # B(0) blocks are interleaved with the A2 groups: they fill
                # the PE while A2 waits on the xcxv eviction pipeline.
                pa_ctx.close()
                pa2 = actx.enter_context(tc.tile_pool(name="pa2", bufs=2, space="PSUM"))
                accp = ctx.enter_context(tc.tile_pool(name="accp", bufs=3,
                                                      space="PSUM", side="right"))
                qts = {0: qtp.tile([P, 4 * DG], MMDT, tag="qt", name="qt0")}
                for jc in range(4):
                    # B(0) first: it has no dependence on the A evictions, so
                    # it covers the xcxv eviction pipeline latency
                    b_block(xt_tiles[0], qts[0], jc)
                    acc = pa2.tile([P, K], F32, tag="kpj")
                    for dd in range(8):
                        nc.tensor.matmul(
                            acc[:],
                            lhsT=wk_sb[:, dd * DG + jc * P: dd * DG + (jc + 1) * P],
                            rhs=xcxv_sb[:, dd * 2 * K: dd * 2 * K + K],
                            start=(dd == 0), stop=(dd == 7))
                    if jc % 2:
                        nc.scalar.copy(out=kprojT_sb[:, jc * K:(jc + 1) * K],
                                       in_=acc[:])
                    else:
                        nc.vector.tensor_copy(kprojT_sb[:, jc * K:(jc + 1) * K],
                                              acc[:])
                for fc in range(2):
                    acc2 = pa2.tile([P, DG], F32, tag="vpj")
                    for dd in range(8):
                        nc.tensor.matmul(
                            acc2[:],
                            lhsT=xcxv_sb[:, dd * 2 * K + K + fc * P:
                                         dd * 2 * K + K + (fc + 1) * P],
                            rhs=wv_sb[:, dd * DG:(dd + 1) * DG],
                            start=(dd == 0), stop=(dd == 7))
                    if fc:
                        nc.scalar.copy(out=vproj_sb[:, fc * DG:(fc + 1) * DG],
                                       in_=acc2[:])
                    else:
                        nc.vector.tensor_copy(vproj_sb[:, fc * DG:(fc + 1) * DG],
                                              acc2[:])

            # ---------------- merged loop pools (PSUM) ----------------
            # scp last on the left stack: it is released after the final
            # score block to make room for the epilogue transpose pool
            pop = ctx.enter_context(tc.tile_pool(name="pop", bufs=2, space="PSUM"))
            smp = ctx.enter_context(tc.tile_pool(name="smp", bufs=1, space="PSUM"))
            scp_ctx = ExitStack()
            scp = scp_ctx.enter_context(tc.tile_pool(name="scp", bufs=2,
                                                     space="PSUM"))

            def sc_block(qt, h, pexps):
                jc, p0 = h // 2, (h % 2) * DH
                if D_FP8:
                    pexps[h] = pexp_p.tile([P, 2 * DG], F8, tag=f"px{h}",
                                           name=f"px{h}")
                for fc in range(2):
                    st = scp.tile([P, DG], F32, tag="sc")
                    nc.tensor.matmul(
                        st[:],
                        lhsT=kprojT_sb[p0:p0 + DH,
                                       jc * K + fc * P: jc * K + (fc + 1) * P],
                        rhs=qt[p0:p0 + DH, jc * DG:(jc + 1) * DG],
                        start=True, stop=True)
                    if D_FP8:
                        # shifted exp keeps values well inside fp8e4 range;
                        # softmax is shift-invariant and the sums are computed
                        # from the same shifted values, so this is exact
                        nc.scalar.activation(
                            pexps[h][:, fc * DG:(fc + 1) * DG], st[:], Exp,
                            bias=nbias_sb[:])
                    else:
                        pexp = pexp_p.tile([P, DG], MMDT, tag=f"px{h}_{fc}")
                        nc.scalar.activation(pexp[:], st[:], Exp)
                        pexps[(h, fc)] = pexp

            def d_group(nb, nn2, pexps, sp, recips, skip_t=False):
                po = pop.tile([P, DG], F32, tag="po")
                for h in range(HL):
                    if D_FP8:
                        # fp8 DoubleRow: both k-chunks (fc) in one matmul at
                        # 0.5 cycles/row — out = sum_f lhsT[:,f].T @ rhs[:,f]
                        px3 = pexps[h][:].rearrange("p (f n) -> p f n", f=2)
                        v3 = vproj_sb[:].rearrange("p (f c) -> p f c", f=2)
                        nc.tensor.matmul(
                            po[:, h * DH:(h + 1) * DH],
                            lhsT=px3[:, :, nn2 * P:(nn2 + 1) * P],
                            rhs=v3[:, :, h * DH:(h + 1) * DH],
                            start=True, stop=True,
                            perf_mode=mybir.MatmulPerfMode.DoubleRow)
                        nc.tensor.matmul(
                            sp[:, nn2 * HL + h: nn2 * HL + h + 1],
                            lhsT=px3[:, :, nn2 * P:(nn2 + 1) * P],
                            rhs=ones2_sb[:].rearrange("p (f o) -> p f o", f=2),
                            start=True, stop=True,
                            perf_mode=mybir.MatmulPerfMode.DoubleRow)
                        continue
                    for fc in range(2):
                        px = pexps[(h, fc)]
                        nc.tensor.matmul(
                            po[:, h * DH:(h + 1) * DH],
                            lhsT=px[:, nn2 * P:(nn2 + 1) * P],
                            rhs=vproj_sb[:, fc * DG + h * DH:
                                         fc * DG + (h + 1) * DH],
                            start=(fc == 0), stop=(fc == 1))
                        nc.tensor.matmul(
                            sp[:, nn2 * HL + h: nn2 * HL + h + 1],
                            lhsT=px[:, nn2 * P:(nn2 + 1) * P],
                            rhs=ones_sb[:],
                            start=(fc == 0), stop=(fc == 1))
                nc.vector.reciprocal(
                    recips[:, nn2 * HL:(nn2 + 1) * HL],
                    sp[:, nn2 * HL:(nn2 + 1) * HL])
                o_t = op_.tile([P, DG], F8 if E_FP8 else MMDT, tag="o",
                               name=f"o{nb}_{nn2}")
                nc.vector.tensor_tensor(
                    out=o_t[:].rearrange("p (h j) -> p h j", h=HL),
                    in0=po[:].rearrange("p (h j) -> p h j", h=HL),
                    in1=recips[:, nn2 * HL:(nn2 + 1) * HL]
                        .broadcast_to([P, HL, DH]),
                    op=mybir.AluOpType.mult)
                if skip_t:
                    return o_t
                if E_FP8:
                    # PE transpose (1-byte dtypes can't use the DMA-transpose
                    # path); the psum target reuses a score-pool slot via
                    # bitcast, so no extra PSUM banks are needed
                    tr = scp.tile([P, DG], F32, tag="sc", name=f"tr{nb}_{nn2}")
                    tr8 = tr[:].bitcast(F8)
                    ot = otp.tile([P, DG], F8, tag="ot", name=f"ot{nb}_{nn2}")
                    for c in range(4):
                        nc.tensor.transpose(tr8[:, c * P:(c + 1) * P],
                                            o_t[:, c * P:(c + 1) * P],
                                            id_mm[:])
                    if nn2 % 2:
                        nc.scalar.copy(out=ot[:], in_=tr8[:, :DG])
                    else:
                        nc.vector.tensor_copy(ot[:], tr8[:, :DG])
                    return ot
                ot = otp.tile([P, DG], MMDT, tag="ot", name=f"ot{nb}_{nn2}")
                nc.sync.dma_start_transpose(
                    out=ot[:].rearrange("p (c j) -> p c j", c=4),
                    in_=o_t[:])
                return ot

            def e_group(nb, nn2, ot, last=False, store_eng=None):
                ci = nb * 4 + nn2
                outsb = outp.tile([P, D], MMDT, tag="outsb")
                ot3 = ot[:].rearrange("p (c n) -> p c n", c=4)
                wp3 = wproj_sb[:].rearrange("p (c d) -> p c d", c=4)
                for half in range(2):
                    pe_acc = accp.tile([P, DG], F32, tag="acc")
                    if E_FP8:
                        for pr in range(2):
                            nc.tensor.matmul(
                                pe_acc[:],
                                lhsT=ot3[:, 2 * pr:2 * pr + 2, :],
                                rhs=wp3[:, 2 * pr:2 * pr + 2,
                                        half * DG:(half + 1) * DG],
                                start=(pr == 0), stop=(pr == 1),
                                perf_mode=mybir.MatmulPerfMode.DoubleRow)
                    else:
                        for jc2 in range(4):
                            nc.tensor.matmul(
                                pe_acc[:],
                                lhsT=ot[:, jc2 * P:(jc2 + 1) * P],
                                rhs=wproj_sb[:, jc2 * D + half * DG:
                                             jc2 * D + (half + 1) * DG],
                                start=(jc2 == 0), stop=(jc2 == 3))
                    if last:
                        # fast tail: evict on both engines, store each half as
                        # soon as it lands (HWDGE has lower fixed latency)
                        if half == 0:
                            nc.scalar.copy(out=outsb[:, :DG], in_=pe_acc[:])
                        else:
                            nc.vector.tensor_copy(outsb[:, DG:], pe_acc[:])
                        nc.sync.dma_start(
                            out=out_d[ci * P:(ci + 1) * P,
                                      half * DG:(half + 1) * DG],
                            in_=outsb[:, half * DG:(half + 1) * DG])
                    else:
                        nc.vector.tensor_copy(
                            outsb[:, half * DG:(half + 1) * DG], pe_acc[:])
                if not last:
                    (store_eng or nc.sync).dma_start(
                        out=out_d[ci * P:(ci + 1) * P, :], in_=outsb[:])

            # ---------------- merged loop (nb = 0..6) ----------------
            xt_tiles[2] = load_xt(2)
            prev_ots = None
            pexps7 = {}
            for nb in range(NB - 1):
                if nb + 3 < NB:
                    xt_tiles[nb + 3] = load_xt(nb + 3)
                pexps = {}
                cur_ots = []
                sp = smp.tile([P, 4 * HL], F32, tag="sums")
                recips = rcp.tile([P, 4 * HL], F32, tag="recips")
                qts[nb + 1] = qtp.tile([P, 4 * DG], MMDT, tag="qt",
                                       name=f"qt{nb + 1}")
                # interleave scores(nb) with B(nb+1) on the PE stream
                for h in range(HL):
                    sc_block(qts[nb], h, pexps)
                    if h % 2 == 1:
                        b_block(xt_tiles[nb + 1], qts[nb + 1], h // 2)
                if prev_ots is not None:
                    for nn2 in range(4):
                        e_group(nb - 1, nn2, prev_ots[nn2])
                for nn2 in range(4):
                    cur_ots.append(d_group(nb, nn2, pexps, sp, recips))
                if nb == NB - 2:
                    # hoist scores(7) into this iter's tail so its exps have
                    # drained before D(7) runs next iter
                    for h in range(HL):
                        sc_block(qts[NB - 1], h, pexps7)
                prev_ots = cur_ots
            # ---- last iter: E(6,3) placed after D(7) to cover latency;
            # o-transposes for block 7 run on the PE (via the freed score
            # banks) instead of the ~3us-latency DMA-transpose path
            scp_ctx.close()
            trp = ctx.enter_context(tc.tile_pool(name="trp", bufs=2,
                                                 space="PSUM"))
            sp = smp.tile([P, 4 * HL], F32, tag="sums")
            recips = rcp.tile([P, 4 * HL], F32, tag="recips")
            for nn2 in range(3):
                e_group(NB - 2, nn2, prev_ots[nn2])
            o7 = [d_group(NB - 1, nn2, pexps7, sp, recips, skip_t=True)
                  for nn2 in range(4)]
            e_group(NB - 2, 3, prev_ots[3], store_eng=nc.scalar)
            ots7 = []
            for nn2 in range(4):
                tr = trp.tile([P, DG], F8 if E_FP8 else MMDT, tag="tr")
                for c in range(4):
                    nc.tensor.transpose(tr[:, c * P:(c + 1) * P],
                                        o7[nn2][:, c * P:(c + 1) * P],
                                        id_mm[:])
                ot = otp.tile([P, DG], F8 if E_FP8 else MMDT, tag="ot",
                              name=f"otz{nn2}")
                nc.scalar.copy(out=ot[:], in_=tr[:])
                ots7.append(ot)
            for nn2 in range(4):
                e_group(NB - 1, nn2, ots7[nn2], last=True)
    nc.compile()
    return nc


def _np_mm(a):
    return np.ascontiguousarray(np.asarray(a), dtype=mybir.dt.np(MMDT))


def kernel(x, Wq, Wkv, Wproj, bproj, proj_k, proj_v):
    x = np.asarray(x)
    Wq, Wkv, Wproj = np.asarray(Wq), np.asarray(Wkv), np.asarray(Wproj)
    bproj, proj_k, proj_v = np.asarray(bproj), np.asarray(proj_k), np.asarray(proj_v)

    if "nc" not in _cache:
        _cache["nc"] = build_nc()
    nc = _cache["nc"]

    scale = np.float32(DH ** -0.5)
    projkv = _np_mm(np.concatenate([proj_k, proj_v], axis=1))
    in_maps = []
    for c in range(8):
        b, g = c // 2, c % 2
        cols = slice(g * DG, (g + 1) * DG)
        xb = _np_mm(x[b])
        in_maps.append({
            "x": xb,
            "xt": np.ascontiguousarray(xb.T),
            "projkv": projkv,
            "wq": _np_mm(scale * Wq[:, cols]),
            "wk": _np_mm(Wkv[:, :D][:, cols]),
            "wv": _np_mm(Wkv[:, D:][:, cols]),
            "wproj": (np.ascontiguousarray(
                64.0 * Wproj[cols, :], dtype=mybir.dt.np(F8))
                if E_FP8 else _np_mm(Wproj[cols, :])),
        })
    res = run_bass_kernel_spmd(nc, in_maps, list(range(8)),
                               trace=bool(os.environ.get("LINF_TRACE")))
    _cache["last_result"] = res
    oscale = np.float32(1.0 / 64.0) if E_FP8 else np.float32(1.0)
    outs = [oscale * np.asarray(r["out"], dtype=np.float32)
            for r in res.results]
    full = np.stack([outs[2 * b] + outs[2 * b + 1] for b in range(4)])
    full = full + np.asarray(bproj, np.float32)
    return full.astype(np.float32)


# revision 89
# speedup vs baseline: 1.3046x; 1.0032x over previous
"""Linformer self-attention on 8 Trainium2 NeuronCores.

Problem (hardcoded shapes): x [4,4096,1024] f32; per batch:
  q = scale*(x@Wq); kv = x@Wkv; keys/values compressed 4096->256 via
  proj_k/proj_v; 16-head attention (dh=64, k=256); out @ Wproj + bproj.

Sharding: 8 cores = 4 batches x 2 head-groups (8 heads / 512 cols each).
Each core computes a partial [4096,1024] output (Wproj row-split); host
sums the pair and adds bias.

Per-core dataflow (all matmuls use out = lhsT.T @ rhs, K<=128 partitions):
  A : xcxvT[1024,512] = x.T @ [proj_k|proj_v]      (contract n, x natural)
  A2: kprojT[512,256] = Wk_g.T @ xcT ; vproj[256,512] = xvT.T @ Wv_g
  B : qT[512,4096] = Wq_g.T @ xT    (xT provided by host, plain DMA)
  C : per (head,fc k-chunk): scoresT[128,512] -> exp (Act) -> pexp bf16
  S : per (n-chunk, head): sums[n,1] = pexp.T @ ones  (N=1 matmuls)
  D : po[n, 8*64] = pexp.T @ vproj_h per head; normalize via DVE
      tensor_tensor with per-head recip broadcast -> o bf16
  T : oT via one batched DMA transpose per [128,512] tile (PE transpose
      through the released score banks for the final block)
  E : out[n,1024] = oT.T-chunks @ Wproj_g, bf16 stores on SP HWDGE

B(nb+1), E(nb-1), D(nb) are hand-interleaved in the PE stream per
n-block so Act exp latency and PSUM-recycle chains hide under PE
matmuls; scores(7) are hoisted into iteration 6 and the last batch of
phase A runs dd-major so evictions overlap its tail.
"""

import os
import numpy as np

import concourse.bass as bass
import concourse.mybir as mybir
import concourse.tile as tile
from concourse import bacc
from concourse.bass_utils import run_bass_kernel_spmd

P = 128
N, D, K, DG, DH = 4096, 1024, 256, 512, 64
NB = 8                    # n-blocks of 512
HL = 8                    # heads per core
F32 = mybir.dt.float32

MMDT_NAME = os.environ.get("LINF_MMDT", "bfloat16")
MMDT = getattr(mybir.dt, MMDT_NAME)
Exp = mybir.ActivationFunctionType.Exp
# fp8(e4m3) attention-value path with DoubleRow matmuls (0.5 cycles/row)
D_FP8 = os.environ.get("LINF_D_FP8", "0") == "1"
# fp8 output-projection path: wproj scaled x64 on host (values are
# subnormal-small in fp8 otherwise), un-scaled after the host gather
E_FP8 = os.environ.get("LINF_E_FP8", "0") == "1"
F8 = mybir.dt.float8e4

_cache = {}


def build_nc():
    nc = bacc.Bacc(None, target_bir_lowering=False, debug=False)

    x_d = nc.dram_tensor("x", [N, D], MMDT, kind="ExternalInput")
    xt_d = nc.dram_tensor("xt", [D, N], MMDT, kind="ExternalInput")
    pkv_d = nc.dram_tensor("projkv", [N, 2 * K], MMDT, kind="ExternalInput")
    wq_d = nc.dram_tensor("wq", [D, DG], MMDT, kind="ExternalInput")
    wk_d = nc.dram_tensor("wk", [D, DG], MMDT, kind="ExternalInput")
    wv_d = nc.dram_tensor("wv", [D, DG], MMDT, kind="ExternalInput")
    wp_d = nc.dram_tensor("wproj", [DG, D], F8 if E_FP8 else MMDT,
                          kind="ExternalInput")
    out_d = nc.dram_tensor("out", [N, D], MMDT, kind="ExternalOutput")

    with tile.TileContext(nc) as tc:
        from contextlib import ExitStack
        with ExitStack() as ctx:
            res = ctx.enter_context(tc.tile_pool(name="res", bufs=1))
            ones_sb = res.tile([P, 1], MMDT, tag="ones")
            nc.vector.memset(ones_sb[:], 1.0)
            if D_FP8:
                ones2_sb = res.tile([P, 2], F8, tag="ones2")
                nc.vector.memset(ones2_sb[:], 1.0)
                nbias_sb = res.tile([P, 1], F32, tag="nbias")
                nc.vector.memset(nbias_sb[:], -1.5)
            from concourse.masks import make_identity
            id_mm = res.tile([P, P], MMDT, tag="id_mm")
            make_identity(nc, id_mm[:])

            wq_sb = res.tile([P, 8 * DG], MMDT, tag="wq")
            wk_sb = res.tile([P, 8 * DG], MMDT, tag="wk")
            wv_sb = res.tile([P, 8 * DG], MMDT, tag="wv")
            wproj_sb = res.tile([P, 4 * D], F8 if E_FP8 else MMDT,
                                tag="wproj")
            kprojT_sb = res.tile([P, 4 * K], MMDT, tag="kprojT")
            vproj_sb = res.tile([P, 2 * DG], F8 if D_FP8 else MMDT, tag="vproj")
            xcxv_sb = res.tile([P, 8 * 2 * K], MMDT, tag="xcxv")

            # rolling pools for the merged loop
            xtp = ctx.enter_context(tc.tile_pool(name="xtp", bufs=4))
            qtp = ctx.enter_context(tc.tile_pool(name="qtp", bufs=2))
            pexp_p = ctx.enter_context(tc.tile_pool(name="pexp", bufs=2))
            op_ = ctx.enter_context(tc.tile_pool(name="op", bufs=8))
            otp = ctx.enter_context(tc.tile_pool(name="otp", bufs=8))
            outp = ctx.enter_context(tc.tile_pool(name="outp", bufs=3))
            rcp = ctx.enter_context(tc.tile_pool(name="rcp", bufs=2))

            def load_w(dst, src, nchunk, w):
                # dst[p, c*w + j] = src[c*128 + p, j]
                nc.sync.dma_start(
                    out=dst[:].rearrange("p (c j) -> p c j", c=nchunk),
                    in_=src[:, :].rearrange("(c p) j -> p c j", p=P))

            def load_xt(nb, split=False):
                xt = xtp.tile([P, 8 * DG], MMDT, tag="xt", name=f"xt{nb}")
                nhalf = 2 if split else 1
                for hf in range(nhalf):
                    dph = 8 // nhalf
                    nc.sync.dma_start(
                        out=xt[:, hf * dph * DG:(hf + 1) * dph * DG]
                            .rearrange("p (d j) -> p d j", d=dph),
                        in_=xt_d[hf * dph * P:(hf + 1) * dph * P,
                                 nb * DG:(nb + 1) * DG]
                            .rearrange("(d p) j -> p d j", p=P))
                return xt

            def b_block(xt, qt, jc):
                accq = accp.tile([P, DG], F32, tag="acc")
                for dd in range(8):
                    nc.tensor.matmul(
                        accq[:],
                        lhsT=wq_sb[:, dd * DG + jc * P: dd * DG + (jc + 1) * P],
                        rhs=xt[:, dd * DG:(dd + 1) * DG],
                        start=(dd == 0), stop=(dd == 7))
                nc.vector.tensor_copy(qt[:, jc * DG:(jc + 1) * DG], accq[:])

            # ---------------- Phase A ----------------
            with ExitStack() as actx:
                xin = actx.enter_context(tc.tile_pool(name="xin", bufs=4))
                pa_ctx = ExitStack()
                pa = pa_ctx.enter_context(tc.tile_pool(name="pa", bufs=1, space="PSUM"))
                accs = [pa.tile([P, 2 * K], F32, tag=f"pa{dd}", name=f"pa{dd}")
                        for dd in range(8)]
                # PE p-state warm-up: keep the tensor engine continuously
                # busy from t~0.3us so the clock is fully ramped (needs 3us
                # of busy) by the time the first x chunk lands (~2.9us).
                # Output goes to a PSUM region that the first real
                # accumulation group resets (start=True).
                for _ in range(16):
                    nc.tensor.matmul(accs[0][:1, :P], lhsT=ones_sb[:],
                                     rhs=id_mm[:], start=True, stop=True)
                # First chunk loads alone (small, fast) so PE starts ASAP;
                # all weight/xt loads go after the 8 batches — the DMA
                # transfer path is a serial resource and phase A is tight.
                for b4 in range(8):
                    if b4 == 0:
                        # per-chunk interleaved loads: chunk c usable as soon
                        # as its own pair of small DMAs lands
                        x4 = xin.tile([P, 4 * D], MMDT, tag="x4")
                        kv4 = xin.tile([P, 4 * 2 * K], MMDT, tag="kv4")
                        # chunk 0 split by dd-slice: the first matmul only
                        # needs x[0:128, 0:128] + kv chunk 0
                        for dd in range(4):
                            nc.sync.dma_start(
                                out=x4[:, dd * 2 * P:(dd + 1) * 2 * P],
                                in_=x_d[0:P, dd * 2 * P:(dd + 1) * 2 * P])
                            if dd == 0:
                                nc.sync.dma_start(out=kv4[:, :2 * K],
                                                  in_=pkv_d[0:P, :])
                        for c in range(1, 4):
                            nc.sync.dma_start(out=x4[:, c * D:(c + 1) * D],
                                              in_=x_d[c * P:(c + 1) * P, :])
                            nc.sync.dma_start(
                                out=kv4[:, c * 2 * K:(c + 1) * 2 * K],
                                in_=pkv_d[c * P:(c + 1) * P, :])
                    elif b4 == 1:
                        # 2+2 split: first half arrives before the PE (still
                        # in p-state ramp) finishes batch 0
                        x4 = xin.tile([P, 4 * D], MMDT, tag="x4")
                        kv4 = xin.tile([P, 4 * 2 * K], MMDT, tag="kv4")
                        for hf in range(2):
                            r0 = 512 + hf * 256
                            nc.sync.dma_start(
                                out=x4[:, hf * 2 * D:(hf + 1) * 2 * D]
                                    .rearrange("p (c j) -> p c j", c=2),
                                in_=x_d[r0:r0 + 256, :]
                                    .rearrange("(c p) j -> p c j", p=P))
                            nc.sync.dma_start(
                                out=kv4[:, hf * 4 * K:(hf + 1) * 4 * K]
                                    .rearrange("p (c j) -> p c j", c=2),
                                in_=pkv_d[r0:r0 + 256, :]
                                    .rearrange("(c p) j -> p c j", p=P))
                    else:
                        x4 = xin.tile([P, 4 * D], MMDT, tag="x4")
                        kv4 = xin.tile([P, 4 * 2 * K], MMDT, tag="kv4")
                        nc.sync.dma_start(
                            out=x4[:].rearrange("p (c j) -> p c j", c=4),
                            in_=x_d[b4 * 512:(b4 + 1) * 512, :]
                                .rearrange("(c p) j -> p c j", p=P))
                        nc.sync.dma_start(
                            out=kv4[:].rearrange("p (c j) -> p c j", c=4),
                            in_=pkv_d[b4 * 512:(b4 + 1) * 512, :]
                                .rearrange("(c p) j -> p c j", p=P))
                    if b4 == 5:
                        # slot wq+xt0 here: the serialized DMA path has slack
                        # against the PE by now, and B(0) needs them right
                        # at the end of phase A
                        load_w(wq_sb, wq_d, 8, DG)
                        xt_tiles = {0: load_xt(0, split=True)}
                    if b4 == 7:
                        # dd-major on the last batch: acc[dd] stops after its
                        # 4 chunks, so evictions overlap the remaining matmuls
                        for dd in range(8):
                            for c in range(4):
                                nc.tensor.matmul(
                                    accs[dd][:],
                                    lhsT=x4[:, c * D + dd * P:
                                            c * D + (dd + 1) * P],
                                    rhs=kv4[:, c * 2 * K:(c + 1) * 2 * K],
                                    start=False, stop=(c == 3))
                            eng_v = dd % 2
                            if eng_v:
                                nc.vector.tensor_copy(
                                    xcxv_sb[:, dd * 2 * K:(dd + 1) * 2 * K],
                                    accs[dd][:])
                            else:
                                nc.scalar.copy(
                                    out=xcxv_sb[:, dd * 2 * K:(dd + 1) * 2 * K],
                                    in_=accs[dd][:])
                    else:
                        for c in range(4):
                            nn = b4 * 4 + c
                            for dd in range(8):
                                nc.tensor.matmul(
                                    accs[dd][:],
                                    lhsT=x4[:, c * D + dd * P:
                                            c * D + (dd + 1) * P],
                                    rhs=kv4[:, c * 2 * K:(c + 1) * 2 * K],
                                    start=(nn == 0), stop=False)
                # ordered by first use: wk/wv (A2), wproj (E(0)), xt1 (B(1))
                load_w(wk_sb, wk_d, 8, DG)
                load_w(wv_sb, wv_d, 8, DG)
                load_w(wproj_sb, wp_d, 4, D)
                xt_tiles[1] = load_xt(1)
                # Phase A2 — release the A accumulators' banks first.
                # B(0) blocks are interleaved with the A2 groups: they fill
                # the PE while A2 waits on the xcxv eviction pipeline.
                pa_ctx.close()
                pa2 = actx.enter_context(tc.tile_pool(name="pa2", bufs=2, space="PSUM"))
                accp = ctx.enter_context(tc.tile_pool(name="accp", bufs=3,
                                                      space="PSUM", side="right"))
                qts = {0: qtp.tile([P, 4 * DG], MMDT, tag="qt", name="qt0")}
                for jc in range(4):
                    # B(0) first: it has no dependence on the A evictions, so
                    # it covers the xcxv eviction pipeline latency
                    b_block(xt_tiles[0], qts[0], jc)
                    acc = pa2.tile([P, K], F32, tag="kpj")
                    for dd in range(8):
                        nc.tensor.matmul(
                            acc[:],
                            lhsT=wk_sb[:, dd * DG + jc * P: dd * DG + (jc + 1) * P],
                            rhs=xcxv_sb[:, dd * 2 * K: dd * 2 * K + K],
                            start=(dd == 0), stop=(dd == 7))
                    if jc % 2:
                        nc.scalar.copy(out=kprojT_sb[:, jc * K:(jc + 1) * K],
                                       in_=acc[:])
                    else:
                        nc.vector.tensor_copy(kprojT_sb[:, jc * K:(jc + 1) * K],
                                              acc[:])
                for fc in range(2):
                    acc2 = pa2.tile([P, DG], F32, tag="vpj")
                    for dd in range(8):
                        nc.tensor.matmul(
                            acc2[:],
                            lhsT=xcxv_sb[:, dd * 2 * K + K + fc * P:
                                         dd * 2 * K + K + (fc + 1) * P],
                            rhs=wv_sb[:, dd * DG:(dd + 1) * DG],
                            start=(dd == 0), stop=(dd == 7))
                    if fc:
                        nc.scalar.copy(out=vproj_sb[:, fc * DG:(fc + 1) * DG],
                                       in_=acc2[:])
                    else:
                        nc.vector.tensor_copy(vproj_sb[:, fc * DG:(fc + 1) * DG],
                                              acc2[:])

            # ---------------- merged loop pools (PSUM) ----------------
            # scp last on the left stack: it is released after the final
            # score block to make room for the epilogue transpose pool
            pop = ctx.enter_context(tc.tile_pool(name="pop", bufs=2, space="PSUM"))
            smp = ctx.enter_context(tc.tile_pool(name="smp", bufs=1, space="PSUM"))
            scp_ctx = ExitStack()
            scp = scp_ctx.enter_context(tc.tile_pool(name="scp", bufs=2,
                                                     space="PSUM"))

            def sc_block(qt, h, pexps):
                jc, p0 = h // 2, (h % 2) * DH
                if D_FP8:
                    pexps[h] = pexp_p.tile([P, 2 * DG], F8, tag=f"px{h}",
                                           name=f"px{h}")
                for fc in range(2):
                    st = scp.tile([P, DG], F32, tag="sc")
                    nc.tensor.matmul(
                        st[:],
                        lhsT=kprojT_sb[p0:p0 + DH,
                                       jc * K + fc * P: jc * K + (fc + 1) * P],
                        rhs=qt[p0:p0 + DH, jc * DG:(jc + 1) * DG],
                        start=True, stop=True)
                    if D_FP8:
                        # shifted exp keeps values well inside fp8e4 range;
                        # softmax is shift-invariant and the sums are computed
                        # from the same shifted values, so this is exact
                        nc.scalar.activation(
                            pexps[h][:, fc * DG:(fc + 1) * DG], st[:], Exp,
                            bias=nbias_sb[:])
                    else:
                        pexp = pexp_p.tile([P, DG], MMDT, tag=f"px{h}_{fc}")
                        nc.scalar.activation(pexp[:], st[:], Exp)
                        pexps[(h, fc)] = pexp

            def d_group(nb, nn2, pexps, sp, recips, skip_t=False):
                po = pop.tile([P, DG], F32, tag="po")
                for h in range(HL):
                    if D_FP8:
                        # fp8 DoubleRow: both k-chunks (fc) in one matmul at
                        # 0.5 cycles/row — out = sum_f lhsT[:,f].T @ rhs[:,f]
                        px3 = pexps[h][:].rearrange("p (f n) -> p f n", f=2)
                        v3 = vproj_sb[:].rearrange("p (f c) -> p f c", f=2)
                        nc.tensor.matmul(
                            po[:, h * DH:(h + 1) * DH],
                            lhsT=px3[:, :, nn2 * P:(nn2 + 1) * P],
                            rhs=v3[:, :, h * DH:(h + 1) * DH],
                            start=True, stop=True,
                            perf_mode=mybir.MatmulPerfMode.DoubleRow)
                        nc.tensor.matmul(
                            sp[:, nn2 * HL + h: nn2 * HL + h + 1],
                            lhsT=px3[:, :, nn2 * P:(nn2 + 1) * P],
                            rhs=ones2_sb[:].rearrange("p (f o) -> p f o", f=2),
                            start=True, stop=True,
                            perf_mode=mybir.MatmulPerfMode.DoubleRow)
                        continue
                    for fc in range(2):
                        px = pexps[(h, fc)]
                        nc.tensor.matmul(
                            po[:, h * DH:(h + 1) * DH],
                            lhsT=px[:, nn2 * P:(nn2 + 1) * P],
                            rhs=vproj_sb[:, fc * DG + h * DH:
                                         fc * DG + (h + 1) * DH],
                            start=(fc == 0), stop=(fc == 1))
                        nc.tensor.matmul(
                            sp[:, nn2 * HL + h: nn2 * HL + h + 1],
                            lhsT=px[:, nn2 * P:(nn2 + 1) * P],
                            rhs=ones_sb[:],
                            start=(fc == 0), stop=(fc == 1))
                nc.vector.reciprocal(
                    recips[:, nn2 * HL:(nn2 + 1) * HL],
                    sp[:, nn2 * HL:(nn2 + 1) * HL])
                o_t = op_.tile([P, DG], F8 if E_FP8 else MMDT, tag="o",
                               name=f"o{nb}_{nn2}")
                nc.vector.tensor_tensor(
                    out=o_t[:].rearrange("p (h j) -> p h j", h=HL),
                    in0=po[:].rearrange("p (h j) -> p h j", h=HL),
                    in1=recips[:, nn2 * HL:(nn2 + 1) * HL]
                        .broadcast_to([P, HL, DH]),
                    op=mybir.AluOpType.mult)
                if skip_t:
                    return o_t
                if E_FP8:
                    # PE transpose (1-byte dtypes can't use the DMA-transpose
                    # path); the psum target reuses a score-pool slot via
                    # bitcast, so no extra PSUM banks are needed
                    tr = scp.tile([P, DG], F32, tag="sc", name=f"tr{nb}_{nn2}")
                    tr8 = tr[:].bitcast(F8)
                    ot = otp.tile([P, DG], F8, tag="ot", name=f"ot{nb}_{nn2}")
                    for c in range(4):
                        nc.tensor.transpose(tr8[:, c * P:(c + 1) * P],
                                            o_t[:, c * P:(c + 1) * P],
                                            id_mm[:])
                    if nn2 % 2:
                        nc.scalar.copy(out=ot[:], in_=tr8[:, :DG])
                    else:
                        nc.vector.tensor_copy(ot[:], tr8[:, :DG])
                    return ot
                ot = otp.tile([P, DG], MMDT, tag="ot", name=f"ot{nb}_{nn2}")
                nc.sync.dma_start_transpose(
                    out=ot[:].rearrange("p (c j) -> p c j", c=4),
                    in_=o_t[:])
                return ot

            def e_group(nb, nn2, ot, last=False, store_eng=None):
                ci = nb * 4 + nn2
                outsb = outp.tile([P, D], MMDT, tag="outsb")
                ot3 = ot[:].rearrange("p (c n) -> p c n", c=4)
                wp3 = wproj_sb[:].rearrange("p (c d) -> p c d", c=4)
                for half in range(2):
                    pe_acc = accp.tile([P, DG], F32, tag="acc")
                    if E_FP8:
                        for pr in range(2):
                            nc.tensor.matmul(
                                pe_acc[:],
                                lhsT=ot3[:, 2 * pr:2 * pr + 2, :],
                                rhs=wp3[:, 2 * pr:2 * pr + 2,
                                        half * DG:(half + 1) * DG],
                                start=(pr == 0), stop=(pr == 1),
                                perf_mode=mybir.MatmulPerfMode.DoubleRow)
                    else:
                        for jc2 in range(4):
                            nc.tensor.matmul(
                                pe_acc[:],
                                lhsT=ot[:, jc2 * P:(jc2 + 1) * P],
                                rhs=wproj_sb[:, jc2 * D + half * DG:
                                             jc2 * D + (half + 1) * DG],
                                start=(jc2 == 0), stop=(jc2 == 3))
                    if last:
                        # fast tail: evict on both engines, store each half as
                        # soon as it lands (HWDGE has lower fixed latency)
                        if half == 0:
                            nc.scalar.copy(out=outsb[:, :DG], in_=pe_acc[:])
                        else:
                            nc.vector.tensor_copy(outsb[:, DG:], pe_acc[:])
                        nc.sync.dma_start(
                            out=out_d[ci * P:(ci + 1) * P,
                                      half * DG:(half + 1) * DG],
                            in_=outsb[:, half * DG:(half + 1) * DG])
                    else:
                        nc.vector.tensor_copy(
                            outsb[:, half * DG:(half + 1) * DG], pe_acc[:])
                if not last:
                    (store_eng or nc.sync).dma_start(
                        out=out_d[ci * P:(ci + 1) * P, :], in_=outsb[:])

            # ---------------- merged loop (nb = 0..6) ----------------
            xt_tiles[2] = load_xt(2)
            prev_ots = None
            pexps7 = {}
            for nb in range(NB - 1):
                if nb + 3 < NB:
                    xt_tiles[nb + 3] = load_xt(nb + 3)
                pexps = {}
                cur_ots = []
                sp = smp.tile([P, 4 * HL], F32, tag="sums")
                recips = rcp.tile([P, 4 * HL], F32, tag="recips")
                qts[nb + 1] = qtp.tile([P, 4 * DG], MMDT, tag="qt",
                                       name=f"qt{nb + 1}")
                # interleave scores(nb) with B(nb+1) on the PE stream
                for h in range(HL):
                    sc_block(qts[nb], h, pexps)
                    if h % 2 == 1:
                        b_block(xt_tiles[nb + 1], qts[nb + 1], h // 2)
                if prev_ots is not None:
                    for nn2 in range(4):
                        e_group(nb - 1, nn2, prev_ots[nn2])
                for nn2 in range(4):
                    cur_ots.append(d_group(nb, nn2, pexps, sp, recips))
                if nb == NB - 2:
                    # hoist scores(7) into this iter's tail so its exps have
                    # drained before D(7) runs next iter
                    for h in range(HL):
                        sc_block(qts[NB - 1], h, pexps7)
                prev_ots = cur_ots
            # ---- last iter: E(6,3) placed after D(7) to cover latency;
            # o-transposes for block 7 run on the PE (via the freed score
            # banks) instead of the ~3us-latency DMA-transpose path
            scp_ctx.close()
            trp = ctx.enter_context(tc.tile_pool(name="trp", bufs=2,
                                                 space="PSUM"))
            sp = smp.tile([P, 4 * HL], F32, tag="sums")
            recips = rcp.tile([P, 4 * HL], F32, tag="recips")
            for nn2 in range(3):
                e_group(NB - 2, nn2, prev_ots[nn2])
            o7 = [d_group(NB - 1, nn2, pexps7, sp, recips, skip_t=True)
                  for nn2 in range(4)]
            e_group(NB - 2, 3, prev_ots[3], store_eng=nc.scalar)
            ots7 = []
            for nn2 in range(4):
                tr = trp.tile([P, DG], F8 if E_FP8 else MMDT, tag="tr")
                for c in range(4):
                    nc.tensor.transpose(tr[:, c * P:(c + 1) * P],
                                        o7[nn2][:, c * P:(c + 1) * P],
                                        id_mm[:])
                ot = otp.tile([P, DG], F8 if E_FP8 else MMDT, tag="ot",
                              name=f"otz{nn2}")
                nc.scalar.copy(out=ot[:], in_=tr[:])
                ots7.append(ot)
            for nn2 in range(4):
                e_group(NB - 1, nn2, ots7[nn2], last=True)
    nc.compile()
    return nc


def _np_mm(a):
    return np.ascontiguousarray(np.asarray(a), dtype=mybir.dt.np(MMDT))


def kernel(x, Wq, Wkv, Wproj, bproj, proj_k, proj_v):
    x = np.asarray(x)
    Wq, Wkv, Wproj = np.asarray(Wq), np.asarray(Wkv), np.asarray(Wproj)
    bproj, proj_k, proj_v = np.asarray(bproj), np.asarray(proj_k), np.asarray(proj_v)

    if "nc" not in _cache:
        _cache["nc"] = build_nc()
    nc = _cache["nc"]

    scale = np.float32(DH ** -0.5)
    projkv = _np_mm(np.concatenate([proj_k, proj_v], axis=1))
    in_maps = []
    for c in range(8):
        b, g = c // 2, c % 2
        cols = slice(g * DG, (g + 1) * DG)
        xb = _np_mm(x[b])
        in_maps.append({
            "x": xb,
            "xt": np.ascontiguousarray(xb.T),
            "projkv": projkv,
            "wq": _np_mm(scale * Wq[:, cols]),
            "wk": _np_mm(Wkv[:, :D][:, cols]),
            "wv": _np_mm(Wkv[:, D:][:, cols]),
            "wproj": (np.ascontiguousarray(
                64.0 * Wproj[cols, :], dtype=mybir.dt.np(F8))
                if E_FP8 else _np_mm(Wproj[cols, :])),
        })
    res = run_bass_kernel_spmd(nc, in_maps, list(range(8)),
                               trace=bool(os.environ.get("LINF_TRACE")))
    _cache["last_result"] = res
    oscale = np.float32(1.0 / 64.0) if E_FP8 else np.float32(1.0)
    outs = [oscale * np.asarray(r["out"], dtype=np.float32)
            for r in res.results]
    full = np.stack([outs[2 * b] + outs[2 * b + 1] for b in range(4)])
    full = full + np.asarray(bproj, np.float32)
    return full.astype(np.float32)


# revision 93
# speedup vs baseline: 1.3065x; 1.0015x over previous
"""Linformer self-attention on 8 Trainium2 NeuronCores.

Problem (hardcoded shapes): x [4,4096,1024] f32; per batch:
  q = scale*(x@Wq); kv = x@Wkv; keys/values compressed 4096->256 via
  proj_k/proj_v; 16-head attention (dh=64, k=256); out @ Wproj + bproj.

Sharding: 8 cores = 4 batches x 2 head-groups (8 heads / 512 cols each).
Each core computes a partial [4096,1024] output (Wproj row-split); host
sums the pair and adds bias.

Per-core dataflow (all matmuls use out = lhsT.T @ rhs, K<=128 partitions):
  A : xcxvT[1024,512] = x.T @ [proj_k|proj_v]      (contract n, x natural)
  A2: kprojT[512,256] = Wk_g.T @ xcT ; vproj[256,512] = xvT.T @ Wv_g
  B : qT[512,4096] = Wq_g.T @ xT    (xT provided by host, plain DMA)
  C : per (head,fc k-chunk): scoresT[128,512] -> exp (Act) -> pexp bf16
  S : per (n-chunk, head): sums[n,1] = pexp.T @ ones  (N=1 matmuls)
  D : po[n, 8*64] = pexp.T @ vproj_h per head; normalize via DVE
      tensor_tensor with per-head recip broadcast -> o bf16
  T : oT via one batched DMA transpose per [128,512] tile (PE transpose
      through the released score banks for the final block)
  E : out[n,1024] = oT.T-chunks @ Wproj_g, bf16 stores on SP HWDGE

B(nb+1), E(nb-1), D(nb) are hand-interleaved in the PE stream per
n-block so Act exp latency and PSUM-recycle chains hide under PE
matmuls; scores(7) are hoisted into iteration 6 and the last batch of
phase A runs dd-major so evictions overlap its tail.
"""

import os
import numpy as np

import concourse.bass as bass
import concourse.mybir as mybir
import concourse.tile as tile
from concourse import bacc
from concourse.bass_utils import run_bass_kernel_spmd

P = 128
N, D, K, DG, DH = 4096, 1024, 256, 512, 64
NB = 8                    # n-blocks of 512
HL = 8                    # heads per core
F32 = mybir.dt.float32

MMDT_NAME = os.environ.get("LINF_MMDT", "bfloat16")
MMDT = getattr(mybir.dt, MMDT_NAME)
Exp = mybir.ActivationFunctionType.Exp
# fp8(e4m3) attention-value path with DoubleRow matmuls (0.5 cycles/row)
D_FP8 = os.environ.get("LINF_D_FP8", "0") == "1"
# fp8 output-projection path: wproj scaled x64 on host (values are
# subnormal-small in fp8 otherwise), un-scaled after the host gather
E_FP8 = os.environ.get("LINF_E_FP8", "0") == "1"
F8 = mybir.dt.float8e4

_cache = {}


def build_nc():
    nc = bacc.Bacc(None, target_bir_lowering=False, debug=False)

    x_d = nc.dram_tensor("x", [N, D], MMDT, kind="ExternalInput")
    xt_d = nc.dram_tensor("xt", [D, N], MMDT, kind="ExternalInput")
    pkv_d = nc.dram_tensor("projkv", [N, 2 * K], MMDT, kind="ExternalInput")
    wq_d = nc.dram_tensor("wq", [D, DG], MMDT, kind="ExternalInput")
    wk_d = nc.dram_tensor("wk", [D, DG], MMDT, kind="ExternalInput")
    wv_d = nc.dram_tensor("wv", [D, DG], MMDT, kind="ExternalInput")
    wp_d = nc.dram_tensor("wproj", [DG, D], F8 if E_FP8 else MMDT,
                          kind="ExternalInput")
    out_d = nc.dram_tensor("out", [N, D], MMDT, kind="ExternalOutput")

    with tile.TileContext(nc) as tc:
        from contextlib import ExitStack
        with ExitStack() as ctx:
            res = ctx.enter_context(tc.tile_pool(name="res", bufs=1))
            ones_sb = res.tile([P, 1], MMDT, tag="ones")
            nc.vector.memset(ones_sb[:], 1.0)
            if D_FP8:
                ones2_sb = res.tile([P, 2], F8, tag="ones2")
                nc.vector.memset(ones2_sb[:], 1.0)
                nbias_sb = res.tile([P, 1], F32, tag="nbias")
                nc.vector.memset(nbias_sb[:], -1.5)
            from concourse.masks import make_identity
            id_mm = res.tile([P, P], MMDT, tag="id_mm")
            make_identity(nc, id_mm[:])

            wq_sb = res.tile([P, 8 * DG], MMDT, tag="wq")
            wk_sb = res.tile([P, 8 * DG], MMDT, tag="wk")
            wv_sb = res.tile([P, 8 * DG], MMDT, tag="wv")
            wproj_sb = res.tile([P, 4 * D], F8 if E_FP8 else MMDT,
                                tag="wproj")
            kprojT_sb = res.tile([P, 4 * K], MMDT, tag="kprojT")
            vproj_sb = res.tile([P, 2 * DG], F8 if D_FP8 else MMDT, tag="vproj")
            xcxv_sb = res.tile([P, 8 * 2 * K], MMDT, tag="xcxv")

            # rolling pools for the merged loop
            xtp = ctx.enter_context(tc.tile_pool(name="xtp", bufs=4))
            qtp = ctx.enter_context(tc.tile_pool(name="qtp", bufs=2))
            pexp_p = ctx.enter_context(tc.tile_pool(name="pexp", bufs=2))
            op_ = ctx.enter_context(tc.tile_pool(name="op", bufs=8))
            otp = ctx.enter_context(tc.tile_pool(name="otp", bufs=8))
            outp = ctx.enter_context(tc.tile_pool(name="outp", bufs=3))
            rcp = ctx.enter_context(tc.tile_pool(name="rcp", bufs=2))

            def load_w(dst, src, nchunk, w):
                # dst[p, c*w + j] = src[c*128 + p, j]
                nc.sync.dma_start(
                    out=dst[:].rearrange("p (c j) -> p c j", c=nchunk),
                    in_=src[:, :].rearrange("(c p) j -> p c j", p=P))

            def load_xt(nb, split=False):
                xt = xtp.tile([P, 8 * DG], MMDT, tag="xt", name=f"xt{nb}")
                nhalf = 2 if split else 1
                for hf in range(nhalf):
                    dph = 8 // nhalf
                    nc.sync.dma_start(
                        out=xt[:, hf * dph * DG:(hf + 1) * dph * DG]
                            .rearrange("p (d j) -> p d j", d=dph),
                        in_=xt_d[hf * dph * P:(hf + 1) * dph * P,
                                 nb * DG:(nb + 1) * DG]
                            .rearrange("(d p) j -> p d j", p=P))
                return xt

            def b_block(xt, qt, jc):
                accq = accp.tile([P, DG], F32, tag="acc")
                for dd in range(8):
                    nc.tensor.matmul(
                        accq[:],
                        lhsT=wq_sb[:, dd * DG + jc * P: dd * DG + (jc + 1) * P],
                        rhs=xt[:, dd * DG:(dd + 1) * DG],
                        start=(dd == 0), stop=(dd == 7))
                nc.vector.tensor_copy(qt[:, jc * DG:(jc + 1) * DG], accq[:])

            # ---------------- Phase A ----------------
            with ExitStack() as actx:
                xin = actx.enter_context(tc.tile_pool(name="xin", bufs=4))
                pa_ctx = ExitStack()
                pa = pa_ctx.enter_context(tc.tile_pool(name="pa", bufs=1, space="PSUM"))
                accs = [pa.tile([P, 2 * K], F32, tag=f"pa{dd}", name=f"pa{dd}")
                        for dd in range(8)]
                # PE p-state warm-up: keep the tensor engine continuously
                # busy from t~0.3us so the clock is fully ramped (needs 3us
                # of busy) by the time the first x chunk lands (~2.9us).
                # Output goes to a PSUM region that the first real
                # accumulation group resets (start=True).
                for _ in range(11):
                    nc.tensor.matmul(accs[0][:1, :P], lhsT=ones_sb[:],
                                     rhs=id_mm[:], start=True, stop=True)
                # First chunk loads alone (small, fast) so PE starts ASAP;
                # all weight/xt loads go after the 8 batches — the DMA
                # transfer path is a serial resource and phase A is tight.
                for b4 in range(8):
                    if b4 == 0:
                        # per-chunk interleaved loads: chunk c usable as soon
                        # as its own pair of small DMAs lands
                        x4 = xin.tile([P, 4 * D], MMDT, tag="x4")
                        kv4 = xin.tile([P, 4 * 2 * K], MMDT, tag="kv4")
                        # chunk 0 split by dd-slice: the first matmul only
                        # needs x[0:128, 0:128] + kv chunk 0
                        for dd in range(4):
                            nc.sync.dma_start(
                                out=x4[:, dd * 2 * P:(dd + 1) * 2 * P],
                                in_=x_d[0:P, dd * 2 * P:(dd + 1) * 2 * P])
                            if dd == 0:
                                nc.sync.dma_start(out=kv4[:, :2 * K],
                                                  in_=pkv_d[0:P, :])
                        for c in range(1, 4):
                            nc.sync.dma_start(out=x4[:, c * D:(c + 1) * D],
                                              in_=x_d[c * P:(c + 1) * P, :])
                            nc.sync.dma_start(
                                out=kv4[:, c * 2 * K:(c + 1) * 2 * K],
                                in_=pkv_d[c * P:(c + 1) * P, :])
                    elif b4 == 1:
                        # 2+2 split: first half arrives before the PE (still
                        # in p-state ramp) finishes batch 0
                        x4 = xin.tile([P, 4 * D], MMDT, tag="x4")
                        kv4 = xin.tile([P, 4 * 2 * K], MMDT, tag="kv4")
                        for hf in range(2):
                            r0 = 512 + hf * 256
                            nc.sync.dma_start(
                                out=x4[:, hf * 2 * D:(hf + 1) * 2 * D]
                                    .rearrange("p (c j) -> p c j", c=2),
                                in_=x_d[r0:r0 + 256, :]
                                    .rearrange("(c p) j -> p c j", p=P))
                            nc.sync.dma_start(
                                out=kv4[:, hf * 4 * K:(hf + 1) * 4 * K]
                                    .rearrange("p (c j) -> p c j", c=2),
                                in_=pkv_d[r0:r0 + 256, :]
                                    .rearrange("(c p) j -> p c j", p=P))
                    else:
                        x4 = xin.tile([P, 4 * D], MMDT, tag="x4")
                        kv4 = xin.tile([P, 4 * 2 * K], MMDT, tag="kv4")
                        nc.sync.dma_start(
                            out=x4[:].rearrange("p (c j) -> p c j", c=4),
                            in_=x_d[b4 * 512:(b4 + 1) * 512, :]
                                .rearrange("(c p) j -> p c j", p=P))
                        nc.sync.dma_start(
                            out=kv4[:].rearrange("p (c j) -> p c j", c=4),
                            in_=pkv_d[b4 * 512:(b4 + 1) * 512, :]
                                .rearrange("(c p) j -> p c j", p=P))
                    if b4 == 5:
                        # slot wq+xt0 here: the serialized DMA path has slack
                        # against the PE by now, and B(0) needs them right
                        # at the end of phase A
                        load_w(wq_sb, wq_d, 8, DG)
                        xt_tiles = {0: load_xt(0, split=True)}
                    if b4 == 7:
                        # dd-major on the last batch: acc[dd] stops after its
                        # 4 chunks, so evictions overlap the remaining matmuls
                        for dd in range(8):
                            for c in range(4):
                                nc.tensor.matmul(
                                    accs[dd][:],
                                    lhsT=x4[:, c * D + dd * P:
                                            c * D + (dd + 1) * P],
                                    rhs=kv4[:, c * 2 * K:(c + 1) * 2 * K],
                                    start=False, stop=(c == 3))
                            eng_v = dd % 2
                            if eng_v:
                                nc.vector.tensor_copy(
                                    xcxv_sb[:, dd * 2 * K:(dd + 1) * 2 * K],
                                    accs[dd][:])
                            else:
                                nc.scalar.copy(
                                    out=xcxv_sb[:, dd * 2 * K:(dd + 1) * 2 * K],
                                    in_=accs[dd][:])
                    else:
                        for c in range(4):
                            nn = b4 * 4 + c
                            for dd in range(8):
                                nc.tensor.matmul(
                                    accs[dd][:],
                                    lhsT=x4[:, c * D + dd * P:
                                            c * D + (dd + 1) * P],
                                    rhs=kv4[:, c * 2 * K:(c + 1) * 2 * K],
                                    start=(nn == 0), stop=False)
                # ordered by first use: wk/wv (A2), wproj (E(0)), xt1 (B(1))
                load_w(wk_sb, wk_d, 8, DG)
                load_w(wv_sb, wv_d, 8, DG)
                load_w(wproj_sb, wp_d, 4, D)
                xt_tiles[1] = load_xt(1)
                # Phase A2 — release the A accumulators' banks first.
                # B(0) blocks are interleaved with the A2 groups: they fill
                # the PE while A2 waits on the xcxv eviction pipeline.
                pa_ctx.close()
                pa2 = actx.enter_context(tc.tile_pool(name="pa2", bufs=2, space="PSUM"))
                accp = ctx.enter_context(tc.tile_pool(name="accp", bufs=3,
                                                      space="PSUM", side="right"))
                qts = {0: qtp.tile([P, 4 * DG], MMDT, tag="qt", name="qt0")}
                for jc in range(4):
                    # B(0) first: it has no dependence on the A evictions, so
                    # it covers the xcxv eviction pipeline latency
                    b_block(xt_tiles[0], qts[0], jc)
                    acc = pa2.tile([P, K], F32, tag="kpj")
                    for dd in range(8):
                        nc.tensor.matmul(
                            acc[:],
                            lhsT=wk_sb[:, dd * DG + jc * P: dd * DG + (jc + 1) * P],
                            rhs=xcxv_sb[:, dd * 2 * K: dd * 2 * K + K],
                            start=(dd == 0), stop=(dd == 7))
                    if jc % 2:
                        nc.scalar.copy(out=kprojT_sb[:, jc * K:(jc + 1) * K],
                                       in_=acc[:])
                    else:
                        nc.vector.tensor_copy(kprojT_sb[:, jc * K:(jc + 1) * K],
                                              acc[:])
                for fc in range(2):
                    acc2 = pa2.tile([P, DG], F32, tag="vpj")
                    for dd in range(8):
                        nc.tensor.matmul(
                            acc2[:],
                            lhsT=xcxv_sb[:, dd * 2 * K + K + fc * P:
                                         dd * 2 * K + K + (fc + 1) * P],
                            rhs=wv_sb[:, dd * DG:(dd + 1) * DG],
                            start=(dd == 0), stop=(dd == 7))
                    if fc:
                        nc.scalar.copy(out=vproj_sb[:, fc * DG:(fc + 1) * DG],
                                       in_=acc2[:])
                    else:
                        nc.vector.tensor_copy(vproj_sb[:, fc * DG:(fc + 1) * DG],
                                              acc2[:])

            # ---------------- merged loop pools (PSUM) ----------------
            # scp last on the left stack: it is released after the final
            # score block to make room for the epilogue transpose pool
            pop = ctx.enter_context(tc.tile_pool(name="pop", bufs=2, space="PSUM"))
            smp = ctx.enter_context(tc.tile_pool(name="smp", bufs=1, space="PSUM"))
            scp_ctx = ExitStack()
            scp = scp_ctx.enter_context(tc.tile_pool(name="scp", bufs=2,
                                                     space="PSUM"))

            def sc_block(qt, h, pexps):
                jc, p0 = h // 2, (h % 2) * DH
                if D_FP8:
                    pexps[h] = pexp_p.tile([P, 2 * DG], F8, tag=f"px{h}",
                                           name=f"px{h}")
                for fc in range(2):
                    st = scp.tile([P, DG], F32, tag="sc")
                    nc.tensor.matmul(
                        st[:],
                        lhsT=kprojT_sb[p0:p0 + DH,
                                       jc * K + fc * P: jc * K + (fc + 1) * P],
                        rhs=qt[p0:p0 + DH, jc * DG:(jc + 1) * DG],
                        start=True, stop=True)
                    if D_FP8:
                        # shifted exp keeps values well inside fp8e4 range;
                        # softmax is shift-invariant and the sums are computed
                        # from the same shifted values, so this is exact
                        nc.scalar.activation(
                            pexps[h][:, fc * DG:(fc + 1) * DG], st[:], Exp,
                            bias=nbias_sb[:])
                    else:
                        pexp = pexp_p.tile([P, DG], MMDT, tag=f"px{h}_{fc}")
                        nc.scalar.activation(pexp[:], st[:], Exp)
                        pexps[(h, fc)] = pexp

            def d_group(nb, nn2, pexps, sp, recips, skip_t=False):
                po = pop.tile([P, DG], F32, tag="po")
                for h in range(HL):
                    if D_FP8:
                        # fp8 DoubleRow: both k-chunks (fc) in one matmul at
                        # 0.5 cycles/row — out = sum_f lhsT[:,f].T @ rhs[:,f]
                        px3 = pexps[h][:].rearrange("p (f n) -> p f n", f=2)
                        v3 = vproj_sb[:].rearrange("p (f c) -> p f c", f=2)
                        nc.tensor.matmul(
                            po[:, h * DH:(h + 1) * DH],
                            lhsT=px3[:, :, nn2 * P:(nn2 + 1) * P],
                            rhs=v3[:, :, h * DH:(h + 1) * DH],
                            start=True, stop=True,
                            perf_mode=mybir.MatmulPerfMode.DoubleRow)
                        nc.tensor.matmul(
                            sp[:, nn2 * HL + h: nn2 * HL + h + 1],
                            lhsT=px3[:, :, nn2 * P:(nn2 + 1) * P],
                            rhs=ones2_sb[:].rearrange("p (f o) -> p f o", f=2),
                            start=True, stop=True,
                            perf_mode=mybir.MatmulPerfMode.DoubleRow)
                        continue
                    for fc in range(2):
                        px = pexps[(h, fc)]
                        nc.tensor.matmul(
                            po[:, h * DH:(h + 1) * DH],
                            lhsT=px[:, nn2 * P:(nn2 + 1) * P],
                            rhs=vproj_sb[:, fc * DG + h * DH:
                                         fc * DG + (h + 1) * DH],
                            start=(fc == 0), stop=(fc == 1))
                        nc.tensor.matmul(
                            sp[:, nn2 * HL + h: nn2 * HL + h + 1],
                            lhsT=px[:, nn2 * P:(nn2 + 1) * P],
                            rhs=ones_sb[:],
                            start=(fc == 0), stop=(fc == 1))
                nc.vector.reciprocal(
                    recips[:, nn2 * HL:(nn2 + 1) * HL],
                    sp[:, nn2 * HL:(nn2 + 1) * HL])
                o_t = op_.tile([P, DG], F8 if E_FP8 else MMDT, tag="o",
                               name=f"o{nb}_{nn2}")
                nc.vector.tensor_tensor(
                    out=o_t[:].rearrange("p (h j) -> p h j", h=HL),
                    in0=po[:].rearrange("p (h j) -> p h j", h=HL),
                    in1=recips[:, nn2 * HL:(nn2 + 1) * HL]
                        .broadcast_to([P, HL, DH]),
                    op=mybir.AluOpType.mult)
                if skip_t:
                    return o_t
                if E_FP8:
                    # PE transpose (1-byte dtypes can't use the DMA-transpose
                    # path); the psum target reuses a score-pool slot via
                    # bitcast, so no extra PSUM banks are needed
                    tr = scp.tile([P, DG], F32, tag="sc", name=f"tr{nb}_{nn2}")
                    tr8 = tr[:].bitcast(F8)
                    ot = otp.tile([P, DG], F8, tag="ot", name=f"ot{nb}_{nn2}")
                    for c in range(4):
                        nc.tensor.transpose(tr8[:, c * P:(c + 1) * P],
                                            o_t[:, c * P:(c + 1) * P],
                                            id_mm[:])
                    if nn2 % 2:
                        nc.scalar.copy(out=ot[:], in_=tr8[:, :DG])
                    else:
                        nc.vector.tensor_copy(ot[:], tr8[:, :DG])
                    return ot
                ot = otp.tile([P, DG], MMDT, tag="ot", name=f"ot{nb}_{nn2}")
                nc.sync.dma_start_transpose(
                    out=ot[:].rearrange("p (c j) -> p c j", c=4),
                    in_=o_t[:])
                return ot

            def e_group(nb, nn2, ot, last=False, store_eng=None):
                ci = nb * 4 + nn2
                outsb = outp.tile([P, D], MMDT, tag="outsb")
                ot3 = ot[:].rearrange("p (c n) -> p c n", c=4)
                wp3 = wproj_sb[:].rearrange("p (c d) -> p c d", c=4)
                for half in range(2):
                    pe_acc = accp.tile([P, DG], F32, tag="acc")
                    if E_FP8:
                        for pr in range(2):
                            nc.tensor.matmul(
                                pe_acc[:],
                                lhsT=ot3[:, 2 * pr:2 * pr + 2, :],
                                rhs=wp3[:, 2 * pr:2 * pr + 2,
                                        half * DG:(half + 1) * DG],
                                start=(pr == 0), stop=(pr == 1),
                                perf_mode=mybir.MatmulPerfMode.DoubleRow)
                    else:
                        for jc2 in range(4):
                            nc.tensor.matmul(
                                pe_acc[:],
                                lhsT=ot[:, jc2 * P:(jc2 + 1) * P],
                                rhs=wproj_sb[:, jc2 * D + half * DG:
                                             jc2 * D + (half + 1) * DG],
                                start=(jc2 == 0), stop=(jc2 == 3))
                    if last:
                        # fast tail: evict on both engines, store each half as
                        # soon as it lands (HWDGE has lower fixed latency)
                        if half == 0:
                            nc.scalar.copy(out=outsb[:, :DG], in_=pe_acc[:])
                        else:
                            nc.vector.tensor_copy(outsb[:, DG:], pe_acc[:])
                        nc.sync.dma_start(
                            out=out_d[ci * P:(ci + 1) * P,
                                      half * DG:(half + 1) * DG],
                            in_=outsb[:, half * DG:(half + 1) * DG])
                    else:
                        nc.vector.tensor_copy(
                            outsb[:, half * DG:(half + 1) * DG], pe_acc[:])
                if not last:
                    (store_eng or nc.sync).dma_start(
                        out=out_d[ci * P:(ci + 1) * P, :], in_=outsb[:])

            # ---------------- merged loop (nb = 0..6) ----------------
            xt_tiles[2] = load_xt(2)
            prev_ots = None
            pexps7 = {}
            for nb in range(NB - 1):
                if nb + 3 < NB:
                    xt_tiles[nb + 3] = load_xt(nb + 3)
                pexps = {}
                cur_ots = []
                sp = smp.tile([P, 4 * HL], F32, tag="sums")
                recips = rcp.tile([P, 4 * HL], F32, tag="recips")
                qts[nb + 1] = qtp.tile([P, 4 * DG], MMDT, tag="qt",
                                       name=f"qt{nb + 1}")
                # interleave scores(nb) with B(nb+1) on the PE stream
                for h in range(HL):
                    sc_block(qts[nb], h, pexps)
                    if h % 2 == 1:
                        b_block(xt_tiles[nb + 1], qts[nb + 1], h // 2)
                if prev_ots is not None:
                    for nn2 in range(4):
                        e_group(nb - 1, nn2, prev_ots[nn2])
                for nn2 in range(4):
                    cur_ots.append(d_group(nb, nn2, pexps, sp, recips))
                if nb == NB - 2:
                    # hoist scores(7) into this iter's tail so its exps have
                    # drained before D(7) runs next iter
                    for h in range(HL):
                        sc_block(qts[NB - 1], h, pexps7)
                prev_ots = cur_ots
            # ---- last iter: E(6,3) placed after D(7) to cover latency;
            # o-transposes for block 7 run on the PE (via the freed score
            # banks) instead of the ~3us-latency DMA-transpose path
            scp_ctx.close()
            trp = ctx.enter_context(tc.tile_pool(name="trp", bufs=2,
                                                 space="PSUM"))
            sp = smp.tile([P, 4 * HL], F32, tag="sums")
            recips = rcp.tile([P, 4 * HL], F32, tag="recips")
            for nn2 in range(3):
                e_group(NB - 2, nn2, prev_ots[nn2])
            o7 = [d_group(NB - 1, nn2, pexps7, sp, recips, skip_t=True)
                  for nn2 in range(4)]
            e_group(NB - 2, 3, prev_ots[3], store_eng=nc.scalar)
            ots7 = []
            for nn2 in range(4):
                tr = trp.tile([P, DG], F8 if E_FP8 else MMDT, tag="tr")
                for c in range(4):
                    nc.tensor.transpose(tr[:, c * P:(c + 1) * P],
                                        o7[nn2][:, c * P:(c + 1) * P],
                                        id_mm[:])
                ot = otp.tile([P, DG], F8 if E_FP8 else MMDT, tag="ot",
                              name=f"otz{nn2}")
                nc.scalar.copy(out=ot[:], in_=tr[:])
                ots7.append(ot)
            for nn2 in range(4):
                e_group(NB - 1, nn2, ots7[nn2], last=True)
    nc.compile()
    return nc


def _np_mm(a):
    return np.ascontiguousarray(np.asarray(a), dtype=mybir.dt.np(MMDT))


def kernel(x, Wq, Wkv, Wproj, bproj, proj_k, proj_v):
    x = np.asarray(x)
    Wq, Wkv, Wproj = np.asarray(Wq), np.asarray(Wkv), np.asarray(Wproj)
    bproj, proj_k, proj_v = np.asarray(bproj), np.asarray(proj_k), np.asarray(proj_v)

    if "nc" not in _cache:
        _cache["nc"] = build_nc()
    nc = _cache["nc"]

    scale = np.float32(DH ** -0.5)
    projkv = _np_mm(np.concatenate([proj_k, proj_v], axis=1))
    in_maps = []
    for c in range(8):
        b, g = c // 2, c % 2
        cols = slice(g * DG, (g + 1) * DG)
        xb = _np_mm(x[b])
        in_maps.append({
            "x": xb,
            "xt": np.ascontiguousarray(xb.T),
            "projkv": projkv,
            "wq": _np_mm(scale * Wq[:, cols]),
            "wk": _np_mm(Wkv[:, :D][:, cols]),
            "wv": _np_mm(Wkv[:, D:][:, cols]),
            "wproj": (np.ascontiguousarray(
                64.0 * Wproj[cols, :], dtype=mybir.dt.np(F8))
                if E_FP8 else _np_mm(Wproj[cols, :])),
        })
    res = run_bass_kernel_spmd(nc, in_maps, list(range(8)),
                               trace=bool(os.environ.get("LINF_TRACE")))
    _cache["last_result"] = res
    oscale = np.float32(1.0 / 64.0) if E_FP8 else np.float32(1.0)
    outs = [oscale * np.asarray(r["out"], dtype=np.float32)
            for r in res.results]
    full = np.stack([outs[2 * b] + outs[2 * b + 1] for b in range(4)])
    full = full + np.asarray(bproj, np.float32)
    return full.astype(np.float32)


# revision 99
# speedup vs baseline: 1.3069x; 1.0003x over previous
"""Linformer self-attention on 8 Trainium2 NeuronCores.

Problem (hardcoded shapes): x [4,4096,1024] f32; per batch:
  q = scale*(x@Wq); kv = x@Wkv; keys/values compressed 4096->256 via
  proj_k/proj_v; 16-head attention (dh=64, k=256); out @ Wproj + bproj.

Sharding: 8 cores = 4 batches x 2 head-groups (8 heads / 512 cols each).
Each core computes a partial [4096,1024] output (Wproj row-split); host
sums the pair and adds bias.

Per-core dataflow (all matmuls use out = lhsT.T @ rhs, K<=128 partitions):
  A : xcxvT[1024,512] = x.T @ [proj_k|proj_v]      (contract n, x natural)
  A2: kprojT[512,256] = Wk_g.T @ xcT ; vproj[256,512] = xvT.T @ Wv_g
  B : qT[512,4096] = Wq_g.T @ xT    (xT provided by host, plain DMA)
  C : per (head,fc k-chunk): scoresT[128,512] -> exp (Act) -> pexp bf16
  S : per (n-chunk, head): sums[n,1] = pexp.T @ ones  (N=1 matmuls)
  D : po[n, 8*64] = pexp.T @ vproj_h per head; normalize via DVE
      tensor_tensor with per-head recip broadcast -> o bf16
  T : oT via one batched DMA transpose per [128,512] tile (PE transpose
      through the released score banks for the final block)
  E : out[n,1024] = oT.T-chunks @ Wproj_g, bf16 stores on SP HWDGE

B(nb+1), E(nb-1), D(nb) are hand-interleaved in the PE stream per
n-block so Act exp latency and PSUM-recycle chains hide under PE
matmuls; scores(7) are hoisted into iteration 6 and the last batch of
phase A runs dd-major so evictions overlap its tail.
"""

import os
import numpy as np

import concourse.bass as bass
import concourse.mybir as mybir
import concourse.tile as tile
from concourse import bacc
from concourse.bass_utils import run_bass_kernel_spmd

P = 128
N, D, K, DG, DH = 4096, 1024, 256, 512, 64
NB = 8                    # n-blocks of 512
HL = 8                    # heads per core
F32 = mybir.dt.float32

MMDT_NAME = os.environ.get("LINF_MMDT", "bfloat16")
MMDT = getattr(mybir.dt, MMDT_NAME)
Exp = mybir.ActivationFunctionType.Exp
# fp8(e4m3) attention-value path with DoubleRow matmuls (0.5 cycles/row)
D_FP8 = os.environ.get("LINF_D_FP8", "0") == "1"
# fp8 output-projection path: wproj scaled x64 on host (values are
# subnormal-small in fp8 otherwise), un-scaled after the host gather
E_FP8 = os.environ.get("LINF_E_FP8", "0") == "1"
F8 = mybir.dt.float8e4

_cache = {}


def build_nc():
    nc = bacc.Bacc(None, target_bir_lowering=False, debug=False)

    x_d = nc.dram_tensor("x", [N, D], MMDT, kind="ExternalInput")
    xt_d = nc.dram_tensor("xt", [D, N], MMDT, kind="ExternalInput")
    pkv_d = nc.dram_tensor("projkv", [N, 2 * K], MMDT, kind="ExternalInput")
    wq_d = nc.dram_tensor("wq", [D, DG], MMDT, kind="ExternalInput")
    wk_d = nc.dram_tensor("wk", [D, DG], MMDT, kind="ExternalInput")
    wv_d = nc.dram_tensor("wv", [D, DG], MMDT, kind="ExternalInput")
    wp_d = nc.dram_tensor("wproj", [DG, D], F8 if E_FP8 else MMDT,
                          kind="ExternalInput")
    out_d = nc.dram_tensor("out", [N, D], MMDT, kind="ExternalOutput")

    with tile.TileContext(nc) as tc:
        from contextlib import ExitStack
        with ExitStack() as ctx:
            res = ctx.enter_context(tc.tile_pool(name="res", bufs=1))
            ones_sb = res.tile([P, 1], MMDT, tag="ones")
            nc.vector.memset(ones_sb[:], 1.0)
            if D_FP8:
                ones2_sb = res.tile([P, 2], F8, tag="ones2")
                nc.vector.memset(ones2_sb[:], 1.0)
                nbias_sb = res.tile([P, 1], F32, tag="nbias")
                nc.vector.memset(nbias_sb[:], -1.5)
            from concourse.masks import make_identity
            id_mm = res.tile([P, P], MMDT, tag="id_mm")
            make_identity(nc, id_mm[:])

            wq_sb = res.tile([P, 8 * DG], MMDT, tag="wq")
            wk_sb = res.tile([P, 8 * DG], MMDT, tag="wk")
            wv_sb = res.tile([P, 8 * DG], MMDT, tag="wv")
            wproj_sb = res.tile([P, 4 * D], F8 if E_FP8 else MMDT,
                                tag="wproj")
            kprojT_sb = res.tile([P, 4 * K], MMDT, tag="kprojT")
            vproj_sb = res.tile([P, 2 * DG], F8 if D_FP8 else MMDT, tag="vproj")
            xcxv_sb = res.tile([P, 8 * 2 * K], MMDT, tag="xcxv")

            # rolling pools for the merged loop
            xtp = ctx.enter_context(tc.tile_pool(name="xtp", bufs=4))
            qtp = ctx.enter_context(tc.tile_pool(name="qtp", bufs=2))
            pexp_p = ctx.enter_context(tc.tile_pool(name="pexp", bufs=2))
            op_ = ctx.enter_context(tc.tile_pool(name="op", bufs=8))
            otp = ctx.enter_context(tc.tile_pool(name="otp", bufs=8))
            outp = ctx.enter_context(tc.tile_pool(name="outp", bufs=3))
            rcp = ctx.enter_context(tc.tile_pool(name="rcp", bufs=2))

            def load_w(dst, src, nchunk, w):
                # dst[p, c*w + j] = src[c*128 + p, j]
                nc.sync.dma_start(
                    out=dst[:].rearrange("p (c j) -> p c j", c=nchunk),
                    in_=src[:, :].rearrange("(c p) j -> p c j", p=P))

            def load_xt(nb, split=False):
                xt = xtp.tile([P, 8 * DG], MMDT, tag="xt", name=f"xt{nb}")
                nhalf = 2 if split else 1
                for hf in range(nhalf):
                    dph = 8 // nhalf
                    nc.sync.dma_start(
                        out=xt[:, hf * dph * DG:(hf + 1) * dph * DG]
                            .rearrange("p (d j) -> p d j", d=dph),
                        in_=xt_d[hf * dph * P:(hf + 1) * dph * P,
                                 nb * DG:(nb + 1) * DG]
                            .rearrange("(d p) j -> p d j", p=P))
                return xt

            def b_block(xt, qt, jc):
                accq = accp.tile([P, DG], F32, tag="acc")
                for dd in range(8):
                    nc.tensor.matmul(
                        accq[:],
                        lhsT=wq_sb[:, dd * DG + jc * P: dd * DG + (jc + 1) * P],
                        rhs=xt[:, dd * DG:(dd + 1) * DG],
                        start=(dd == 0), stop=(dd == 7))
                nc.vector.tensor_copy(qt[:, jc * DG:(jc + 1) * DG], accq[:])

            # ---------------- Phase A ----------------
            with ExitStack() as actx:
                xin = actx.enter_context(tc.tile_pool(name="xin", bufs=4))
                pa_ctx = ExitStack()
                pa = pa_ctx.enter_context(tc.tile_pool(name="pa", bufs=1, space="PSUM"))
                accs = [pa.tile([P, 2 * K], F32, tag=f"pa{dd}", name=f"pa{dd}")
                        for dd in range(8)]
                # PE p-state warm-up: keep the tensor engine continuously
                # busy from t~0.3us so the clock is fully ramped (needs 3us
                # of busy) by the time the first x chunk lands (~2.9us).
                # Output goes to a PSUM region that the first real
                # accumulation group resets (start=True).
                for _ in range(11):
                    nc.tensor.matmul(accs[0][:1, :P], lhsT=ones_sb[:],
                                     rhs=id_mm[:], start=True, stop=True)
                # First chunk loads alone (small, fast) so PE starts ASAP;
                # all weight/xt loads go after the 8 batches — the DMA
                # transfer path is a serial resource and phase A is tight.
                for b4 in range(8):
                    if b4 == 0:
                        # per-chunk interleaved loads: chunk c usable as soon
                        # as its own pair of small DMAs lands
                        x4 = xin.tile([P, 4 * D], MMDT, tag="x4")
                        kv4 = xin.tile([P, 4 * 2 * K], MMDT, tag="kv4")
                        # chunk 0 split by dd-slice: the first matmul only
                        # needs x[0:128, 0:128] + kv chunk 0
                        for dd in range(4):
                            nc.sync.dma_start(
                                out=x4[:, dd * 2 * P:(dd + 1) * 2 * P],
                                in_=x_d[0:P, dd * 2 * P:(dd + 1) * 2 * P])
                            if dd == 0:
                                nc.sync.dma_start(out=kv4[:, :2 * K],
                                                  in_=pkv_d[0:P, :])
                        for c in range(1, 4):
                            nc.sync.dma_start(out=x4[:, c * D:(c + 1) * D],
                                              in_=x_d[c * P:(c + 1) * P, :])
                            nc.sync.dma_start(
                                out=kv4[:, c * 2 * K:(c + 1) * 2 * K],
                                in_=pkv_d[c * P:(c + 1) * P, :])
                    elif b4 == 1:
                        # 2+2 split: first half arrives before the PE (still
                        # in p-state ramp) finishes batch 0
                        x4 = xin.tile([P, 4 * D], MMDT, tag="x4")
                        kv4 = xin.tile([P, 4 * 2 * K], MMDT, tag="kv4")
                        for hf in range(2):
                            r0 = 512 + hf * 256
                            nc.sync.dma_start(
                                out=x4[:, hf * 2 * D:(hf + 1) * 2 * D]
                                    .rearrange("p (c j) -> p c j", c=2),
                                in_=x_d[r0:r0 + 256, :]
                                    .rearrange("(c p) j -> p c j", p=P))
                            nc.sync.dma_start(
                                out=kv4[:, hf * 4 * K:(hf + 1) * 4 * K]
                                    .rearrange("p (c j) -> p c j", c=2),
                                in_=pkv_d[r0:r0 + 256, :]
                                    .rearrange("(c p) j -> p c j", p=P))
                    else:
                        x4 = xin.tile([P, 4 * D], MMDT, tag="x4")
                        kv4 = xin.tile([P, 4 * 2 * K], MMDT, tag="kv4")
                        nc.sync.dma_start(
                            out=x4[:].rearrange("p (c j) -> p c j", c=4),
                            in_=x_d[b4 * 512:(b4 + 1) * 512, :]
                                .rearrange("(c p) j -> p c j", p=P))
                        nc.sync.dma_start(
                            out=kv4[:].rearrange("p (c j) -> p c j", c=4),
                            in_=pkv_d[b4 * 512:(b4 + 1) * 512, :]
                                .rearrange("(c p) j -> p c j", p=P))
                    if b4 == 5:
                        # slot wq+xt0 here: the serialized DMA path has slack
                        # against the PE by now, and B(0) needs them right
                        # at the end of phase A
                        load_w(wq_sb, wq_d, 8, DG)
                        xt_tiles = {0: load_xt(0, split=True)}
                    if b4 == 7:
                        # dd-major on the last batch: acc[dd] stops after its
                        # 4 chunks, so evictions overlap the remaining matmuls
                        for dd in range(8):
                            for c in range(4):
                                nc.tensor.matmul(
                                    accs[dd][:],
                                    lhsT=x4[:, c * D + dd * P:
                                            c * D + (dd + 1) * P],
                                    rhs=kv4[:, c * 2 * K:(c + 1) * 2 * K],
                                    start=False, stop=(c == 3))
                            eng_v = dd % 2
                            if eng_v:
                                nc.vector.tensor_copy(
                                    xcxv_sb[:, dd * 2 * K:(dd + 1) * 2 * K],
                                    accs[dd][:])
                            else:
                                nc.scalar.copy(
                                    out=xcxv_sb[:, dd * 2 * K:(dd + 1) * 2 * K],
                                    in_=accs[dd][:])
                    else:
                        for c in range(4):
                            nn = b4 * 4 + c
                            for dd in range(8):
                                nc.tensor.matmul(
                                    accs[dd][:],
                                    lhsT=x4[:, c * D + dd * P:
                                            c * D + (dd + 1) * P],
                                    rhs=kv4[:, c * 2 * K:(c + 1) * 2 * K],
                                    start=(nn == 0), stop=False)
                # ordered by first use: wk/wv (A2), wproj (E(0)), xt1 (B(1))
                load_w(wk_sb, wk_d, 8, DG)
                load_w(wv_sb, wv_d, 8, DG)
                load_w(wproj_sb, wp_d, 4, D)
                xt_tiles[1] = load_xt(1)
                # Phase A2 — release the A accumulators' banks first.
                # B(0) blocks are interleaved with the A2 groups: they fill
                # the PE while A2 waits on the xcxv eviction pipeline.
                pa_ctx.close()
                pa2 = actx.enter_context(tc.tile_pool(name="pa2", bufs=2, space="PSUM"))
                accp = ctx.enter_context(tc.tile_pool(name="accp", bufs=3,
                                                      space="PSUM", side="right"))
                qts = {0: qtp.tile([P, 4 * DG], MMDT, tag="qt", name="qt0")}
                for jc in range(4):
                    # B(0) first: it has no dependence on the A evictions, so
                    # it covers the xcxv eviction pipeline latency
                    b_block(xt_tiles[0], qts[0], jc)
                    acc = pa2.tile([P, K], F32, tag="kpj")
                    for dd in range(8):
                        nc.tensor.matmul(
                            acc[:],
                            lhsT=wk_sb[:, dd * DG + jc * P: dd * DG + (jc + 1) * P],
                            rhs=xcxv_sb[:, dd * 2 * K: dd * 2 * K + K],
                            start=(dd == 0), stop=(dd == 7))
                    if jc % 2:
                        nc.scalar.copy(out=kprojT_sb[:, jc * K:(jc + 1) * K],
                                       in_=acc[:])
                    else:
                        nc.vector.tensor_copy(kprojT_sb[:, jc * K:(jc + 1) * K],
                                              acc[:])
                for fc in range(2):
                    acc2 = pa2.tile([P, DG], F32, tag="vpj")
                    for dd in range(8):
                        nc.tensor.matmul(
                            acc2[:],
                            lhsT=xcxv_sb[:, dd * 2 * K + K + fc * P:
                                         dd * 2 * K + K + (fc + 1) * P],
                            rhs=wv_sb[:, dd * DG:(dd + 1) * DG],
                            start=(dd == 0), stop=(dd == 7))
                    if fc:
                        nc.scalar.copy(out=vproj_sb[:, fc * DG:(fc + 1) * DG],
                                       in_=acc2[:])
                    else:
                        nc.vector.tensor_copy(vproj_sb[:, fc * DG:(fc + 1) * DG],
                                              acc2[:])

            # ---------------- merged loop pools (PSUM) ----------------
            # scp last on the left stack: it is released after the final
            # score block to make room for the epilogue transpose pool
            pop = ctx.enter_context(tc.tile_pool(name="pop", bufs=2, space="PSUM"))
            smp = ctx.enter_context(tc.tile_pool(name="smp", bufs=1, space="PSUM"))
            scp_ctx = ExitStack()
            scp = scp_ctx.enter_context(tc.tile_pool(name="scp", bufs=2,
                                                     space="PSUM"))

            def sc_block(qt, h, pexps):
                jc, p0 = h // 2, (h % 2) * DH
                if D_FP8:
                    pexps[h] = pexp_p.tile([P, 2 * DG], F8, tag=f"px{h}",
                                           name=f"px{h}")
                for fc in range(2):
                    st = scp.tile([P, DG], F32, tag="sc")
                    nc.tensor.matmul(
                        st[:],
                        lhsT=kprojT_sb[p0:p0 + DH,
                                       jc * K + fc * P: jc * K + (fc + 1) * P],
                        rhs=qt[p0:p0 + DH, jc * DG:(jc + 1) * DG],
                        start=True, stop=True)
                    if D_FP8:
                        # shifted exp keeps values well inside fp8e4 range;
                        # softmax is shift-invariant and the sums are computed
                        # from the same shifted values, so this is exact
                        nc.scalar.activation(
                            pexps[h][:, fc * DG:(fc + 1) * DG], st[:], Exp,
                            bias=nbias_sb[:])
                    else:
                        pexp = pexp_p.tile([P, DG], MMDT, tag=f"px{h}_{fc}")
                        nc.scalar.activation(pexp[:], st[:], Exp)
                        pexps[(h, fc)] = pexp

            def d_group(nb, nn2, pexps, sp, recips, skip_t=False):
                po = pop.tile([P, DG], F32, tag="po")
                for h in range(HL):
                    if D_FP8:
                        # fp8 DoubleRow: both k-chunks (fc) in one matmul at
                        # 0.5 cycles/row — out = sum_f lhsT[:,f].T @ rhs[:,f]
                        px3 = pexps[h][:].rearrange("p (f n) -> p f n", f=2)
                        v3 = vproj_sb[:].rearrange("p (f c) -> p f c", f=2)
                        nc.tensor.matmul(
                            po[:, h * DH:(h + 1) * DH],
                            lhsT=px3[:, :, nn2 * P:(nn2 + 1) * P],
                            rhs=v3[:, :, h * DH:(h + 1) * DH],
                            start=True, stop=True,
                            perf_mode=mybir.MatmulPerfMode.DoubleRow)
                        nc.tensor.matmul(
                            sp[:, nn2 * HL + h: nn2 * HL + h + 1],
                            lhsT=px3[:, :, nn2 * P:(nn2 + 1) * P],
                            rhs=ones2_sb[:].rearrange("p (f o) -> p f o", f=2),
                            start=True, stop=True,
                            perf_mode=mybir.MatmulPerfMode.DoubleRow)
                        continue
                    for fc in range(2):
                        px = pexps[(h, fc)]
                        nc.tensor.matmul(
                            po[:, h * DH:(h + 1) * DH],
                            lhsT=px[:, nn2 * P:(nn2 + 1) * P],
                            rhs=vproj_sb[:, fc * DG + h * DH:
                                         fc * DG + (h + 1) * DH],
                            start=(fc == 0), stop=(fc == 1))
                        nc.tensor.matmul(
                            sp[:, nn2 * HL + h: nn2 * HL + h + 1],
                            lhsT=px[:, nn2 * P:(nn2 + 1) * P],
                            rhs=ones_sb[:],
                            start=(fc == 0), stop=(fc == 1))
                nc.vector.reciprocal(
                    recips[:, nn2 * HL:(nn2 + 1) * HL],
                    sp[:, nn2 * HL:(nn2 + 1) * HL])
                o_t = op_.tile([P, DG], F8 if E_FP8 else MMDT, tag="o",
                               name=f"o{nb}_{nn2}")
                nc.vector.tensor_tensor(
                    out=o_t[:].rearrange("p (h j) -> p h j", h=HL),
                    in0=po[:].rearrange("p (h j) -> p h j", h=HL),
                    in1=recips[:, nn2 * HL:(nn2 + 1) * HL]
                        .broadcast_to([P, HL, DH]),
                    op=mybir.AluOpType.mult)
                if skip_t:
                    return o_t
                if E_FP8:
                    # PE transpose (1-byte dtypes can't use the DMA-transpose
                    # path); the psum target reuses a score-pool slot via
                    # bitcast, so no extra PSUM banks are needed
                    tr = scp.tile([P, DG], F32, tag="sc", name=f"tr{nb}_{nn2}")
                    tr8 = tr[:].bitcast(F8)
                    ot = otp.tile([P, DG], F8, tag="ot", name=f"ot{nb}_{nn2}")
                    for c in range(4):
                        nc.tensor.transpose(tr8[:, c * P:(c + 1) * P],
                                            o_t[:, c * P:(c + 1) * P],
                                            id_mm[:])
                    if nn2 % 2:
                        nc.scalar.copy(out=ot[:], in_=tr8[:, :DG])
                    else:
                        nc.vector.tensor_copy(ot[:], tr8[:, :DG])
                    return ot
                ot = otp.tile([P, DG], MMDT, tag="ot", name=f"ot{nb}_{nn2}")
                nc.sync.dma_start_transpose(
                    out=ot[:].rearrange("p (c j) -> p c j", c=4),
                    in_=o_t[:])
                return ot

            def e_group(nb, nn2, ot, last=False, store_eng=None):
                ci = nb * 4 + nn2
                outsb = outp.tile([P, D], MMDT, tag="outsb")
                ot3 = ot[:].rearrange("p (c n) -> p c n", c=4)
                wp3 = wproj_sb[:].rearrange("p (c d) -> p c d", c=4)
                for half in range(2):
                    pe_acc = accp.tile([P, DG], F32, tag="acc")
                    if E_FP8:
                        for pr in range(2):
                            nc.tensor.matmul(
                                pe_acc[:],
                                lhsT=ot3[:, 2 * pr:2 * pr + 2, :],
                                rhs=wp3[:, 2 * pr:2 * pr + 2,
                                        half * DG:(half + 1) * DG],
                                start=(pr == 0), stop=(pr == 1),
                                perf_mode=mybir.MatmulPerfMode.DoubleRow)
                    else:
                        for jc2 in range(4):
                            nc.tensor.matmul(
                                pe_acc[:],
                                lhsT=ot[:, jc2 * P:(jc2 + 1) * P],
                                rhs=wproj_sb[:, jc2 * D + half * DG:
                                             jc2 * D + (half + 1) * DG],
                                start=(jc2 == 0), stop=(jc2 == 3))
                    if last:
                        # fast tail: evict on both engines, store each half as
                        # soon as it lands (HWDGE has lower fixed latency)
                        if half == 0:
                            nc.scalar.copy(out=outsb[:, :DG], in_=pe_acc[:])
                        else:
                            nc.vector.tensor_copy(outsb[:, DG:], pe_acc[:])
                        nc.sync.dma_start(
                            out=out_d[ci * P:(ci + 1) * P,
                                      half * DG:(half + 1) * DG],
                            in_=outsb[:, half * DG:(half + 1) * DG])
                    else:
                        nc.vector.tensor_copy(
                            outsb[:, half * DG:(half + 1) * DG], pe_acc[:])
                if not last:
                    (store_eng or nc.sync).dma_start(
                        out=out_d[ci * P:(ci + 1) * P, :], in_=outsb[:])

            # ---------------- merged loop (nb = 0..6) ----------------
            xt_tiles[2] = load_xt(2)
            prev_ots = None
            pexps7 = {}
            for nb in range(NB - 1):
                if nb + 3 < NB:
                    xt_tiles[nb + 3] = load_xt(nb + 3)
                pexps = {}
                cur_ots = []
                sp = smp.tile([P, 4 * HL], F32, tag="sums")
                recips = rcp.tile([P, 4 * HL], F32, tag="recips")
                qts[nb + 1] = qtp.tile([P, 4 * DG], MMDT, tag="qt",
                                       name=f"qt{nb + 1}")
                # interleave scores(nb) with B(nb+1) on the PE stream
                for h in range(HL):
                    sc_block(qts[nb], h, pexps)
                    if h % 2 == 1:
                        b_block(xt_tiles[nb + 1], qts[nb + 1], h // 2)
                if prev_ots is not None:
                    for nn2 in range(4):
                        e_group(nb - 1, nn2, prev_ots[nn2])
                if nb == NB - 2:
                    # hoist scores(7) into this iter's tail, interleaved with
                    # D(6) so the PE covers the po-recycle chains and the Act
                    # queue drains the extra exps early
                    for nn2 in range(4):
                        cur_ots.append(d_group(nb, nn2, pexps, sp, recips))
                        sc_block(qts[NB - 1], 2 * nn2, pexps7)
                        sc_block(qts[NB - 1], 2 * nn2 + 1, pexps7)
                else:
                    for nn2 in range(4):
                        cur_ots.append(d_group(nb, nn2, pexps, sp, recips))
                prev_ots = cur_ots
            # ---- last iter: E(6,3) placed after D(7) to cover latency;
            # o-transposes for block 7 run on the PE (via the freed score
            # banks) instead of the ~3us-latency DMA-transpose path
            scp_ctx.close()
            trp = ctx.enter_context(tc.tile_pool(name="trp", bufs=2,
                                                 space="PSUM"))
            sp = smp.tile([P, 4 * HL], F32, tag="sums")
            recips = rcp.tile([P, 4 * HL], F32, tag="recips")
            for nn2 in range(3):
                e_group(NB - 2, nn2, prev_ots[nn2])
            o7 = [d_group(NB - 1, nn2, pexps7, sp, recips, skip_t=True)
                  for nn2 in range(4)]
            e_group(NB - 2, 3, prev_ots[3], store_eng=nc.scalar)
            ots7 = []
            for nn2 in range(4):
                tr = trp.tile([P, DG], F8 if E_FP8 else MMDT, tag="tr")
                for c in range(4):
                    nc.tensor.transpose(tr[:, c * P:(c + 1) * P],
                                        o7[nn2][:, c * P:(c + 1) * P],
                                        id_mm[:])
                ot = otp.tile([P, DG], F8 if E_FP8 else MMDT, tag="ot",
                              name=f"otz{nn2}")
                nc.scalar.copy(out=ot[:], in_=tr[:])
                ots7.append(ot)
            for nn2 in range(4):
                e_group(NB - 1, nn2, ots7[nn2], last=True)
    nc.compile()
    return nc


def _np_mm(a):
    return np.ascontiguousarray(np.asarray(a), dtype=mybir.dt.np(MMDT))


def kernel(x, Wq, Wkv, Wproj, bproj, proj_k, proj_v):
    x = np.asarray(x)
    Wq, Wkv, Wproj = np.asarray(Wq), np.asarray(Wkv), np.asarray(Wproj)
    bproj, proj_k, proj_v = np.asarray(bproj), np.asarray(proj_k), np.asarray(proj_v)

    if "nc" not in _cache:
        _cache["nc"] = build_nc()
    nc = _cache["nc"]

    scale = np.float32(DH ** -0.5)
    projkv = _np_mm(np.concatenate([proj_k, proj_v], axis=1))
    in_maps = []
    for c in range(8):
        b, g = c // 2, c % 2
        cols = slice(g * DG, (g + 1) * DG)
        xb = _np_mm(x[b])
        in_maps.append({
            "x": xb,
            "xt": np.ascontiguousarray(xb.T),
            "projkv": projkv,
            "wq": _np_mm(scale * Wq[:, cols]),
            "wk": _np_mm(Wkv[:, :D][:, cols]),
            "wv": _np_mm(Wkv[:, D:][:, cols]),
            "wproj": (np.ascontiguousarray(
                64.0 * Wproj[cols, :], dtype=mybir.dt.np(F8))
                if E_FP8 else _np_mm(Wproj[cols, :])),
        })
    res = run_bass_kernel_spmd(nc, in_maps, list(range(8)),
                               trace=bool(os.environ.get("LINF_TRACE")))
    _cache["last_result"] = res
    oscale = np.float32(1.0 / 64.0) if E_FP8 else np.float32(1.0)
    outs = [oscale * np.asarray(r["out"], dtype=np.float32)
            for r in res.results]
    full = np.stack([outs[2 * b] + outs[2 * b + 1] for b in range(4)])
    full = full + np.asarray(bproj, np.float32)
    return full.astype(np.float32)
